# revision 1
# baseline (speedup 1.0000x reference)
"""Ernie4 decoder layer (RMSNorm + GQA attention + shared expert + 16-expert
top-2 MoE) on 8 Trainium2 NeuronCores.

Sharding:
  - Attention: head-parallel (2 q-heads + 1 kv-head per core); o_proj partials
    combined with a ReduceScatter (token-sharded result).
  - Shared expert: intermediate-sharded (IS/8 per core); its partial is the
    initializer of the MoE combine buffer (summed in the second ReduceScatter).
  - MoE: expert-parallel (2 experts per core); token lists built on device
    (top-2 via running-max, compaction via triangular-matmul prefix sums),
    token gather via indirect DMA from the AllGathered activations, combine
    via indirect scatter-add into the ReduceScatter input.
"""
import sys
sys.path.insert(0, "/opt/trn_rl_repo")

import numpy as np

import concourse.bass as bass
import concourse.bacc as bacc
import concourse.tile as tile
import concourse.mybir as mybir
from concourse import bass_utils
from concourse.masks import make_identity, make_upper_triangular

dt = mybir.dt
F32 = dt.float32
F32R = dt.float32r
I32 = dt.int32
BF16 = dt.bfloat16
AF = mybir.ActivationFunctionType
ALU = mybir.AluOpType
AX = mybir.AxisListType

T, H, NH, NKV, D = 1024, 2048, 16, 4, 128
E, I, IS = 16, 1024, 2048
EPS = 1e-6
THETA = 10000.0
NCN = 8
P = 128
TB = T // P            # 8 token blocks
HC = H // P            # 16 hidden chunks
IP = I // P            # 8 expert-intermediate chunks
SP = IS // NCN // P    # 2 shared-intermediate chunks per core
CAP = 256              # per-expert token capacity
BIG = 1.0e6  # OOB sentinel; sentinel*2048 must stay < 2^31 (sim int32 math)
NEG = -1e9
SIM = False          # CoreSim lacks Silu; emit sigmoid*x instead


def _silu(nc, pool, out_ap, in_ap, shape):
    if not SIM:
        nc.scalar.activation(out_ap, in_ap, AF.Silu)
    else:
        tmp = pool.tile(shape, F32, tag="silu_tmp", name="silu_tmp")
        nc.scalar.activation(tmp[:], in_ap, AF.Sigmoid)
        nc.vector.tensor_mul(out_ap, tmp[:], in_ap)


def _emit(nc, tc):
    ex = {}
    for name, shape, d in [
        ("hid", [T, H], F32), ("hid_slice", [P, H], F32),
        ("wq_s", [H, 2 * D], F32), ("wk_s", [H, D], F32), ("wv_s", [H, D], F32),
        ("wo_s", [2 * D, H], F32),
        ("cosq", [D, T], F32), ("sinq", [D, T], F32),
        ("cosk", [D, T], F32), ("sink", [D, T], F32),
        ("perm", [P, P], F32),
        ("masks", [T, T], BF16),
        ("gate_wT", [H, E], F32), ("gate_b", [P, E], F32),
        ("ws_g", [H, SP * P], F32), ("ws_u", [H, SP * P], F32),
        ("ws_d", [SP * P, H], F32),
        ("we_g", [2, H, I], BF16), ("we_u", [2, H, I], BF16),
        ("we_d", [2, I, H], BF16),
        ("identr_in", [P, P], F32), ("ut_in", [P, P], F32),
        ("identb_in", [P, P], BF16),
        ("slb_in", [8, TB * P], F32),
    ]:
        ex[name] = nc.dram_tensor(name, shape, d, kind="ExternalInput").ap()
    out_slice = nc.dram_tensor("out_slice", [P, H], F32, kind="ExternalOutput").ap()
    res_slice = nc.dram_tensor("res_slice", [P, H], F32, kind="ExternalOutput").ap()
    dbg_x = nc.dram_tensor("dbg_x", [P, H], F32, kind="ExternalOutput").ap()
    dbg_w = nc.dram_tensor("dbg_w", [T, E], F32, kind="ExternalOutput").ap()
    dbg_tok = nc.dram_tensor("dbg_tok", [2 * CAP, 1], I32,
                             kind="ExternalOutput").ap()
    dbg_rank = nc.dram_tensor("dbg_rank", [T, E], F32,
                              kind="ExternalOutput").ap()

    with tc.tile_pool(name="persist", bufs=1) as pp, \
         tc.tile_pool(name="dram", bufs=1, space="DRAM") as dram:
        rs_in = dram.tile([T, H], F32)
        rs_out = dram.tile([P, H], F32)
        ag_tm = dram.tile([P, H], BF16)
        x_tm = dram.tile([T, H], BF16)
        ag_tr = dram.tile([H, P], F32)
        xT_blocks = dram.tile([NCN * H, P], F32)
        W_dram = dram.tile([T, E], F32)
        tok_lists = dram.tile([2 * CAP, 1], I32)
        rs2_in = dram.tile([T, H], F32)
        rs2_out = dram.tile([P, H], F32)

        ident = pp.tile([P, P], F32)
        make_identity(nc, ident[:])
        identr = pp.tile([P, P], F32R)
        nc.sync.dma_start(identr[:], ex["identr_in"][:].bitcast(F32R))
        identb = pp.tile([P, P], BF16)
        nc.sync.dma_start(identb[:], ex["identb_in"][:])
        hid_sl = pp.tile([P, H], F32)
        nc.sync.dma_start(hid_sl[:], ex["hid_slice"][:])
        eps_t = pp.tile([P, 1], F32)
        nc.vector.memset(eps_t[:], EPS)

        # ======== Phases A-C: attention ========
        with tc.tile_pool(name="pab", bufs=1) as pab:
            qT = [pab.tile([P, T], F32R, tag=f"qT{j}", name=f"qT{j}") for j in range(2)]
            kT = pab.tile([P, T], F32R)
            vT = pab.tile([P, T], F32R)
            v_tm = [pab.tile([P, D], F32R, tag=f"vtm{b}", name=f"vtm{b}") for b in range(TB)]
            oT = [pab.tile([P, T], F32R, tag=f"oT{j}", name=f"oT{j}") for j in range(2)]

            # ---- A: norm + transpose + QKV + rope ----
            with tc.tile_pool(name="pa", bufs=1) as pa, \
                 tc.tile_pool(name="pa2", bufs=3) as pa2:
                cosq = pa.tile([D, T], F32)
                sinq = pa.tile([D, T], F32)
                cosk = pa.tile([D, T], F32)
                sink = pa.tile([D, T], F32)
                for t_, s_ in [(cosq, "cosq"), (sinq, "sinq"),
                               (cosk, "cosk"), (sink, "sink")]:
                    nc.sync.dma_start(t_[:], ex[s_][:])
                permr = pa.tile([P, P], F32R)
                nc.sync.dma_start(permr[:], ex["perm"][:].bitcast(F32R))
                wq_sb = pa.tile([P, HC * 2 * D], F32R)
                wk_sb = pa.tile([P, HC * D], F32R)
                wv_sb = pa.tile([P, HC * D], F32R)
                for t_, s_, m in [(wq_sb, "wq_s", 2 * D), (wk_sb, "wk_s", D),
                                  (wv_sb, "wv_s", D)]:
                    nc.sync.dma_start(
                        t_[:].rearrange("p (hc m) -> p hc m", hc=HC),
                        ex[s_][:].bitcast(F32R)
                        .rearrange("(hc p) m -> p hc m", p=P))

                dump = pa.tile([P, H], F32)
                qraw = [pa.tile([P, T], F32R, tag=f"qraw{j}", name=f"qraw{j}") for j in range(2)]
                kraw = pa.tile([P, T], F32R)
                with tc.tile_pool(name="psA1", bufs=2, space="PSUM") as psA1, \
                     tc.tile_pool(name="psA2", bufs=2, space="PSUM") as psA2:
                    for n in range(2):
                        x0T = [pa.tile([P, 512], F32R, tag=f"x0T{hc}",
                                       name=f"x0T{hc}_{n}") for hc in range(HC)]
                        for bb in range(TB // 2):
                            b = n * (TB // 2) + bb
                            hidb = pa2.tile([P, H], F32, tag="hidb", bufs=2)
                            nc.sync.dma_start(hidb[:],
                                              ex["hid"][b * P:(b + 1) * P, :])
                            ssum = pa2.tile([P, 1], F32, tag="ssum")
                            nc.scalar.activation(dump[:], hidb[:], AF.Square,
                                                 accum_out=ssum[:, :1])
                            rms = pa2.tile([P, 1], F32, tag="rms")
                            nc.scalar.activation(rms[:], ssum[:],
                                                 AF.Sqrt, bias=eps_t[:, :1],
                                                 scale=1.0 / H)
                            inv = pa2.tile([P, 1], F32, tag="inv")
                            nc.vector.reciprocal(inv[:], rms[:])
                            x0b = pa2.tile([P, H], F32R, tag="x0b", bufs=2)
                            nc.vector.tensor_scalar_mul(x0b[:], hidb[:],
                                                        inv[:, :1])
                            for hc in range(HC):
                                tp = psA1.tile([P, P], F32R, tag="tpA")
                                nc.tensor.transpose(
                                    tp[:], x0b[:, hc * P:(hc + 1) * P],
                                    identr[:])
                                nc.vector.tensor_copy(
                                    x0T[hc][:, bb * P:(bb + 1) * P], tp[:])

                        def proj(w_sb, m, c0, dst, n=n, x0T=x0T):
                            ps = psA2.tile([P, 512], F32, tag="psQKV",
                                           name="psQKV")
                            for hc in range(HC):
                                nc.tensor.matmul(
                                    ps[:],
                                    w_sb[:, hc * m + c0:hc * m + c0 + P],
                                    x0T[hc][:],
                                    start=(hc == 0), stop=(hc == HC - 1))
                            nc.vector.tensor_copy(
                                dst[:, n * 512:(n + 1) * 512], ps[:])
                        proj(wq_sb, 2 * D, 0, qraw[0])
                        proj(wq_sb, 2 * D, D, qraw[1])
                        proj(wk_sb, D, 0, kraw)
                        proj(wv_sb, D, 0, vT)

                with tc.tile_pool(name="psA3", bufs=2, space="PSUM") as psA3:
                    for src, dst, c_, s_ in [(qraw[0], qT[0], cosq, sinq),
                                             (qraw[1], qT[1], cosq, sinq),
                                             (kraw, kT, cosk, sink)]:
                        for n in range(2):
                            sl = slice(n * 512, (n + 1) * 512)
                            sw = psA3.tile([P, 512], F32, tag="psSW")
                            nc.tensor.matmul(sw[:], permr[:], src[:, sl],
                                             start=True, stop=True)
                            t1 = pa2.tile([P, 512], F32, tag="ropeT1")
                            nc.vector.tensor_mul(t1[:], src[:, sl], c_[:, sl])
                            t2 = pa2.tile([P, 512], F32, tag="ropeT2")
                            nc.vector.tensor_mul(t2[:], sw[:], s_[:, sl])
                            nc.vector.tensor_add(dst[:, sl], t1[:], t2[:])
                    for b in range(TB):
                        tp = psA3.tile([P, P], F32R, tag="tpV")
                        nc.tensor.transpose(tp[:], vT[:, b * P:(b + 1) * P],
                                            identr[:])
                        nc.vector.tensor_copy(v_tm[b][:], tp[:])

            # ---- B: attention + o_proj ----
            with tc.tile_pool(name="pb", bufs=1) as pb, \
                 tc.tile_pool(name="pb2", bufs=3) as pb2, \
                 tc.tile_pool(name="psB", bufs=2, space="PSUM") as psB:
                mask_sb = [pb.tile([P, T], BF16, tag=f"mask{qc}", name=f"mask{qc}")
                           for qc in range(TB)]
                for qc in range(TB):
                    nc.sync.dma_start(mask_sb[qc][:],
                                      ex["masks"][qc * P:(qc + 1) * P, :])
                wo_sb = [pb.tile([P, H], F32R, tag=f"wo{j}", name=f"wo{j}") for j in range(2)]
                nc.sync.dma_start(wo_sb[0][:], ex["wo_s"][0:P, :].bitcast(F32R))
                nc.sync.dma_start(wo_sb[1][:],
                                  ex["wo_s"][P:2 * P, :].bitcast(F32R))

                attnT = [pb.tile([P, T], F32R, tag=f"attnT{kc}", name=f"attnT{kc}")
                         for kc in range(TB)]
                for h in range(2):
                    for qc in range(TB):
                        prob = pb2.tile([P, T], F32, tag="prob")
                        for n in range(2):
                            sl = slice(n * 512, (n + 1) * 512)
                            ps = psB.tile([P, 512], F32, tag="psSC")
                            nc.tensor.matmul(ps[:],
                                             qT[h][:, qc * P:(qc + 1) * P],
                                             kT[:, sl], start=True, stop=True)
                            nc.vector.tensor_add(prob[:, sl], ps[:],
                                                 mask_sb[qc][:, sl])
                        mx = pb2.tile([P, 1], F32, tag="mx")
                        nc.vector.reduce_max(mx[:], prob[:], axis=AX.X)
                        negm = pb2.tile([P, 1], F32, tag="negm")
                        nc.vector.tensor_scalar_mul(negm[:], mx[:], -1.0)
                        ssum = pb2.tile([P, 1], F32, tag="esum")
                        probe_ = pb2.tile([P, T], F32, tag="probe")
                        nc.scalar.activation(probe_[:], prob[:], AF.Exp,
                                             bias=negm[:, :1],
                                             accum_out=ssum[:, :1])
                        rec = pb2.tile([P, 1], F32, tag="rec")
                        nc.vector.reciprocal(rec[:], ssum[:])
                        nc.vector.tensor_scalar_mul(probe_[:], probe_[:],
                                                    rec[:, :1])
                        for kc in range(TB):
                            tp = psB.tile([P, P], F32, tag="tpB")
                            nc.tensor.transpose(
                                tp[:], probe_[:, kc * P:(kc + 1) * P], ident[:])
                            nc.vector.tensor_copy(
                                attnT[kc][:, qc * P:(qc + 1) * P], tp[:])
                    for n in range(2):
                        sl = slice(n * 512, (n + 1) * 512)
                        ps = psB.tile([P, 512], F32, tag="psAV")
                        for kc in range(TB):
                            nc.tensor.matmul(ps[:], v_tm[kc][:],
                                             attnT[kc][:, sl],
                                             start=(kc == 0), stop=(kc == TB - 1))
                        nc.vector.tensor_copy(oT[h][:, sl], ps[:])

                for tb_ in range(TB):
                    for n in range(4):
                        sl = slice(n * 512, (n + 1) * 512)
                        ps = psB.tile([P, 512], F32, tag="psO")
                        for hp in range(2):
                            nc.tensor.matmul(ps[:],
                                             oT[hp][:, tb_ * P:(tb_ + 1) * P],
                                             wo_sb[hp][:, sl],
                                             start=(hp == 0), stop=(hp == 1))
                        ob = pb2.tile([P, 512], F32, tag="ob")
                        nc.vector.tensor_copy(ob[:], ps[:])
                        nc.sync.dma_start(rs_in[tb_ * P:(tb_ + 1) * P, sl], ob[:])

        nc.gpsimd.collective_compute(
            "ReduceScatter", ALU.add, ins=[rs_in.opt()], outs=[rs_out.opt()],
            replica_groups=[list(range(NCN))])

        # ======== D: residual + norm + AGs ========
        with tc.tile_pool(name="pd", bufs=1) as pd, \
             tc.tile_pool(name="psD", bufs=2, space="PSUM") as psD:
            attn_sl = pd.tile([P, H], F32)
            nc.sync.dma_start(attn_sl[:], rs_out[:])
            res_sb = pd.tile([P, H], F32)
            nc.vector.tensor_add(res_sb[:], hid_sl[:], attn_sl[:])
            nc.sync.dma_start(res_slice[:], res_sb[:])
            dump2 = pd.tile([P, H], F32)
            ssum = pd.tile([P, 1], F32)
            nc.scalar.activation(dump2[:], res_sb[:], AF.Square,
                                 accum_out=ssum[:, :1])
            rms = pd.tile([P, 1], F32)
            nc.scalar.activation(rms[:], ssum[:], AF.Sqrt, bias=eps_t[:, :1],
                                 scale=1.0 / H)
            inv = pd.tile([P, 1], F32)
            nc.vector.reciprocal(inv[:], rms[:])
            x_sl = pd.tile([P, H], F32)
            nc.vector.tensor_scalar_mul(x_sl[:], res_sb[:], inv[:, :1])
            x_sl_b = pd.tile([P, H], BF16)
            nc.vector.tensor_copy(x_sl_b[:], x_sl[:])
            nc.sync.dma_start(ag_tm[:], x_sl_b[:])
            nc.sync.dma_start(dbg_x[:], x_sl[:])
            x_slT = pd.tile([P, H], F32)
            for hc in range(HC):
                tp = psD.tile([P, P], F32, tag="tpD")
                nc.tensor.transpose(tp[:], x_sl[:, hc * P:(hc + 1) * P],
                                    ident[:])
                nc.vector.tensor_copy(x_slT[:, hc * P:(hc + 1) * P], tp[:])
            nc.sync.dma_start(
                ag_tr[:].rearrange("(hc p) t -> p hc t", p=P),
                x_slT[:].rearrange("p (hc t) -> p hc t", hc=HC))

        nc.gpsimd.collective_compute(
            "AllGather", ALU.bypass, ins=[ag_tm.opt()], outs=[x_tm.opt()],
            replica_groups=[list(range(NCN))])
        nc.gpsimd.collective_compute(
            "AllGather", ALU.bypass, ins=[ag_tr.opt()], outs=[xT_blocks.opt()],
            replica_groups=[list(range(NCN))])

        # chunk hc of full x^T as [P, T] (free = global token index)
        xT_r = xT_blocks[:].bitcast(F32R).rearrange(
            "(b hc p) t -> hc p b t", b=NCN, p=P)

        def load_xT_chunk(pool, hc, tag):
            xc = pool.tile([P, T], F32R, tag=tag)
            nc.sync.dma_start(xc[:].rearrange("p (b t) -> p b t", b=NCN),
                              xT_r[hc])
            return xc

        # ======== EG: router + shared expert ========
        with tc.tile_pool(name="pe", bufs=1) as pe, \
             tc.tile_pool(name="pe2", bufs=3) as pe2:
            gw_sb = pe.tile([P, HC * E], F32R)
            nc.sync.dma_start(
                gw_sb[:].rearrange("p (hc e) -> p hc e", hc=HC),
                ex["gate_wT"][:].bitcast(F32R)
                .rearrange("(hc p) e -> p hc e", p=P))
            gate_b = pe.tile([P, E], F32)
            nc.sync.dma_start(gate_b[:], ex["gate_b"][:])
            ut = pe.tile([P, P], F32R)
            nc.sync.dma_start(ut[:], ex["ut_in"][:].bitcast(F32R))
            # slb[b', b*P+j] = 1 if b' < b  (prefix-broadcast operator)
            slb = pe.tile([8, TB * P], F32R)
            nc.sync.dma_start(slb[:], ex["slb_in"][:].bitcast(F32R))
            wsg_sb = pe.tile([P, HC * SP * P], F32R)
            wsu_sb = pe.tile([P, HC * SP * P], F32R)
            for t_, s_ in [(wsg_sb, "ws_g"), (wsu_sb, "ws_u")]:
                nc.sync.dma_start(
                    t_[:].rearrange("p (hc m) -> p hc m", hc=HC),
                    ex[s_][:].bitcast(F32R).rearrange("(hc p) m -> p hc m", p=P))
            wsd_sb = [pe.tile([P, H], F32R, tag=f"wsd{sp}", name=f"wsd{sp}") for sp in range(SP)]
            for sp in range(SP):
                nc.sync.dma_start(
                    wsd_sb[sp][:], ex["ws_d"][sp * P:(sp + 1) * P, :].bitcast(F32R))

            lg_sb = pe.tile([16, T], F32)
            g_act = [pe.tile([P, T], F32R, tag=f"gact{sp}", name=f"gact{sp}") for sp in range(SP)]
            hs = [pe.tile([P, T], F32R, tag=f"hs{sp}", name=f"hs{sp}") for sp in range(SP)]

            # pass 1: logits + shared gate
            with tc.tile_pool(name="psE1", bufs=1, space="PSUM") as psE1:
                lg_ps = psE1.tile([16, T], F32, tag="lgps", name="lgps")
                g_ps = [psE1.tile([P, T], F32, tag=f"gps{sp}", name=f"gps{sp}")
                        for sp in range(SP)]
                for hc in range(HC):
                    xc = load_xT_chunk(pe2, hc, "xcE1")
                    for n in range(2):
                        sl = slice(n * 512, (n + 1) * 512)
                        nc.tensor.matmul(lg_ps[:, sl],
                                         gw_sb[:, hc * E:(hc + 1) * E],
                                         xc[:, sl],
                                         start=(hc == 0), stop=(hc == HC - 1))
                        for sp in range(SP):
                            c0 = hc * SP * P + sp * P
                            nc.tensor.matmul(g_ps[sp][:, sl],
                                             wsg_sb[:, c0:c0 + P], xc[:, sl],
                                             start=(hc == 0),
                                             stop=(hc == HC - 1))
                nc.vector.tensor_copy(lg_sb[:], lg_ps[:])
                for sp in range(SP):
                    _silu(nc, pe2, g_act[sp][:], g_ps[sp][:], [P, T])

            # router small ops
            sel = [pe.tile([P, E], F32R, tag=f"sel{b}", name=f"sel{b}") for b in range(TB)]
            pre_sb = [pe.tile([P, E], F32, tag=f"pre{b}", name=f"pre{b}") for b in range(TB)]
            grank = [pe.tile([P, E], F32, tag=f"grank{b}", name=f"grank{b}") for b in range(TB)]
            totals = pe.tile([8, E], F32R)
            with tc.tile_pool(name="psE2", bufs=2, space="PSUM") as psE2:
                for b in range(TB):
                    lt_ps = psE2.tile([P, 16], F32, tag="ltps")
                    nc.tensor.transpose(lt_ps[:], lg_sb[:, b * P:(b + 1) * P],
                                        ident[:16, :16])
                    sig = pe2.tile([P, E], F32, tag="sig")
                    nc.scalar.activation(sig[:], lt_ps[:], AF.Sigmoid)
                    sb_ = pe2.tile([P, E], F32, tag="sb_")
                    nc.vector.tensor_add(sb_[:], sig[:], gate_b[:])
                    mx = pe2.tile([P, 8], F32, tag="mx8")
                    nc.vector.max(out=mx[:], in_=sb_[:])
                    s1 = pe2.tile([P, E], F32, tag="s1")
                    nc.vector.tensor_tensor(out=s1[:], in0=sb_[:],
                                            in1=mx[:, 0:1].to_broadcast([P, E]),
                                            op=ALU.is_equal)
                    s2 = pe2.tile([P, E], F32, tag="s2")
                    nc.vector.tensor_tensor(out=s2[:], in0=sb_[:],
                                            in1=mx[:, 1:2].to_broadcast([P, E]),
                                            op=ALU.is_equal)
                    nc.vector.tensor_add(s1[:], s1[:], s2[:])
                    nc.vector.tensor_scalar_min(sel[b][:], s1[:], 1.0)
                    wa = pe2.tile([P, E], F32, tag="wa")
                    nc.vector.tensor_mul(wa[:], sel[b][:], sig[:])
                    nrm = pe2.tile([P, 1], F32, tag="nrm")
                    nc.vector.reduce_sum(nrm[:], wa[:], axis=AX.X)
                    rec = pe2.tile([P, 1], F32, tag="recw")
                    nc.vector.reciprocal(rec[:], nrm[:])
                    w_tm = pe2.tile([P, E], F32, tag="wtm")
                    nc.vector.tensor_scalar_mul(w_tm[:], wa[:], rec[:, :1])
                    nc.sync.dma_start(W_dram[b * P:(b + 1) * P, :], w_tm[:])
                    nc.sync.dma_start(dbg_w[b * P:(b + 1) * P, :], w_tm[:])
                    pr_ps = psE2.tile([P, E], F32, tag="prps")
                    nc.tensor.matmul(pr_ps[:], ut[:], sel[b][:],
                                     start=True, stop=True)
                    nc.vector.tensor_copy(pre_sb[b][:], pr_ps[:])
                    nc.sync.dma_start(totals[b:b + 1, :],
                                      pre_sb[b][127:128, :].bitcast(F32R))
                for b in range(TB):
                    ofs_ps = psE2.tile([P, E], F32, tag="ofsps", name="ofsps")
                    nc.tensor.matmul(ofs_ps[:], slb[:, b * P:(b + 1) * P],
                                     totals[:], start=True, stop=True)
                    nc.vector.tensor_add(grank[b][:], pre_sb[b][:], ofs_ps[:])
                    nc.vector.tensor_scalar_add(grank[b][:], grank[b][:], -1.0)

            for b in range(TB):
                gm = pe2.tile([P, E], F32, tag="gm")
                nc.vector.tensor_scalar(out=gm[:], in0=grank[b][:],
                                        scalar1=float(CAP - 1), scalar2=BIG,
                                        op0=ALU.is_gt, op1=ALU.mult)
                nc.vector.tensor_add(grank[b][:], grank[b][:], gm[:])
                um = pe2.tile([P, E], F32, tag="um")
                nc.vector.tensor_scalar(out=um[:], in0=sel[b][:],
                                        scalar1=-BIG, scalar2=BIG,
                                        op0=ALU.mult, op1=ALU.add)
                nc.vector.tensor_add(grank[b][:], grank[b][:], um[:])
                nc.sync.dma_start(dbg_rank[b * P:(b + 1) * P, :], grank[b][:])

            sent = pe.tile([P, 1], I32)
            nc.vector.memset(sent[:], 1000000)
            for k in range(2 * CAP // P):
                nc.sync.dma_start(tok_lists[k * P:(k + 1) * P, :], sent[:])
            for b in range(TB):
                tok = pe2.tile([P, 1], I32, tag="tok")
                nc.gpsimd.iota(tok[:], pattern=[[0, 1]], base=b * P,
                               channel_multiplier=1)
                for ei in range(2):
                    ridx = pe2.tile([P, 1], F32, tag="ridx")
                    nc.vector.tensor_scalar_add(
                        ridx[:], grank[b][:, ei:ei + 1], float(ei * CAP))
                    ridx_i = pe2.tile([P, 1], I32, tag="ridxi")
                    nc.vector.tensor_copy(ridx_i[:], ridx[:])
                    nc.gpsimd.indirect_dma_start(
                        out=tok_lists[:],
                        out_offset=bass.IndirectOffsetOnAxis(
                            ap=ridx_i[:, :1], axis=0),
                        in_=tok[:], in_offset=None,
                        bounds_check=2 * CAP - 1, oob_is_err=False)

            # pass 2: shared up + act-mul
            with tc.tile_pool(name="psE3", bufs=1, space="PSUM") as psE3:
                u_ps = [psE3.tile([P, T], F32, tag=f"ups{sp}", name=f"ups{sp}")
                        for sp in range(SP)]
                for hc in range(HC):
                    xc = load_xT_chunk(pe2, hc, "xcE2")
                    for n in range(2):
                        sl = slice(n * 512, (n + 1) * 512)
                        for sp in range(SP):
                            c0 = hc * SP * P + sp * P
                            nc.tensor.matmul(u_ps[sp][:, sl],
                                             wsu_sb[:, c0:c0 + P], xc[:, sl],
                                             start=(hc == 0),
                                             stop=(hc == HC - 1))
                for sp in range(SP):
                    nc.vector.tensor_mul(hs[sp][:], g_act[sp][:], u_ps[sp][:])

            # shared down -> rs2_in (token-major)
            with tc.tile_pool(name="psE4", bufs=2, space="PSUM") as psE4:
                for tb_ in range(TB):
                    for n in range(4):
                        sl = slice(n * 512, (n + 1) * 512)
                        ps = psE4.tile([P, 512], F32, tag="psGd")
                        for sp in range(SP):
                            nc.tensor.matmul(ps[:],
                                             hs[sp][:, tb_ * P:(tb_ + 1) * P],
                                             wsd_sb[sp][:, sl],
                                             start=(sp == 0),
                                             stop=(sp == SP - 1))
                        sb_ = pe2.tile([P, 512], F32, tag="sbGd")
                        nc.vector.tensor_copy(sb_[:], ps[:])
                        nc.sync.dma_start(
                            rs2_in[tb_ * P:(tb_ + 1) * P, sl], sb_[:])

        with tc.tile_pool(name="pdbg", bufs=2) as pdbg:
            for k in range(2 * CAP // P):
                dt_ = pdbg.tile([P, 1], I32, tag="dtok")
                nc.sync.dma_start(dt_[:], tok_lists[k * P:(k + 1) * P, :])
                nc.sync.dma_start(dbg_tok[k * P:(k + 1) * P, :], dt_[:])

        # ======== F: experts ========
        for ei in range(2):
            with tc.tile_pool(name=f"pf{ei}", bufs=1) as pf, \
                 tc.tile_pool(name=f"pf2{ei}", bufs=2) as pf2:
                idx_sb = [pf.tile([P, 1], I32, tag=f"idx{k}", name=f"idx{k}") for k in range(2)]
                gxT = pf.tile([P, HC * CAP], BF16)
                wd_res = [pf.tile([P, H], BF16, tag=f"wd{ip}", name=f"wd{ip}")
                          for ip in range(IP)]
                for ip in range(IP):
                    nc.sync.dma_start(
                        wd_res[ip][:],
                        ex["we_d"][ei, ip * P:(ip + 1) * P, :])
                with tc.tile_pool(name=f"psF1{ei}", bufs=2, space="PSUM") as psF1:
                    for k in range(2):
                        nc.sync.dma_start(
                            idx_sb[k][:],
                            tok_lists[ei * CAP + k * P:ei * CAP + (k + 1) * P, :])
                        gx = pf2.tile([P, H], BF16, tag="gx")
                        nc.vector.memset(gx[:], 0.0)
                        nc.gpsimd.indirect_dma_start(
                            out=gx[:], out_offset=None,
                            in_=x_tm[:],
                            in_offset=bass.IndirectOffsetOnAxis(
                                ap=idx_sb[k][:, :1], axis=0),
                            bounds_check=T - 1, oob_is_err=False)
                        for hc in range(HC):
                            tp = psF1.tile([P, P], BF16, tag="tpF")
                            nc.tensor.transpose(
                                tp[:], gx[:, hc * P:(hc + 1) * P], identb[:])
                            nc.vector.tensor_copy(
                                gxT[:, hc * CAP + k * P:hc * CAP + (k + 1) * P],
                                tp[:])

                # gate pass (token-major h): h_tm[k] [P, I]
                g_tm = [pf.tile([P, I], BF16, tag=f"gtm{k}", name=f"gtm{k}") for k in range(2)]
                with tc.tile_pool(name=f"psF2{ei}", bufs=1, space="PSUM") as psF2:
                    g_ps = [[psF2.tile([P, 512], F32, tag=f"gps{k}{n}", name=f"gpsF{k}{n}")
                             for n in range(2)] for k in range(2)]
                    for hc in range(HC):
                        wg = pf2.tile([P, I], BF16, tag="wgF", bufs=3)
                        nc.sync.dma_start(
                            wg[:],
                            ex["we_g"][ei, hc * P:(hc + 1) * P, :])
                        for k in range(2):
                            for n in range(2):
                                nc.tensor.matmul(
                                    g_ps[k][n][:],
                                    gxT[:, hc * CAP + k * P:hc * CAP + (k + 1) * P],
                                    wg[:, n * 512:(n + 1) * 512],
                                    start=(hc == 0), stop=(hc == HC - 1))
                    for k in range(2):
                        for n in range(2):
                            _silu(nc, pf2,
                                  g_tm[k][:, n * 512:(n + 1) * 512],
                                  g_ps[k][n][:], [P, 512])
                h_tm = [pf.tile([P, I], BF16, tag=f"htm{k}", name=f"htm{k}") for k in range(2)]
                with tc.tile_pool(name=f"psF3{ei}", bufs=1, space="PSUM") as psF3:
                    u_ps = [[psF3.tile([P, 512], F32, tag=f"ups{k}{n}", name=f"upsF{k}{n}")
                             for n in range(2)] for k in range(2)]
                    for hc in range(HC):
                        wu = pf2.tile([P, I], BF16, tag="wuF", bufs=3)
                        nc.sync.dma_start(
                            wu[:],
                            ex["we_u"][ei, hc * P:(hc + 1) * P, :])
                        for k in range(2):
                            for n in range(2):
                                nc.tensor.matmul(
                                    u_ps[k][n][:],
                                    gxT[:, hc * CAP + k * P:hc * CAP + (k + 1) * P],
                                    wu[:, n * 512:(n + 1) * 512],
                                    start=(hc == 0), stop=(hc == HC - 1))
                    for k in range(2):
                        for n in range(2):
                            sl = slice(n * 512, (n + 1) * 512)
                            nc.vector.tensor_mul(h_tm[k][:, sl], g_tm[k][:, sl],
                                                 u_ps[k][n][:])
                # transpose h to [I-part, tok]
                h_sb = [pf.tile([P, 2 * P], BF16, tag=f"hsb{ip}", name=f"hsb{ip}")
                        for ip in range(IP)]
                with tc.tile_pool(name=f"psF4{ei}", bufs=2, space="PSUM") as psF4:
                    for k in range(2):
                        for ip in range(IP):
                            tp = psF4.tile([P, P], BF16, tag="tpF2")
                            nc.tensor.transpose(
                                tp[:], h_tm[k][:, ip * P:(ip + 1) * P], identb[:])
                            nc.vector.tensor_copy(
                                h_sb[ip][:, k * P:(k + 1) * P], tp[:])
                # down + gate-weight scale + scatter-add
                with tc.tile_pool(name=f"psF5{ei}", bufs=2, space="PSUM") as psF5:
                    for k in range(2):
                        wg_t = pf2.tile([P, E], F32, tag="wgt")
                        nc.vector.memset(wg_t[:], 0.0)
                        nc.gpsimd.indirect_dma_start(
                            out=wg_t[:], out_offset=None, in_=W_dram[:],
                            in_offset=bass.IndirectOffsetOnAxis(
                                ap=idx_sb[k][:, :1], axis=0),
                            bounds_check=T - 1, oob_is_err=False)
                        out_sb = pf.tile([P, H], F32, tag=f"outsb{k}")
                        for n in range(4):
                            sl = slice(n * 512, (n + 1) * 512)
                            ps = psF5.tile([P, 512], F32, tag="psFd")
                            for ip in range(IP):
                                nc.tensor.matmul(
                                    ps[:], h_sb[ip][:, k * P:(k + 1) * P],
                                    wd_res[ip][:, sl],
                                    start=(ip == 0), stop=(ip == IP - 1))
                            nc.vector.tensor_scalar_mul(out_sb[:, sl], ps[:],
                                                        wg_t[:, ei:ei + 1])
                        nc.gpsimd.indirect_dma_start(
                            out=rs2_in[:],
                            out_offset=bass.IndirectOffsetOnAxis(
                                ap=idx_sb[k][:, :1], axis=0),
                            in_=out_sb[:], in_offset=None,
                            bounds_check=T - 1, oob_is_err=False,
                            compute_op=ALU.add)

        nc.gpsimd.collective_compute(
            "ReduceScatter", ALU.add, ins=[rs2_in.opt()], outs=[rs2_out.opt()],
            replica_groups=[list(range(NCN))])
        with tc.tile_pool(name="pz", bufs=2) as pz:
            fin = pz.tile([P, H], F32)
            nc.sync.dma_start(fin[:], rs2_out[:])
            nc.sync.dma_start(out_slice[:], fin[:])


_CACHE = {}


def _build():
    key = ("nc", SIM)
    if key in _CACHE:
        return _CACHE[key]
    nc = bacc.Bacc("TRN2", target_bir_lowering=False, debug=False,
                   num_devices=NCN)
    with tile.TileContext(nc) as tc:
        _emit(nc, tc)
    nc.compile()
    _CACHE[key] = nc
    return nc


def _host_prep(inputs):
    pos = np.asarray(inputs["positions"]).astype(np.float64)
    hid = np.asarray(inputs["hidden_states"], np.float32)
    w_in = np.asarray(inputs["w_in_ln"], np.float32)
    w_post = np.asarray(inputs["w_post_ln"], np.float32)
    wq = np.asarray(inputs["wq"], np.float32) * w_in[:, None]
    wk = np.asarray(inputs["wk"], np.float32) * w_in[:, None]
    wv = np.asarray(inputs["wv"], np.float32) * w_in[:, None]
    wo = np.asarray(inputs["wo"], np.float32)
    gate_w = np.asarray(inputs["gate_w"], np.float32) * w_post[None, :]
    gate_b = np.asarray(inputs["gate_bias"], np.float32).reshape(1, E)
    import ml_dtypes
    bf = ml_dtypes.bfloat16
    we_g = (np.asarray(inputs["we_gate"], np.float32)
            * w_post[None, :, None]).astype(bf)
    we_u = (np.asarray(inputs["we_up"], np.float32)
            * w_post[None, :, None]).astype(bf)
    we_d = np.asarray(inputs["we_down"], np.float32).astype(bf)
    ws_g = np.asarray(inputs["ws_gate"], np.float32) * w_post[:, None]
    ws_u = np.asarray(inputs["ws_up"], np.float32) * w_post[:, None]
    ws_d = np.asarray(inputs["ws_down"], np.float32)

    inv_freq = 1.0 / (THETA ** (np.arange(0, D, 2, dtype=np.float64) / D))
    f = pos[None, :] * inv_freq[:, None]
    cos2, sin2 = np.cos(f), np.sin(f)
    cosT = np.repeat(cos2, 2, axis=0).astype(np.float32)
    sinT = np.empty((D, T), np.float32)
    sinT[0::2] = -sin2
    sinT[1::2] = sin2
    s = 1.0 / np.sqrt(D)
    cosq, sinq = (cosT * s).astype(np.float32), (sinT * s).astype(np.float32)

    posi = np.asarray(inputs["positions"]).astype(np.int64)
    mask = np.where(posi[:, None] >= posi[None, :], 0.0, NEG).astype(bf)

    identr_in = np.eye(P, dtype=np.float32)
    ut_in = np.triu(np.ones((P, P), np.float32))
    slb_in = np.zeros((8, TB * P), np.float32)
    for b in range(TB):
        slb_in[:b, b * P:(b + 1) * P] = 1.0
    perm = np.zeros((P, P), np.float32)
    for i in range(0, P, 2):
        perm[i, i + 1] = 1.0
        perm[i + 1, i] = 1.0

    ISC = IS // NCN
    maps = []
    for c in range(NCN):
        g = c // 2
        eorder = [2 * c, 2 * c + 1] + [e for e in range(E)
                                       if e not in (2 * c, 2 * c + 1)]
        maps.append({
            "hid": hid,
            "hid_slice": np.ascontiguousarray(hid[c * P:(c + 1) * P]),
            "wq_s": np.ascontiguousarray(wq[:, 2 * c * D:(2 * c + 2) * D]),
            "wk_s": np.ascontiguousarray(wk[:, g * D:(g + 1) * D]),
            "wv_s": np.ascontiguousarray(wv[:, g * D:(g + 1) * D]),
            "wo_s": np.ascontiguousarray(wo[2 * c * D:(2 * c + 2) * D, :]),
            "cosq": cosq, "sinq": sinq, "cosk": cosT, "sink": sinT,
            "perm": perm, "masks": mask,
            "identr_in": identr_in, "ut_in": ut_in, "slb_in": slb_in,
            "identb_in": identr_in.astype(bf),
            "gate_wT": np.ascontiguousarray(gate_w[eorder, :].T),
            "gate_b": np.broadcast_to(gate_b[:, eorder], (P, E)).copy(),
            "ws_g": np.ascontiguousarray(ws_g[:, c * ISC:(c + 1) * ISC]),
            "ws_u": np.ascontiguousarray(ws_u[:, c * ISC:(c + 1) * ISC]),
            "ws_d": np.ascontiguousarray(ws_d[c * ISC:(c + 1) * ISC, :]),
            "we_g": np.ascontiguousarray(we_g[2 * c:2 * c + 2]),
            "we_u": np.ascontiguousarray(we_u[2 * c:2 * c + 2]),
            "we_d": np.ascontiguousarray(we_d[2 * c:2 * c + 2]),
        })
    return maps


def kernel(trace=False, **inputs):
    nc = _build()
    maps = _host_prep(inputs)
    res = bass_utils.run_bass_kernel_spmd(
        nc, maps, core_ids=list(range(NCN)), trace=trace)
    out = np.concatenate([res.results[c]["out_slice"] for c in range(NCN)], 0)
    resid = np.concatenate([res.results[c]["res_slice"] for c in range(NCN)], 0)
    kernel.last_results = res
    return out, resid



# revision 6
# speedup vs baseline: 1.3527x; 1.3527x over previous
"""Ernie4 decoder layer (RMSNorm + GQA attention + shared expert + 16-expert
top-2 MoE) on 8 Trainium2 NeuronCores.

v2 — fp16 data path everywhere except the router (which must reproduce the
reference top-2 selection exactly; margins are ~3e-5 so it stays fp32 and is
computed locally per core before the AllGather):
  - Attention: head-parallel (2 q-heads + 1 kv-head per core), fp16 QKV /
    scores / probs / o_proj with causal-block skipping; fp16 ReduceScatter.
  - Router: fp32 logits on each core's own 128 tokens; W+sel AllGathered in a
    tiny fp32 collective that precedes the fp16 x AllGather so the token-list
    build overlaps it.
  - Shared expert: intermediate-sharded (IS/8 per core) fp16, output seeds
    the MoE combine buffer.
  - MoE: expert-parallel (2 experts per core), token lists via
    triangular-matmul prefix ranks, indirect-DMA gather/scatter-add in fp16,
    fp16 ReduceScatter for the combine.
"""
import sys
sys.path.insert(0, "/opt/trn_rl_repo")

import numpy as np

import concourse.bass as bass
import concourse.bacc as bacc
import concourse.tile as tile
import concourse.mybir as mybir
from concourse import bass_utils
from concourse.masks import make_identity

dt = mybir.dt
F32 = dt.float32
F32R = dt.float32r
F16 = dt.float16
I32 = dt.int32
BF16 = dt.bfloat16
AF = mybir.ActivationFunctionType
ALU = mybir.AluOpType
AX = mybir.AxisListType

T, H, NH, NKV, D = 1024, 2048, 16, 4, 128
E, I, IS = 16, 1024, 2048
EPS = 1e-6
THETA = 10000.0
NCN = 8
P = 128
TB = T // P            # 8 token blocks
HC = H // P            # 16 hidden chunks
IP = I // P            # 8 expert-intermediate chunks
SP = IS // NCN // P    # 2 shared-intermediate chunks per core
CAP = 256              # per-expert token capacity
BIG = 1.0e6            # OOB sentinel
NEG = -1e9


def _emit(nc, tc):
    ex = {}
    for name, shape, d in [
        ("hid", [T, H], F32), ("hid_slice", [P, H], F32),
        ("wq_s", [H, 2 * D], F16), ("wk_s", [H, D], F16), ("wv_s", [H, D], F16),
        ("wo_s", [2 * D, H], F16),
        ("cosq", [D, T], F32), ("sinq", [D, T], F32),
        ("cosk", [D, T], F32), ("sink", [D, T], F32),
        ("perm", [P, P], F32),
        ("diag_mask", [P, P], BF16),
        ("gate_wT", [H, E], F32), ("gate_b", [P, E], F32),
        ("emask01", [P, 2 * E], F32),
        ("ws_g", [H, SP * P], F16), ("ws_u", [H, SP * P], F16),
        ("ws_d", [SP * P, H], F16),
        ("we_g", [2, H, I], F16), ("we_u", [2, H, I], F16),
        ("we_d", [2, I, H], F16),
        ("identr_in", [P, P], F32), ("identh_in", [P, P], F16),
        ("ut_in", [P, P], F32),
        ("slb_in", [8, TB * P], F32),
    ]:
        ex[name] = nc.dram_tensor(name, shape, d, kind="ExternalInput").ap()
    out_slice = nc.dram_tensor("out_slice", [P, H], F32, kind="ExternalOutput").ap()
    res_slice = nc.dram_tensor("res_slice", [P, H], F32, kind="ExternalOutput").ap()
    dbg_w = nc.dram_tensor("dbg_w", [P, E], F32, kind="ExternalOutput").ap()

    with tc.tile_pool(name="persist", bufs=1) as pp, \
         tc.tile_pool(name="dram", bufs=1, space="DRAM") as dram:
        rs_in = dram.tile([T, H], F16)
        rs_out = dram.tile([P, H], F16)
        agw_in = dram.tile([P, 2 * E], F32)
        w_all = dram.tile([T, 2 * E], F32, addr_space="Shared")
        agx_in = dram.tile([P, H], F16)
        x_tm = dram.tile([T, H], F16, addr_space="Shared")
        tok_lists = dram.tile([2 * CAP, 1], I32)
        rs2_in = dram.tile([T, H], F16)
        rs2_out = dram.tile([P, H], F16)

        ident = pp.tile([P, P], F32)
        make_identity(nc, ident[:])
        identr = pp.tile([P, P], F32R)
        nc.sync.dma_start(identr[:], ex["identr_in"][:].bitcast(F32R))
        identh = pp.tile([P, P], F16)
        nc.sync.dma_start(identh[:], ex["identh_in"][:])
        hid_sl = pp.tile([P, H], F32)
        nc.sync.dma_start(hid_sl[:], ex["hid_slice"][:])
        eps_t = pp.tile([P, 1], F32)
        nc.vector.memset(eps_t[:], EPS)
        emask01 = pp.tile([P, 2 * E], F32)
        nc.sync.dma_start(emask01[:], ex["emask01"][:])

        # ======== Phases A-C: attention (fp16) ========
        with tc.tile_pool(name="pab", bufs=1) as pab:
            qT = [pab.tile([P, T], F16, tag=f"qT{j}", name=f"qT{j}")
                  for j in range(2)]
            kT = pab.tile([P, T], F16)
            vT = pab.tile([P, T], F16)
            v_tm = [pab.tile([P, D], F16, tag=f"vtm{b}", name=f"vtm{b}")
                    for b in range(TB)]
            oT = [pab.tile([P, T], F16, tag=f"oT{j}", name=f"oT{j}")
                  for j in range(2)]

            # ---- A: norm + transpose + QKV + rope ----
            with tc.tile_pool(name="pa", bufs=1) as pa, \
                 tc.tile_pool(name="pa2", bufs=3) as pa2:
                cosq = pa.tile([D, T], F32)
                sinq = pa.tile([D, T], F32)
                cosk = pa.tile([D, T], F32)
                sink = pa.tile([D, T], F32)
                for t_, s_ in [(cosq, "cosq"), (sinq, "sinq"),
                               (cosk, "cosk"), (sink, "sink")]:
                    nc.sync.dma_start(t_[:], ex[s_][:])
                permr = pa.tile([P, P], F32R)
                nc.sync.dma_start(permr[:], ex["perm"][:].bitcast(F32R))
                wq_sb = pa.tile([P, HC * 2 * D], F16)
                wk_sb = pa.tile([P, HC * D], F16)
                wv_sb = pa.tile([P, HC * D], F16)
                for t_, s_, m in [(wq_sb, "wq_s", 2 * D), (wk_sb, "wk_s", D),
                                  (wv_sb, "wv_s", D)]:
                    nc.sync.dma_start(
                        t_[:].rearrange("p (hc m) -> p hc m", hc=HC),
                        ex[s_][:].rearrange("(hc p) m -> p hc m", p=P))

                dump = pa.tile([P, H], F32)
                qraw = [pa.tile([P, T], F32R, tag=f"qraw{j}", name=f"qraw{j}")
                        for j in range(2)]
                kraw = pa.tile([P, T], F32R)
                with tc.tile_pool(name="psA1", bufs=2, space="PSUM") as psA1, \
                     tc.tile_pool(name="psA2", bufs=2, space="PSUM") as psA2:
                    for n in range(2):
                        x0T = [pa.tile([P, 512], F16, tag=f"x0T{hc}",
                                       name=f"x0T{hc}_{n}") for hc in range(HC)]
                        for bb in range(TB // 2):
                            b = n * (TB // 2) + bb
                            hidb = pa2.tile([P, H], F32, tag="hidb", bufs=2)
                            nc.sync.dma_start(hidb[:],
                                              ex["hid"][b * P:(b + 1) * P, :])
                            ssum = pa2.tile([P, 1], F32, tag="ssum")
                            nc.scalar.activation(dump[:], hidb[:], AF.Square,
                                                 accum_out=ssum[:, :1])
                            rms = pa2.tile([P, 1], F32, tag="rms")
                            nc.scalar.activation(rms[:], ssum[:],
                                                 AF.Sqrt, bias=eps_t[:, :1],
                                                 scale=1.0 / H)
                            inv = pa2.tile([P, 1], F32, tag="inv")
                            nc.vector.reciprocal(inv[:], rms[:])
                            x0b = pa2.tile([P, H], F16, tag="x0b", bufs=2)
                            nc.vector.tensor_scalar_mul(x0b[:], hidb[:],
                                                        inv[:, :1])
                            for hc in range(HC):
                                tp = psA1.tile([P, P], F16, tag="tpA")
                                nc.tensor.transpose(
                                    tp[:], x0b[:, hc * P:(hc + 1) * P],
                                    identh[:])
                                nc.vector.tensor_copy(
                                    x0T[hc][:, bb * P:(bb + 1) * P], tp[:])

                        def proj(w_sb, m, c0, dst, n=n, x0T=x0T, fp16=False):
                            ps = psA2.tile([P, 512], F32, tag="psQKV",
                                           name="psQKV")
                            for hc in range(HC):
                                nc.tensor.matmul(
                                    ps[:],
                                    w_sb[:, hc * m + c0:hc * m + c0 + P],
                                    x0T[hc][:],
                                    start=(hc == 0), stop=(hc == HC - 1))
                            nc.vector.tensor_copy(
                                dst[:, n * 512:(n + 1) * 512], ps[:])
                        proj(wq_sb, 2 * D, 0, qraw[0])
                        proj(wq_sb, 2 * D, D, qraw[1])
                        proj(wk_sb, D, 0, kraw)
                        proj(wv_sb, D, 0, vT, fp16=True)

                with tc.tile_pool(name="psA3", bufs=2, space="PSUM") as psA3:
                    for src, dst, c_, s_ in [(qraw[0], qT[0], cosq, sinq),
                                             (qraw[1], qT[1], cosq, sinq),
                                             (kraw, kT, cosk, sink)]:
                        for n in range(2):
                            sl = slice(n * 512, (n + 1) * 512)
                            sw = psA3.tile([P, 512], F32, tag="psSW")
                            nc.tensor.matmul(sw[:], permr[:], src[:, sl],
                                             start=True, stop=True)
                            t1 = pa2.tile([P, 512], F32, tag="ropeT1")
                            nc.vector.tensor_mul(t1[:], src[:, sl], c_[:, sl])
                            t2 = pa2.tile([P, 512], F32, tag="ropeT2")
                            nc.vector.tensor_mul(t2[:], sw[:], s_[:, sl])
                            nc.vector.tensor_add(dst[:, sl], t1[:], t2[:])
                    for b in range(TB):
                        tp = psA3.tile([P, P], F16, tag="tpV")
                        nc.tensor.transpose(tp[:], vT[:, b * P:(b + 1) * P],
                                            identh[:])
                        nc.vector.tensor_copy(v_tm[b][:], tp[:])

            # ---- B: attention (causal-block skipped) ----
            with tc.tile_pool(name="pb", bufs=1) as pb, \
                 tc.tile_pool(name="pb2", bufs=3) as pb2:
                dmask = pb.tile([P, P], BF16)
                nc.sync.dma_start(dmask[:], ex["diag_mask"][:])
                wo_sb = [pb.tile([P, H], F16, tag=f"wo{j}", name=f"wo{j}")
                         for j in range(2)]
                nc.sync.dma_start(wo_sb[0][:], ex["wo_s"][0:P, :])
                nc.sync.dma_start(wo_sb[1][:], ex["wo_s"][P:2 * P, :])

                attnT = [pb.tile([P, T], F16, tag=f"attnT{kc}",
                                 name=f"attnT{kc}") for kc in range(TB)]
                for kc in range(1, TB):
                    nc.vector.memset(attnT[kc][:, 0:kc * P], 0.0)
                with tc.tile_pool(name="psB1", bufs=2, space="PSUM") as psB1, \
                     tc.tile_pool(name="psB2", bufs=2, space="PSUM") as psB2, \
                     tc.tile_pool(name="psB3", bufs=2, space="PSUM") as psB3:
                  for h in range(2):
                    for qc in range(TB):
                        cols = (qc + 1) * P
                        prob = pb2.tile([P, T], F32, tag="prob")
                        nsl = (cols + 511) // 512
                        for n in range(nsl):
                            w_ = min(512, cols - n * 512)
                            ps = psB1.tile([P, 512], F32, tag="psSC")
                            nc.tensor.matmul(ps[:, :w_],
                                             qT[h][:, qc * P:(qc + 1) * P],
                                             kT[:, n * 512:n * 512 + w_],
                                             start=True, stop=True)
                            # diagonal block gets the causal mask; the rest
                            # of this slice is fully visible
                            d0 = qc * P - n * 512
                            if 0 <= d0 < w_:
                                if d0 > 0:
                                    nc.vector.tensor_copy(
                                        prob[:, n * 512:n * 512 + d0],
                                        ps[:, :d0])
                                nc.vector.tensor_add(
                                    prob[:, qc * P:qc * P + P],
                                    ps[:, d0:d0 + P], dmask[:])
                            else:
                                nc.vector.tensor_copy(
                                    prob[:, n * 512:n * 512 + w_], ps[:, :w_])
                        mx = pb2.tile([P, 1], F32, tag="mx")
                        nc.vector.reduce_max(mx[:], prob[:, :cols], axis=AX.X)
                        negm = pb2.tile([P, 1], F32, tag="negm")
                        nc.vector.tensor_scalar_mul(negm[:], mx[:], -1.0)
                        ssum = pb2.tile([P, 1], F32, tag="esum")
                        probe_ = pb2.tile([P, T], F32, tag="probe")
                        nc.scalar.activation(probe_[:, :cols], prob[:, :cols],
                                             AF.Exp, bias=negm[:, :1],
                                             accum_out=ssum[:, :1])
                        rec = pb2.tile([P, 1], F32, tag="rec")
                        nc.vector.reciprocal(rec[:], ssum[:])
                        probS = pb2.tile([P, T], F16, tag="probS")
                        nc.vector.tensor_scalar_mul(probS[:, :cols],
                                                    probe_[:, :cols],
                                                    rec[:, :1])
                        for kc in range(qc + 1):
                            tp = psB2.tile([P, P], F16, tag="tpB")
                            nc.tensor.transpose(
                                tp[:], probS[:, kc * P:(kc + 1) * P],
                                identh[:])
                            nc.vector.tensor_copy(
                                attnT[kc][:, qc * P:(qc + 1) * P], tp[:])
                    for n in range(2):
                        sl = slice(n * 512, (n + 1) * 512)
                        kc_hi = 4 * n + 3
                        ps = psB3.tile([P, 512], F32, tag="psAV")
                        for kc in range(kc_hi + 1):
                            nc.tensor.matmul(ps[:], v_tm[kc][:],
                                             attnT[kc][:, sl],
                                             start=(kc == 0),
                                             stop=(kc == kc_hi))
                        nc.vector.tensor_copy(oT[h][:, sl], ps[:])

                # ---- C: o_proj ----
                with tc.tile_pool(name="psC", bufs=8, space="PSUM") as psC:
                    for tb_ in range(TB):
                        pso = [psC.tile([P, 512], F32, tag="psO",
                                        name=f"psO{n}") for n in range(4)]
                        for hp in range(2):
                            for n in range(4):
                                nc.tensor.matmul(
                                    pso[n][:],
                                    oT[hp][:, tb_ * P:(tb_ + 1) * P],
                                    wo_sb[hp][:, n * 512:(n + 1) * 512],
                                    start=(hp == 0), stop=(hp == 1))
                        ob = pb2.tile([P, H], F16, tag="ob", bufs=2)
                        for n in range(4):
                            nc.vector.tensor_copy(
                                ob[:, n * 512:(n + 1) * 512], pso[n][:])
                        nc.sync.dma_start(rs_in[tb_ * P:(tb_ + 1) * P, :],
                                          ob[:])

        nc.gpsimd.collective_compute(
            "ReduceScatter", ALU.add, ins=[rs_in.opt()], outs=[rs_out.opt()],
            replica_groups=[list(range(NCN))])

        # ======== D: residual + norm + local fp32 router + AGs ========
        with tc.tile_pool(name="pd", bufs=1) as pd, \
             tc.tile_pool(name="pd2", bufs=2) as pd2, \
             tc.tile_pool(name="psD", bufs=2, space="PSUM") as psD:
            attn_sl = pd.tile([P, H], F16)
            nc.sync.dma_start(attn_sl[:], rs_out[:])
            res_sb = pd.tile([P, H], F32)
            nc.vector.tensor_add(res_sb[:], hid_sl[:], attn_sl[:])
            nc.sync.dma_start(res_slice[:], res_sb[:])
            dump2 = pd.tile([P, H], F32)
            ssum = pd.tile([P, 1], F32)
            nc.scalar.activation(dump2[:], res_sb[:], AF.Square,
                                 accum_out=ssum[:, :1])
            rms = pd.tile([P, 1], F32)
            nc.scalar.activation(rms[:], ssum[:], AF.Sqrt, bias=eps_t[:, :1],
                                 scale=1.0 / H)
            inv = pd.tile([P, 1], F32)
            nc.vector.reciprocal(inv[:], rms[:])
            x_sl = pd.tile([P, H], F32)
            nc.vector.tensor_scalar_mul(x_sl[:], res_sb[:], inv[:, :1])
            x_sl_h = pd.tile([P, H], F16)
            nc.vector.tensor_copy(x_sl_h[:], x_sl[:])
            nc.sync.dma_start(agx_in[:], x_sl_h[:])

            # fp32 transpose of own slice for exact router logits
            gw_sb = pd.tile([P, HC * E], F32)
            nc.sync.dma_start(
                gw_sb[:].rearrange("p (hc e) -> p hc e", hc=HC),
                ex["gate_wT"][:].rearrange("(hc p) e -> p hc e", p=P))
            gate_b = pd.tile([P, E], F32)
            nc.sync.dma_start(gate_b[:], ex["gate_b"][:])
            x_slT = pd.tile([P, H], F32)
            for hc in range(HC):
                tp = psD.tile([P, P], F32, tag="tpD")
                nc.tensor.transpose(tp[:], x_sl[:, hc * P:(hc + 1) * P],
                                    ident[:])
                nc.vector.tensor_copy(x_slT[:, hc * P:(hc + 1) * P], tp[:])
            lg_ps = psD.tile([P, E], F32, tag="lgps", name="lgps")
            for hc in range(HC):
                nc.tensor.matmul(lg_ps[:], x_slT[:, hc * P:(hc + 1) * P],
                                 gw_sb[:, hc * E:(hc + 1) * E],
                                 start=(hc == 0), stop=(hc == HC - 1))
            sig = pd2.tile([P, E], F32, tag="sig")
            nc.scalar.activation(sig[:], lg_ps[:], AF.Sigmoid)
            sb_ = pd2.tile([P, E], F32, tag="sb_")
            nc.vector.tensor_add(sb_[:], sig[:], gate_b[:])
            mx = pd2.tile([P, 8], F32, tag="mx8")
            nc.vector.max(out=mx[:], in_=sb_[:])
            s1 = pd2.tile([P, E], F32, tag="s1")
            nc.vector.tensor_tensor(out=s1[:], in0=sb_[:],
                                    in1=mx[:, 0:1].to_broadcast([P, E]),
                                    op=ALU.is_equal)
            s2 = pd2.tile([P, E], F32, tag="s2")
            nc.vector.tensor_tensor(out=s2[:], in0=sb_[:],
                                    in1=mx[:, 1:2].to_broadcast([P, E]),
                                    op=ALU.is_equal)
            nc.vector.tensor_add(s1[:], s1[:], s2[:])
            sel_own = pd2.tile([P, E], F32, tag="sel_own")
            nc.vector.tensor_scalar_min(sel_own[:], s1[:], 1.0)
            wa = pd2.tile([P, E], F32, tag="wa")
            nc.vector.tensor_mul(wa[:], sel_own[:], sig[:])
            nrm = pd2.tile([P, 1], F32, tag="nrm")
            nc.vector.reduce_sum(nrm[:], wa[:], axis=AX.X)
            rec = pd2.tile([P, 1], F32, tag="recw")
            nc.vector.reciprocal(rec[:], nrm[:])
            w_tm = pd2.tile([P, E], F32, tag="wtm")
            nc.vector.tensor_scalar_mul(w_tm[:], wa[:], rec[:, :1])
            nc.sync.dma_start(agw_in[:, 0:E], w_tm[:])
            nc.sync.dma_start(agw_in[:, E:2 * E], sel_own[:])
            nc.sync.dma_start(dbg_w[:], w_tm[:])

        nc.gpsimd.collective_compute(
            "AllGather", ALU.bypass, ins=[agw_in.opt()], outs=[w_all.opt()],
            replica_groups=[list(range(NCN))])
        nc.gpsimd.collective_compute(
            "AllGather", ALU.bypass, ins=[agx_in.opt()], outs=[x_tm.opt()],
            replica_groups=[list(range(NCN))])

        # ======== E: token lists from AllGathered router decisions ========
        with tc.tile_pool(name="pe", bufs=1) as pe, \
             tc.tile_pool(name="pe2", bufs=3) as pe2, \
             tc.tile_pool(name="psE", bufs=2, space="PSUM") as psE:
            ut = pe.tile([P, P], F32R)
            nc.sync.dma_start(ut[:], ex["ut_in"][:].bitcast(F32R))
            slb = pe.tile([8, TB * P], F32R)
            nc.sync.dma_start(slb[:], ex["slb_in"][:].bitcast(F32R))
            totals = pe.tile([8, E], F32R)
            pre_sb = [pe.tile([P, E], F32, tag=f"pre{b}", name=f"pre{b}")
                      for b in range(TB)]
            sel_all = [pe.tile([P, E], F32, tag=f"sela{b}", name=f"sela{b}")
                       for b in range(TB)]
            sent = pe.tile([P, 1], I32)
            nc.vector.memset(sent[:], 1000000)
            for k in range(2 * CAP // P):
                nc.sync.dma_start(tok_lists[k * P:(k + 1) * P, :], sent[:])
            for b in range(TB):
                nc.sync.dma_start(sel_all[b][:],
                                  w_all[b * P:(b + 1) * P, E:2 * E])
                pr_ps = psE.tile([P, E], F32, tag="prps")
                nc.tensor.matmul(pr_ps[:], ut[:],
                                 sel_all[b][:].bitcast(F32R),
                                 start=True, stop=True)
                nc.vector.tensor_copy(pre_sb[b][:], pr_ps[:])
                nc.sync.dma_start(totals[b:b + 1, :],
                                  pre_sb[b][127:128, :].bitcast(F32R))
            for b in range(TB):
                ofs_ps = psE.tile([P, E], F32, tag="ofsps", name="ofsps")
                nc.tensor.matmul(ofs_ps[:], slb[:, b * P:(b + 1) * P],
                                 totals[:], start=True, stop=True)
                grank = pe2.tile([P, E], F32, tag="grank")
                nc.vector.tensor_add(grank[:], pre_sb[b][:], ofs_ps[:])
                nc.vector.tensor_scalar_add(grank[:], grank[:], -1.0)
                gm = pe2.tile([P, E], F32, tag="gm")
                nc.vector.tensor_scalar(out=gm[:], in0=grank[:],
                                        scalar1=float(CAP - 1), scalar2=BIG,
                                        op0=ALU.is_gt, op1=ALU.mult)
                nc.vector.tensor_add(grank[:], grank[:], gm[:])
                um = pe2.tile([P, E], F32, tag="um")
                nc.vector.tensor_scalar(out=um[:], in0=sel_all[b][:],
                                        scalar1=-BIG, scalar2=BIG,
                                        op0=ALU.mult, op1=ALU.add)
                nc.vector.tensor_add(grank[:], grank[:], um[:])
                tok = pe2.tile([P, 1], I32, tag="tok")
                nc.gpsimd.iota(tok[:], pattern=[[0, 1]], base=b * P,
                               channel_multiplier=1)
                for ei in range(2):
                    ge = pe2.tile([P, E], F32, tag="ge")
                    nc.vector.tensor_mul(ge[:], grank[:],
                                         emask01[:, ei * E:(ei + 1) * E])
                    ridx = pe2.tile([P, 1], F32, tag="ridx")
                    nc.vector.reduce_sum(ridx[:], ge[:], axis=AX.X)
                    nc.vector.tensor_scalar_add(ridx[:], ridx[:],
                                                float(ei * CAP))
                    ridx_i = pe2.tile([P, 1], I32, tag="ridxi")
                    nc.vector.tensor_copy(ridx_i[:], ridx[:])
                    nc.gpsimd.indirect_dma_start(
                        out=tok_lists[:],
                        out_offset=bass.IndirectOffsetOnAxis(
                            ap=ridx_i[:, :1], axis=0),
                        in_=tok[:], in_offset=None,
                        bounds_check=2 * CAP - 1, oob_is_err=False)

        # ======== F: xT + shared expert + experts (fp16) ========
        with tc.tile_pool(name="pxt", bufs=1) as pxt:
            xc = [pxt.tile([P, T], F16, tag=f"xc{hc}", name=f"xc{hc}")
                  for hc in range(HC)]
            with tc.tile_pool(name="pxt2", bufs=3) as pxt2, \
                 tc.tile_pool(name="psX", bufs=2, space="PSUM") as psX:
                for b in range(TB):
                    xb = pxt2.tile([P, H], F16, tag="xb", bufs=2)
                    nc.sync.dma_start(xb[:], x_tm[b * P:(b + 1) * P, :])
                    for hc in range(HC):
                        tp = psX.tile([P, P], F16, tag="tpX")
                        nc.tensor.transpose(tp[:], xb[:, hc * P:(hc + 1) * P],
                                            identh[:])
                        nc.vector.tensor_copy(xc[hc][:, b * P:(b + 1) * P],
                                              tp[:])

            # ---- shared expert ----
            with tc.tile_pool(name="pg", bufs=1) as pg, \
                 tc.tile_pool(name="pg2", bufs=3) as pg2:
                wsg_sb = pg.tile([P, HC * SP * P], F16)
                wsu_sb = pg.tile([P, HC * SP * P], F16)
                for t_, s_ in [(wsg_sb, "ws_g"), (wsu_sb, "ws_u")]:
                    nc.sync.dma_start(
                        t_[:].rearrange("p (hc m) -> p hc m", hc=HC),
                        ex[s_][:].rearrange("(hc p) m -> p hc m", p=P))
                wsd_sb = [pg.tile([P, H], F16, tag=f"wsd{sp}", name=f"wsd{sp}")
                          for sp in range(SP)]
                for sp in range(SP):
                    nc.sync.dma_start(wsd_sb[sp][:],
                                      ex["ws_d"][sp * P:(sp + 1) * P, :])
                g_act = [pg.tile([P, T], F16, tag=f"gact{sp}", name=f"gact{sp}")
                         for sp in range(SP)]
                hs = [pg.tile([P, T], F16, tag=f"hs{sp}", name=f"hs{sp}")
                      for sp in range(SP)]
                with tc.tile_pool(name="psG1", bufs=1, space="PSUM") as psG1:
                    g_ps = [psG1.tile([P, T], F32, tag=f"gps{sp}",
                                      name=f"gps{sp}") for sp in range(SP)]
                    for hc in range(HC):
                        for sp in range(SP):
                            c0 = hc * SP * P + sp * P
                            for n in range(2):
                                sl = slice(n * 512, (n + 1) * 512)
                                nc.tensor.matmul(g_ps[sp][:, sl],
                                                 wsg_sb[:, c0:c0 + P],
                                                 xc[hc][:, sl],
                                                 start=(hc == 0),
                                                 stop=(hc == HC - 1))
                    for sp in range(SP):
                        nc.scalar.activation(g_act[sp][:], g_ps[sp][:],
                                             AF.Silu)
                with tc.tile_pool(name="psG2", bufs=1, space="PSUM") as psG2:
                    u_ps = [psG2.tile([P, T], F32, tag=f"ups{sp}",
                                      name=f"ups{sp}") for sp in range(SP)]
                    for hc in range(HC):
                        for sp in range(SP):
                            c0 = hc * SP * P + sp * P
                            for n in range(2):
                                sl = slice(n * 512, (n + 1) * 512)
                                nc.tensor.matmul(u_ps[sp][:, sl],
                                                 wsu_sb[:, c0:c0 + P],
                                                 xc[hc][:, sl],
                                                 start=(hc == 0),
                                                 stop=(hc == HC - 1))
                    for sp in range(SP):
                        nc.vector.tensor_mul(hs[sp][:], g_act[sp][:],
                                             u_ps[sp][:])
                with tc.tile_pool(name="psG3", bufs=8, space="PSUM") as psG3:
                    for tb_ in range(TB):
                        psd = [psG3.tile([P, 512], F32, tag="psGd",
                                         name=f"psGd{n}") for n in range(4)]
                        for sp in range(SP):
                            for n in range(4):
                                nc.tensor.matmul(
                                    psd[n][:],
                                    hs[sp][:, tb_ * P:(tb_ + 1) * P],
                                    wsd_sb[sp][:, n * 512:(n + 1) * 512],
                                    start=(sp == 0), stop=(sp == SP - 1))
                        sbd = pg2.tile([P, H], F16, tag="sbGd", bufs=2)
                        for n in range(4):
                            nc.vector.tensor_copy(
                                sbd[:, n * 512:(n + 1) * 512], psd[n][:])
                        nc.sync.dma_start(rs2_in[tb_ * P:(tb_ + 1) * P, :],
                                          sbd[:])

            # ---- experts ----
            for ei in range(2):
                with tc.tile_pool(name=f"pf{ei}", bufs=1) as pf, \
                     tc.tile_pool(name=f"pf2{ei}", bufs=2) as pf2:
                    idx_sb = [pf.tile([P, 1], I32, tag=f"idx{k}",
                                      name=f"idx{k}") for k in range(2)]
                    gxT = pf.tile([P, HC * 2 * P], F16)
                    wd_res = [pf.tile([P, H], F16, tag=f"wd{ip}",
                                      name=f"wd{ip}") for ip in range(IP)]
                    for ip in range(IP):
                        nc.sync.dma_start(
                            wd_res[ip][:],
                            ex["we_d"][ei, ip * P:(ip + 1) * P, :])
                    with tc.tile_pool(name=f"psF1{ei}", bufs=2,
                                      space="PSUM") as psF1:
                        for k in range(2):
                            nc.sync.dma_start(
                                idx_sb[k][:],
                                tok_lists[ei * CAP + k * P:
                                          ei * CAP + (k + 1) * P, :])
                            gx = pf2.tile([P, H], F16, tag="gx")
                            nc.vector.memset(gx[:], 0.0)
                            nc.gpsimd.indirect_dma_start(
                                out=gx[:], out_offset=None,
                                in_=x_tm[:],
                                in_offset=bass.IndirectOffsetOnAxis(
                                    ap=idx_sb[k][:, :1], axis=0),
                                bounds_check=T - 1, oob_is_err=False)
                            for hc in range(HC):
                                tp = psF1.tile([P, P], F16, tag="tpF")
                                nc.tensor.transpose(
                                    tp[:], gx[:, hc * P:(hc + 1) * P],
                                    identh[:])
                                nc.vector.tensor_copy(
                                    gxT[:, hc * 2 * P + k * P:
                                        hc * 2 * P + (k + 1) * P], tp[:])

                    # merged gate+up pass (8 PSUM banks)
                    g_tm = [pf.tile([P, I], F16, tag=f"gtm{k}", name=f"gtm{k}")
                            for k in range(2)]
                    h_tm = [pf.tile([P, I], F16, tag=f"htm{k}", name=f"htm{k}")
                            for k in range(2)]
                    with tc.tile_pool(name=f"psF2{ei}", bufs=1,
                                      space="PSUM") as psF2:
                        gu_ps = [[psF2.tile([P, 512], F32, tag=f"gups{k}{j}",
                                            name=f"gups{k}{j}")
                                  for j in range(4)] for k in range(2)]
                        for hc in range(HC):
                            wg_c = pf2.tile([P, I], F16, tag="wgF", bufs=3)
                            nc.sync.dma_start(
                                wg_c[:], ex["we_g"][ei, hc * P:(hc + 1) * P, :])
                            wu_c = pf2.tile([P, I], F16, tag="wuF", bufs=3)
                            nc.sync.dma_start(
                                wu_c[:], ex["we_u"][ei, hc * P:(hc + 1) * P, :])
                            for k in range(2):
                                s_ = gxT[:, hc * 2 * P + k * P:
                                         hc * 2 * P + (k + 1) * P]
                                for n in range(2):
                                    nc.tensor.matmul(
                                        gu_ps[k][n][:], s_,
                                        wg_c[:, n * 512:(n + 1) * 512],
                                        start=(hc == 0), stop=(hc == HC - 1))
                                for n in range(2):
                                    nc.tensor.matmul(
                                        gu_ps[k][2 + n][:], s_,
                                        wu_c[:, n * 512:(n + 1) * 512],
                                        start=(hc == 0), stop=(hc == HC - 1))
                        for k in range(2):
                            for n in range(2):
                                sl = slice(n * 512, (n + 1) * 512)
                                nc.scalar.activation(g_tm[k][:, sl],
                                                     gu_ps[k][n][:], AF.Silu)
                                nc.vector.tensor_mul(h_tm[k][:, sl],
                                                     g_tm[k][:, sl],
                                                     gu_ps[k][2 + n][:])
                    h_sb = [pf.tile([P, 2 * P], F16, tag=f"hsb{ip}",
                                    name=f"hsb{ip}") for ip in range(IP)]
                    with tc.tile_pool(name=f"psF4{ei}", bufs=2,
                                      space="PSUM") as psF4:
                        for k in range(2):
                            for ip in range(IP):
                                tp = psF4.tile([P, P], F16, tag="tpF2")
                                nc.tensor.transpose(
                                    tp[:], h_tm[k][:, ip * P:(ip + 1) * P],
                                    identh[:])
                                nc.vector.tensor_copy(
                                    h_sb[ip][:, k * P:(k + 1) * P], tp[:])
                    with tc.tile_pool(name=f"psF5{ei}", bufs=8,
                                      space="PSUM") as psF5:
                        for k in range(2):
                            wrow = pf2.tile([P, 2 * E], F32, tag="wrow")
                            nc.vector.memset(wrow[:], 0.0)
                            nc.gpsimd.indirect_dma_start(
                                out=wrow[:], out_offset=None, in_=w_all[:],
                                in_offset=bass.IndirectOffsetOnAxis(
                                    ap=idx_sb[k][:, :1], axis=0),
                                bounds_check=T - 1, oob_is_err=False)
                            we_ = pf2.tile([P, E], F32, tag="we_")
                            nc.vector.tensor_mul(we_[:], wrow[:, 0:E],
                                                 emask01[:, ei * E:(ei + 1) * E])
                            wg_own = pf2.tile([P, 1], F32, tag="wgown")
                            nc.vector.reduce_sum(wg_own[:], we_[:], axis=AX.X)
                            psd = [psF5.tile([P, 512], F32, tag="psFd",
                                             name=f"psFd{n}")
                                   for n in range(4)]
                            for ip in range(IP):
                                for n in range(4):
                                    nc.tensor.matmul(
                                        psd[n][:],
                                        h_sb[ip][:, k * P:(k + 1) * P],
                                        wd_res[ip][:, n * 512:(n + 1) * 512],
                                        start=(ip == 0), stop=(ip == IP - 1))
                            out_sb = pf.tile([P, H], F16, tag=f"outsb{k}")
                            for n in range(4):
                                nc.vector.tensor_scalar_mul(
                                    out_sb[:, n * 512:(n + 1) * 512],
                                    psd[n][:], wg_own[:, :1])
                            nc.gpsimd.indirect_dma_start(
                                out=rs2_in[:],
                                out_offset=bass.IndirectOffsetOnAxis(
                                    ap=idx_sb[k][:, :1], axis=0),
                                in_=out_sb[:], in_offset=None,
                                bounds_check=T - 1, oob_is_err=False,
                                compute_op=ALU.add)

        nc.gpsimd.collective_compute(
            "ReduceScatter", ALU.add, ins=[rs2_in.opt()], outs=[rs2_out.opt()],
            replica_groups=[list(range(NCN))])
        with tc.tile_pool(name="pz", bufs=2) as pz:
            fin16 = pz.tile([P, H], F16)
            nc.sync.dma_start(fin16[:], rs2_out[:])
            fin = pz.tile([P, H], F32)
            nc.vector.tensor_copy(fin[:], fin16[:])
            nc.sync.dma_start(out_slice[:], fin[:])


_CACHE = {}


def _build():
    key = "nc"
    if key in _CACHE:
        return _CACHE[key]
    nc = bacc.Bacc("TRN2", target_bir_lowering=False, debug=False,
                   num_devices=NCN)
    with tile.TileContext(nc) as tc:
        _emit(nc, tc)
    nc.compile()
    _CACHE[key] = nc
    return nc


def _host_prep(inputs):
    f16 = np.float16
    pos = np.asarray(inputs["positions"]).astype(np.float64)
    hid = np.asarray(inputs["hidden_states"], np.float32)
    w_in = np.asarray(inputs["w_in_ln"], np.float32)
    w_post = np.asarray(inputs["w_post_ln"], np.float32)
    wq = (np.asarray(inputs["wq"], np.float32) * w_in[:, None]).astype(f16)
    wk = (np.asarray(inputs["wk"], np.float32) * w_in[:, None]).astype(f16)
    wv = (np.asarray(inputs["wv"], np.float32) * w_in[:, None]).astype(f16)
    wo = np.asarray(inputs["wo"], np.float32).astype(f16)
    gate_w = np.asarray(inputs["gate_w"], np.float32) * w_post[None, :]
    gate_b = np.asarray(inputs["gate_bias"], np.float32).reshape(1, E)
    we_g = (np.asarray(inputs["we_gate"], np.float32)
            * w_post[None, :, None]).astype(f16)
    we_u = (np.asarray(inputs["we_up"], np.float32)
            * w_post[None, :, None]).astype(f16)
    we_d = np.asarray(inputs["we_down"], np.float32).astype(f16)
    ws_g = (np.asarray(inputs["ws_gate"], np.float32)
            * w_post[:, None]).astype(f16)
    ws_u = (np.asarray(inputs["ws_up"], np.float32)
            * w_post[:, None]).astype(f16)
    ws_d = np.asarray(inputs["ws_down"], np.float32).astype(f16)

    inv_freq = 1.0 / (THETA ** (np.arange(0, D, 2, dtype=np.float64) / D))
    f = pos[None, :] * inv_freq[:, None]
    cos2, sin2 = np.cos(f), np.sin(f)
    cosT = np.repeat(cos2, 2, axis=0).astype(np.float32)
    sinT = np.empty((D, T), np.float32)
    sinT[0::2] = -sin2
    sinT[1::2] = sin2
    s = 1.0 / np.sqrt(D)
    cosq, sinq = (cosT * s).astype(np.float32), (sinT * s).astype(np.float32)

    import ml_dtypes
    bf = ml_dtypes.bfloat16
    ii = np.arange(P)
    diag_mask = np.where(ii[:, None] >= ii[None, :], 0.0, NEG).astype(bf)

    identr_in = np.eye(P, dtype=np.float32)
    identh_in = np.eye(P, dtype=f16)
    ut_in = np.triu(np.ones((P, P), np.float32))
    slb_in = np.zeros((8, TB * P), np.float32)
    for b in range(TB):
        slb_in[:b, b * P:(b + 1) * P] = 1.0
    perm = np.zeros((P, P), np.float32)
    for i in range(0, P, 2):
        perm[i, i + 1] = 1.0
        perm[i + 1, i] = 1.0

    ISC = IS // NCN
    maps = []
    for c in range(NCN):
        g = c // 2
        emask01 = np.zeros((P, 2 * E), np.float32)
        emask01[:, 2 * c] = 1.0          # ei = 0 -> expert 2c
        emask01[:, E + 2 * c + 1] = 1.0  # ei = 1 -> expert 2c+1
        maps.append({
            "hid": hid,
            "hid_slice": np.ascontiguousarray(hid[c * P:(c + 1) * P]),
            "wq_s": np.ascontiguousarray(wq[:, 2 * c * D:(2 * c + 2) * D]),
            "wk_s": np.ascontiguousarray(wk[:, g * D:(g + 1) * D]),
            "wv_s": np.ascontiguousarray(wv[:, g * D:(g + 1) * D]),
            "wo_s": np.ascontiguousarray(wo[2 * c * D:(2 * c + 2) * D, :]),
            "cosq": cosq, "sinq": sinq, "cosk": cosT, "sink": sinT,
            "perm": perm, "diag_mask": diag_mask,
            "identr_in": identr_in, "identh_in": identh_in,
            "ut_in": ut_in, "slb_in": slb_in,
            "gate_wT": np.ascontiguousarray(gate_w.T),
            "gate_b": np.broadcast_to(gate_b, (P, E)).copy(),
            "emask01": emask01,
            "ws_g": np.ascontiguousarray(ws_g[:, c * ISC:(c + 1) * ISC]),
            "ws_u": np.ascontiguousarray(ws_u[:, c * ISC:(c + 1) * ISC]),
            "ws_d": np.ascontiguousarray(ws_d[c * ISC:(c + 1) * ISC, :]),
            "we_g": np.ascontiguousarray(we_g[2 * c:2 * c + 2]),
            "we_u": np.ascontiguousarray(we_u[2 * c:2 * c + 2]),
            "we_d": np.ascontiguousarray(we_d[2 * c:2 * c + 2]),
        })
    return maps


def kernel(trace=False, **inputs):
    nc = _build()
    maps = _host_prep(inputs)
    res = bass_utils.run_bass_kernel_spmd(
        nc, maps, core_ids=list(range(NCN)), trace=trace)
    out = np.concatenate([res.results[c]["out_slice"] for c in range(NCN)], 0)
    resid = np.concatenate([res.results[c]["res_slice"] for c in range(NCN)], 0)
    kernel.last_results = res
    return out, resid


# revision 15
# speedup vs baseline: 1.4552x; 1.0758x over previous
"""Ernie4 decoder layer (RMSNorm + GQA attention + shared expert + 16-expert
top-2 MoE) on 8 Trainium2 NeuronCores.

v2 — fp16 data path everywhere except the router (which must reproduce the
reference top-2 selection exactly; margins are ~3e-5 so it stays fp32 and is
computed locally per core before the AllGather):
  - Attention: head-parallel (2 q-heads + 1 kv-head per core), fp16 QKV /
    scores / probs / o_proj with causal-block skipping; fp16 ReduceScatter.
  - Router: fp32 logits on each core's own 128 tokens; W+sel AllGathered in a
    tiny fp32 collective that precedes the fp16 x AllGather so the token-list
    build overlaps it.
  - Shared expert: intermediate-sharded (IS/8 per core) fp16, output seeds
    the MoE combine buffer.
  - MoE: expert-parallel (2 experts per core), token lists via
    triangular-matmul prefix ranks, indirect-DMA gather/scatter-add in fp16,
    fp16 ReduceScatter for the combine.
"""
import sys
sys.path.insert(0, "/opt/trn_rl_repo")

import numpy as np

import concourse.bass as bass
import concourse.bacc as bacc
import concourse.tile as tile
import concourse.mybir as mybir
from concourse import bass_utils
from concourse.masks import make_identity
from concourse.tile import add_dep_helper

dt = mybir.dt
F32 = dt.float32
F32R = dt.float32r
F16 = dt.float16
I32 = dt.int32
BF16 = dt.bfloat16
AF = mybir.ActivationFunctionType
ALU = mybir.AluOpType
AX = mybir.AxisListType

T, H, NH, NKV, D = 1024, 2048, 16, 4, 128
E, I, IS = 16, 1024, 2048
EPS = 1e-6
THETA = 10000.0
NCN = 8
P = 128
TB = T // P            # 8 token blocks
HC = H // P            # 16 hidden chunks
IP = I // P            # 8 expert-intermediate chunks
SP = IS // NCN // P    # 2 shared-intermediate chunks per core
CAP = 256              # per-expert token capacity
BIG = 1.0e6            # OOB sentinel
NEG = -1e9


def _emit(nc, tc):
    ex = {}
    for name, shape, d in [
        ("hid", [T, H], F32), ("hid_slice", [P, H], F32),
        ("wq_s", [H, 2 * D], F16), ("wk_s", [H, D], F16), ("wv_s", [H, D], F16),
        ("wo_s", [2 * D, H], F16),
        ("cosq", [D, T], F32), ("sinq", [D, T], F32),
        ("cosk", [D, T], F32), ("sink", [D, T], F32),
        ("perm", [P, P], F32),
        ("diag_mask", [P, P], BF16),
        ("gate_wT", [H, E], F32), ("gate_b", [P, E], F32),
        ("emask01", [P, 2 * E], F32),
        ("ws_g", [H, SP * P], F16), ("ws_u", [H, SP * P], F16),
        ("ws_d", [SP * P, H], F16),
        ("we_g", [2, H, I], F16), ("we_u", [2, H, I], F16),
        ("we_d", [2, I, H], F16),
        ("identr_in", [P, P], F32), ("identh_in", [P, P], F16),
        ("ut_in", [P, P], F32),
        ("slb_in", [8, TB * P], F32),
    ]:
        ex[name] = nc.dram_tensor(name, shape, d, kind="ExternalInput").ap()
    out_slice = nc.dram_tensor("out_slice", [P, H], F32, kind="ExternalOutput").ap()
    res_slice = nc.dram_tensor("res_slice", [P, H], F32, kind="ExternalOutput").ap()
    dbg_w = nc.dram_tensor("dbg_w", [P, E], F32, kind="ExternalOutput").ap()

    with tc.tile_pool(name="persist", bufs=1) as pp, \
         tc.tile_pool(name="dram", bufs=1, space="DRAM") as dram:
        rs_in = dram.tile([T, H], F16)
        rs_out = dram.tile([P, H], F16)
        agw_in = dram.tile([P, 2 * E], F32)
        w_all = dram.tile([T, 2 * E], F32, addr_space="Shared")
        agx_in = dram.tile([P, H], F16)
        x_tm = dram.tile([T, H], F16, addr_space="Shared")
        tok_lists = dram.tile([2 * CAP, 1], I32)
        rs2_in = dram.tile([T, H], F16)
        rs2_out = dram.tile([P, H], F16)

        ident = pp.tile([P, P], F32)
        make_identity(nc, ident[:])
        identr = pp.tile([P, P], F32R)
        nc.sync.dma_start(identr[:], ex["identr_in"][:].bitcast(F32R))
        identh = pp.tile([P, P], F16)
        nc.sync.dma_start(identh[:], ex["identh_in"][:])
        hid_sl = pp.tile([P, H], F32)
        nc.sync.dma_start(hid_sl[:], ex["hid_slice"][:])
        eps_t = pp.tile([P, 1], F32)
        nc.vector.memset(eps_t[:], EPS)
        emask01 = pp.tile([P, 2 * E], F32)
        nc.sync.dma_start(emask01[:], ex["emask01"][:])
        sent = pp.tile([P, 1], I32)
        nc.vector.memset(sent[:], 1000000)
        for k in range(2 * CAP // P):
            nc.sync.dma_start(tok_lists[k * P:(k + 1) * P, :], sent[:])

        # ======== Phases A-C: attention (fp16) ========
        with tc.tile_pool(name="pab", bufs=1) as pab:
            qT = [pab.tile([P, T], F16, tag=f"qT{j}", name=f"qT{j}")
                  for j in range(2)]
            kT = pab.tile([P, T], F16)
            vT = pab.tile([P, T], F16)
            v_tm = [pab.tile([P, D], F16, tag=f"vtm{b}", name=f"vtm{b}")
                    for b in range(TB)]
            oT = [pab.tile([P, T], F16, tag=f"oT{j}", name=f"oT{j}")
                  for j in range(2)]

            # ---- A: norm + transpose + QKV + rope ----
            with tc.tile_pool(name="pa", bufs=1) as pa, \
                 tc.tile_pool(name="pa2", bufs=3) as pa2:
                cosq = pa.tile([D, T], F32)
                sinq = pa.tile([D, T], F32)
                cosk = pa.tile([D, T], F32)
                sink = pa.tile([D, T], F32)
                for t_, s_ in [(cosq, "cosq"), (sinq, "sinq"),
                               (cosk, "cosk"), (sink, "sink")]:
                    nc.sync.dma_start(t_[:], ex[s_][:])
                permr = pa.tile([P, P], F32R)
                nc.sync.dma_start(permr[:], ex["perm"][:].bitcast(F32R))
                wq_sb = pa.tile([P, HC * 2 * D], F16)
                wk_sb = pa.tile([P, HC * D], F16)
                wv_sb = pa.tile([P, HC * D], F16)
                for t_, s_, m in [(wq_sb, "wq_s", 2 * D), (wk_sb, "wk_s", D),
                                  (wv_sb, "wv_s", D)]:
                    nc.sync.dma_start(
                        t_[:].rearrange("p (hc m) -> p hc m", hc=HC),
                        ex[s_][:].rearrange("(hc p) m -> p hc m", p=P))

                dump = pa.tile([P, H], F32)
                qraw = [pa.tile([P, T], F32R, tag=f"qraw{j}", name=f"qraw{j}")
                        for j in range(2)]
                kraw = pa.tile([P, T], F32R)
                with tc.tile_pool(name="psA1", bufs=2, space="PSUM") as psA1, \
                     tc.tile_pool(name="psA2", bufs=2, space="PSUM") as psA2:
                    for n in range(2):
                        x0T = [pa.tile([P, 512], F16, tag=f"x0T{hc}",
                                       name=f"x0T{hc}_{n}") for hc in range(HC)]
                        for bb in range(TB // 2):
                            b = n * (TB // 2) + bb
                            hidb = pa2.tile([P, H], F32, tag="hidb", bufs=2)
                            nc.sync.dma_start(hidb[:],
                                              ex["hid"][b * P:(b + 1) * P, :])
                            ssum = pa2.tile([P, 1], F32, tag="ssum")
                            nc.scalar.activation(dump[:], hidb[:], AF.Square,
                                                 accum_out=ssum[:, :1])
                            rms = pa2.tile([P, 1], F32, tag="rms")
                            nc.scalar.activation(rms[:], ssum[:],
                                                 AF.Sqrt, bias=eps_t[:, :1],
                                                 scale=1.0 / H)
                            inv = pa2.tile([P, 1], F32, tag="inv")
                            nc.vector.reciprocal(inv[:], rms[:])
                            x0b = pa2.tile([P, H], F16, tag="x0b", bufs=2)
                            nc.vector.tensor_scalar_mul(x0b[:], hidb[:],
                                                        inv[:, :1])
                            for hc in range(HC):
                                tp = psA1.tile([P, P], F16, tag="tpA")
                                nc.tensor.transpose(
                                    tp[:], x0b[:, hc * P:(hc + 1) * P],
                                    identh[:])
                                nc.vector.tensor_copy(
                                    x0T[hc][:, bb * P:(bb + 1) * P], tp[:])

                        def proj(w_sb, m, c0, dst, n=n, x0T=x0T, fp16=False):
                            ps = psA2.tile([P, 512], F32, tag="psQKV",
                                           name="psQKV")
                            for hc in range(HC):
                                nc.tensor.matmul(
                                    ps[:],
                                    w_sb[:, hc * m + c0:hc * m + c0 + P],
                                    x0T[hc][:],
                                    start=(hc == 0), stop=(hc == HC - 1))
                            nc.vector.tensor_copy(
                                dst[:, n * 512:(n + 1) * 512], ps[:])
                        proj(wq_sb, 2 * D, 0, qraw[0])
                        proj(wq_sb, 2 * D, D, qraw[1])
                        proj(wk_sb, D, 0, kraw)
                        proj(wv_sb, D, 0, vT, fp16=True)

                with tc.tile_pool(name="psA3", bufs=2, space="PSUM") as psA3:
                    for src, dst, c_, s_ in [(qraw[0], qT[0], cosq, sinq),
                                             (qraw[1], qT[1], cosq, sinq),
                                             (kraw, kT, cosk, sink)]:
                        for n in range(2):
                            sl = slice(n * 512, (n + 1) * 512)
                            sw = psA3.tile([P, 512], F32, tag="psSW")
                            nc.tensor.matmul(sw[:], permr[:], src[:, sl],
                                             start=True, stop=True)
                            t1 = pa2.tile([P, 512], F32, tag="ropeT1")
                            nc.vector.tensor_mul(t1[:], src[:, sl], c_[:, sl])
                            t2 = pa2.tile([P, 512], F32, tag="ropeT2")
                            nc.vector.tensor_mul(t2[:], sw[:], s_[:, sl])
                            nc.vector.tensor_add(dst[:, sl], t1[:], t2[:])
                    for b in range(TB):
                        tp = psA3.tile([P, P], F16, tag="tpV")
                        nc.tensor.transpose(tp[:], vT[:, b * P:(b + 1) * P],
                                            identh[:])
                        nc.vector.tensor_copy(v_tm[b][:], tp[:])

            # ---- B: attention (causal-block skipped) ----
            with tc.tile_pool(name="pb", bufs=1) as pb, \
                 tc.tile_pool(name="pb2", bufs=3) as pb2:
                dmask = pb.tile([P, P], BF16)
                nc.sync.dma_start(dmask[:], ex["diag_mask"][:])
                wo_sb = [pb.tile([P, H], F16, tag=f"wo{j}", name=f"wo{j}")
                         for j in range(2)]
                nc.sync.dma_start(wo_sb[0][:], ex["wo_s"][0:P, :])
                nc.sync.dma_start(wo_sb[1][:], ex["wo_s"][P:2 * P, :])

                attnT = [pb.tile([P, T], F16, tag=f"attnT{kc}",
                                 name=f"attnT{kc}") for kc in range(TB)]
                for kc in range(1, TB):
                    nc.vector.memset(attnT[kc][:, 0:kc * P], 0.0)
                with tc.tile_pool(name="psB1", bufs=2, space="PSUM") as psB1, \
                     tc.tile_pool(name="psB2", bufs=2, space="PSUM") as psB2, \
                     tc.tile_pool(name="psB3", bufs=2, space="PSUM") as psB3:
                  for h in range(2):
                    for qc in range(TB):
                        cols = (qc + 1) * P
                        prob = pb2.tile([P, T], F32, tag="prob")
                        nsl = (cols + 511) // 512
                        for n in range(nsl):
                            w_ = min(512, cols - n * 512)
                            ps = psB1.tile([P, 512], F32, tag="psSC")
                            nc.tensor.matmul(ps[:, :w_],
                                             qT[h][:, qc * P:(qc + 1) * P],
                                             kT[:, n * 512:n * 512 + w_],
                                             start=True, stop=True)
                            # diagonal block gets the causal mask; the rest
                            # of this slice is fully visible
                            d0 = qc * P - n * 512
                            if 0 <= d0 < w_:
                                if d0 > 0:
                                    nc.vector.tensor_copy(
                                        prob[:, n * 512:n * 512 + d0],
                                        ps[:, :d0])
                                nc.vector.tensor_add(
                                    prob[:, qc * P:qc * P + P],
                                    ps[:, d0:d0 + P], dmask[:])
                            else:
                                nc.vector.tensor_copy(
                                    prob[:, n * 512:n * 512 + w_], ps[:, :w_])
                        mx = pb2.tile([P, 1], F32, tag="mx")
                        nc.vector.reduce_max(mx[:], prob[:, :cols], axis=AX.X)
                        negm = pb2.tile([P, 1], F32, tag="negm")
                        nc.vector.tensor_scalar_mul(negm[:], mx[:], -1.0)
                        ssum = pb2.tile([P, 1], F32, tag="esum")
                        probe_ = pb2.tile([P, T], F32, tag="probe")
                        nc.scalar.activation(probe_[:, :cols], prob[:, :cols],
                                             AF.Exp, bias=negm[:, :1],
                                             accum_out=ssum[:, :1])
                        rec = pb2.tile([P, 1], F32, tag="rec")
                        nc.vector.reciprocal(rec[:], ssum[:])
                        probS = pb2.tile([P, T], F16, tag="probS")
                        nc.vector.tensor_scalar_mul(probS[:, :cols],
                                                    probe_[:, :cols],
                                                    rec[:, :1])
                        for kc in range(qc + 1):
                            tp = psB2.tile([P, P], F16, tag="tpB")
                            nc.tensor.transpose(
                                tp[:], probS[:, kc * P:(kc + 1) * P],
                                identh[:])
                            nc.vector.tensor_copy(
                                attnT[kc][:, qc * P:(qc + 1) * P], tp[:])
                    for n in range(2):
                        sl = slice(n * 512, (n + 1) * 512)
                        kc_hi = 4 * n + 3
                        ps = psB3.tile([P, 512], F32, tag="psAV")
                        for kc in range(kc_hi + 1):
                            nc.tensor.matmul(ps[:], v_tm[kc][:],
                                             attnT[kc][:, sl],
                                             start=(kc == 0),
                                             stop=(kc == kc_hi))
                        nc.vector.tensor_copy(oT[h][:, sl], ps[:])

                # ---- C: o_proj ----
                with tc.tile_pool(name="psC", bufs=8, space="PSUM") as psC:
                    for tb_ in range(TB):
                        pso = [psC.tile([P, 512], F32, tag="psO",
                                        name=f"psO{n}") for n in range(4)]
                        for hp in range(2):
                            for n in range(4):
                                nc.tensor.matmul(
                                    pso[n][:],
                                    oT[hp][:, tb_ * P:(tb_ + 1) * P],
                                    wo_sb[hp][:, n * 512:(n + 1) * 512],
                                    start=(hp == 0), stop=(hp == 1))
                        ob = pb2.tile([P, H], F16, tag="ob", bufs=2)
                        for n in range(4):
                            nc.vector.tensor_copy(
                                ob[:, n * 512:(n + 1) * 512], pso[n][:])
                        nc.sync.dma_start(rs_in[tb_ * P:(tb_ + 1) * P, :],
                                          ob[:])

        nc.gpsimd.collective_compute(
            "ReduceScatter", ALU.add, ins=[rs_in.opt()], outs=[rs_out.opt()],
            replica_groups=[list(range(NCN))])

        # ======== D: residual + norm + local fp32 router + AGs ========
        with tc.tile_pool(name="pd", bufs=1) as pd, \
             tc.tile_pool(name="pd2", bufs=2) as pd2, \
             tc.tile_pool(name="psD", bufs=2, space="PSUM") as psD:
            attn_sl = pd.tile([P, H], F16)
            nc.sync.dma_start(attn_sl[:], rs_out[:])
            res_sb = pd.tile([P, H], F32)
            nc.vector.tensor_add(res_sb[:], hid_sl[:], attn_sl[:])
            nc.sync.dma_start(res_slice[:], res_sb[:])
            dump2 = pd.tile([P, H], F32)
            ssum = pd.tile([P, 1], F32)
            nc.scalar.activation(dump2[:], res_sb[:], AF.Square,
                                 accum_out=ssum[:, :1])
            rms = pd.tile([P, 1], F32)
            nc.scalar.activation(rms[:], ssum[:], AF.Sqrt, bias=eps_t[:, :1],
                                 scale=1.0 / H)
            inv = pd.tile([P, 1], F32)
            nc.vector.reciprocal(inv[:], rms[:])
            x_sl = pd.tile([P, H], F32)
            nc.vector.tensor_scalar_mul(x_sl[:], res_sb[:], inv[:, :1])
            x_sl_h = pd.tile([P, H], F16)
            nc.vector.tensor_copy(x_sl_h[:], x_sl[:])
            nc.sync.dma_start(agx_in[:], x_sl_h[:])

            # fp32 transpose of own slice for exact router logits
            gw_sb = pd.tile([P, HC * E], F32)
            nc.sync.dma_start(
                gw_sb[:].rearrange("p (hc e) -> p hc e", hc=HC),
                ex["gate_wT"][:].rearrange("(hc p) e -> p hc e", p=P))
            gate_b = pd.tile([P, E], F32)
            nc.sync.dma_start(gate_b[:], ex["gate_b"][:])
            x_slT = pd.tile([P, H], F32)
            for hc in range(HC):
                tp = psD.tile([P, P], F32, tag="tpD")
                nc.tensor.transpose(tp[:], x_sl[:, hc * P:(hc + 1) * P],
                                    ident[:])
                nc.vector.tensor_copy(x_slT[:, hc * P:(hc + 1) * P], tp[:])
            lg_ps = psD.tile([P, E], F32, tag="lgps", name="lgps")
            for hc in range(HC):
                nc.tensor.matmul(lg_ps[:], x_slT[:, hc * P:(hc + 1) * P],
                                 gw_sb[:, hc * E:(hc + 1) * E],
                                 start=(hc == 0), stop=(hc == HC - 1))
            sig = pd2.tile([P, E], F32, tag="sig")
            nc.scalar.activation(sig[:], lg_ps[:], AF.Sigmoid)
            sb_ = pd2.tile([P, E], F32, tag="sb_")
            nc.vector.tensor_add(sb_[:], sig[:], gate_b[:])
            mx = pd2.tile([P, 8], F32, tag="mx8")
            nc.vector.max(out=mx[:], in_=sb_[:])
            s1 = pd2.tile([P, E], F32, tag="s1")
            nc.vector.tensor_tensor(out=s1[:], in0=sb_[:],
                                    in1=mx[:, 0:1].to_broadcast([P, E]),
                                    op=ALU.is_equal)
            s2 = pd2.tile([P, E], F32, tag="s2")
            nc.vector.tensor_tensor(out=s2[:], in0=sb_[:],
                                    in1=mx[:, 1:2].to_broadcast([P, E]),
                                    op=ALU.is_equal)
            nc.vector.tensor_add(s1[:], s1[:], s2[:])
            sel_own = pd2.tile([P, E], F32, tag="sel_own")
            nc.vector.tensor_scalar_min(sel_own[:], s1[:], 1.0)
            wa = pd2.tile([P, E], F32, tag="wa")
            nc.vector.tensor_mul(wa[:], sel_own[:], sig[:])
            nrm = pd2.tile([P, 1], F32, tag="nrm")
            nc.vector.reduce_sum(nrm[:], wa[:], axis=AX.X)
            rec = pd2.tile([P, 1], F32, tag="recw")
            nc.vector.reciprocal(rec[:], nrm[:])
            w_tm = pd2.tile([P, E], F32, tag="wtm")
            nc.vector.tensor_scalar_mul(w_tm[:], wa[:], rec[:, :1])
            nc.sync.dma_start(agw_in[:, 0:E], w_tm[:])
            nc.sync.dma_start(agw_in[:, E:2 * E], sel_own[:])
            nc.sync.dma_start(dbg_w[:], w_tm[:])

        cc_w = nc.gpsimd.collective_compute(
            "AllGather", ALU.bypass, ins=[agw_in.opt()], outs=[w_all.opt()],
            replica_groups=[list(range(NCN))])
        cc_x = nc.gpsimd.collective_compute(
            "AllGather", ALU.bypass, ins=[agx_in.opt()], outs=[x_tm.opt()],
            replica_groups=[list(range(NCN))])
        # The tiny router AllGather must run first so the token-list build
        # overlaps the big x AllGather (CC queue executes in trigger order).
        add_dep_helper(cc_x.ins, cc_w.ins, sync=True,
                       reason="AG_W before AG_x")

        # ======== E: token lists from AllGathered router decisions ========
        with tc.tile_pool(name="pe", bufs=1) as pe, \
             tc.tile_pool(name="pe2", bufs=3) as pe2, \
             tc.tile_pool(name="psE", bufs=2, space="PSUM") as psE:
            ut = pe.tile([P, P], F32R)
            nc.sync.dma_start(ut[:], ex["ut_in"][:].bitcast(F32R))
            slb = pe.tile([8, TB * P], F32R)
            nc.sync.dma_start(slb[:], ex["slb_in"][:].bitcast(F32R))
            totals = pe.tile([8, E], F32R)
            pre_sb = [pe.tile([P, E], F32, tag=f"pre{b}", name=f"pre{b}")
                      for b in range(TB)]
            sel_all = [pe.tile([P, E], F32, tag=f"sela{b}", name=f"sela{b}")
                       for b in range(TB)]
            for b in range(TB):
                nc.sync.dma_start(sel_all[b][:],
                                  w_all[b * P:(b + 1) * P, E:2 * E])
                pr_ps = psE.tile([P, E], F32, tag="prps")
                nc.tensor.matmul(pr_ps[:], ut[:],
                                 sel_all[b][:].bitcast(F32R),
                                 start=True, stop=True)
                nc.vector.tensor_copy(pre_sb[b][:], pr_ps[:])
                nc.sync.dma_start(totals[b:b + 1, :],
                                  pre_sb[b][127:128, :].bitcast(F32R))
            for b in range(TB):
                ofs_ps = psE.tile([P, E], F32, tag="ofsps", name="ofsps")
                nc.tensor.matmul(ofs_ps[:], slb[:, b * P:(b + 1) * P],
                                 totals[:], start=True, stop=True)
                grank = pe2.tile([P, E], F32, tag="grank")
                nc.vector.tensor_add(grank[:], pre_sb[b][:], ofs_ps[:])
                nc.vector.tensor_scalar_add(grank[:], grank[:], -1.0)
                gm = pe2.tile([P, E], F32, tag="gm")
                nc.vector.tensor_scalar(out=gm[:], in0=grank[:],
                                        scalar1=float(CAP - 1), scalar2=BIG,
                                        op0=ALU.is_gt, op1=ALU.mult)
                nc.vector.tensor_add(grank[:], grank[:], gm[:])
                um = pe2.tile([P, E], F32, tag="um")
                nc.vector.tensor_scalar(out=um[:], in0=sel_all[b][:],
                                        scalar1=-BIG, scalar2=BIG,
                                        op0=ALU.mult, op1=ALU.add)
                nc.vector.tensor_add(grank[:], grank[:], um[:])
                tok = pe2.tile([P, 1], I32, tag="tok")
                nc.gpsimd.iota(tok[:], pattern=[[0, 1]], base=b * P,
                               channel_multiplier=1)
                for ei in range(2):
                    ge = pe2.tile([P, E], F32, tag="ge")
                    nc.vector.tensor_mul(ge[:], grank[:],
                                         emask01[:, ei * E:(ei + 1) * E])
                    ridx = pe2.tile([P, 1], F32, tag="ridx")
                    nc.vector.reduce_sum(ridx[:], ge[:], axis=AX.X)
                    nc.vector.tensor_scalar_add(ridx[:], ridx[:],
                                                float(ei * CAP))
                    ridx_i = pe2.tile([P, 1], I32, tag="ridxi")
                    nc.vector.tensor_copy(ridx_i[:], ridx[:])
                    nc.gpsimd.indirect_dma_start(
                        out=tok_lists[:],
                        out_offset=bass.IndirectOffsetOnAxis(
                            ap=ridx_i[:, :1], axis=0),
                        in_=tok[:], in_offset=None,
                        bounds_check=2 * CAP - 1, oob_is_err=False)

        # ======== F: xT + shared expert + experts (fp16) ========
        with tc.tile_pool(name="pxt", bufs=1) as pxt, \
             tc.tile_pool(name="pfs", bufs=1) as pfs, \
             tc.tile_pool(name="pfs2", bufs=2) as pfs2:
            xc = [pxt.tile([P, T], F16, tag=f"xc{hc}", name=f"xc{hc}")
                  for hc in range(HC)]
            with tc.tile_pool(name="pxt2", bufs=3) as pxt2, \
                 tc.tile_pool(name="psX", bufs=2, space="PSUM") as psX:
                for b in range(TB):
                    xb = pxt2.tile([P, H], F16, tag="xb", bufs=2)
                    nc.sync.dma_start(xb[:], x_tm[b * P:(b + 1) * P, :])
                    for hc in range(HC):
                        tp = psX.tile([P, P], F16, tag="tpX")
                        nc.tensor.transpose(tp[:], xb[:, hc * P:(hc + 1) * P],
                                            identh[:])
                        nc.vector.tensor_copy(xc[hc][:, b * P:(b + 1) * P],
                                              tp[:])

            # ---- both experts' setup: token lists, gathers, gxT, weights,
            # per-token gate weights — overlaps the shared expert below ----
            idx_sb2 = [[pfs.tile([P, 1], I32, tag=f"idx{ei}{k}",
                                 name=f"idx{ei}{k}") for k in range(2)]
                       for ei in range(2)]
            gxT2 = [pfs.tile([P, HC * 2 * P], F16, tag=f"gxT{ei}",
                             name=f"gxT{ei}") for ei in range(2)]
            wd_res2 = [[pfs.tile([P, H], F16, tag=f"wd{ei}{ip}",
                                 name=f"wd{ei}{ip}") for ip in range(IP)]
                       for ei in range(2)]
            wg_own2 = [[pfs.tile([P, 1], F32, tag=f"wgo{ei}{k}",
                                 name=f"wgo{ei}{k}") for k in range(2)]
                       for ei in range(2)]
            psS_cm = tc.tile_pool(name="psS", bufs=2, space="PSUM")
            psS = psS_cm.__enter__()
            for ei in range(2):
                for ip in range(IP):
                    nc.sync.dma_start(wd_res2[ei][ip][:],
                                      ex["we_d"][ei, ip * P:(ip + 1) * P, :])
                for k in range(2):
                    nc.sync.dma_start(
                        idx_sb2[ei][k][:],
                        tok_lists[ei * CAP + k * P:ei * CAP + (k + 1) * P, :])
                    gx = pfs2.tile([P, H], F16, tag="gx")
                    nc.vector.memset(gx[:], 0.0)
                    nc.gpsimd.indirect_dma_start(
                        out=gx[:], out_offset=None,
                        in_=x_tm[:],
                        in_offset=bass.IndirectOffsetOnAxis(
                            ap=idx_sb2[ei][k][:, :1], axis=0),
                        bounds_check=T - 1, oob_is_err=False)
                    for hc in range(HC):
                        tp = psS.tile([P, P], F16, tag="tpS")
                        nc.tensor.transpose(tp[:], gx[:, hc * P:(hc + 1) * P],
                                            identh[:])
                        nc.vector.tensor_copy(
                            gxT2[ei][:, hc * 2 * P + k * P:
                                  hc * 2 * P + (k + 1) * P], tp[:])
                    wrow = pfs2.tile([P, 2 * E], F32, tag="wrow")
                    nc.vector.memset(wrow[:], 0.0)
                    nc.gpsimd.indirect_dma_start(
                        out=wrow[:], out_offset=None, in_=w_all[:],
                        in_offset=bass.IndirectOffsetOnAxis(
                            ap=idx_sb2[ei][k][:, :1], axis=0),
                        bounds_check=T - 1, oob_is_err=False)
                    we_ = pfs2.tile([P, E], F32, tag="we_")
                    nc.vector.tensor_mul(we_[:], wrow[:, 0:E],
                                         emask01[:, ei * E:(ei + 1) * E])
                    nc.vector.reduce_sum(wg_own2[ei][k][:], we_[:], axis=AX.X)

            # ---- shared expert ----
            with tc.tile_pool(name="pg", bufs=1) as pg, \
                 tc.tile_pool(name="pg2", bufs=3) as pg2:
                wsg_sb = pg.tile([P, HC * SP * P], F16)
                wsu_sb = pg.tile([P, HC * SP * P], F16)
                for t_, s_ in [(wsg_sb, "ws_g"), (wsu_sb, "ws_u")]:
                    nc.sync.dma_start(
                        t_[:].rearrange("p (hc m) -> p hc m", hc=HC),
                        ex[s_][:].rearrange("(hc p) m -> p hc m", p=P))
                wsd_sb = [pg.tile([P, H], F16, tag=f"wsd{sp}", name=f"wsd{sp}")
                          for sp in range(SP)]
                for sp in range(SP):
                    nc.sync.dma_start(wsd_sb[sp][:],
                                      ex["ws_d"][sp * P:(sp + 1) * P, :])
                g_act = [pg.tile([P, T], F16, tag=f"gact{sp}", name=f"gact{sp}")
                         for sp in range(SP)]
                hs = [pg.tile([P, T], F16, tag=f"hs{sp}", name=f"hs{sp}")
                      for sp in range(SP)]
                with tc.tile_pool(name="psG1", bufs=1, space="PSUM") as psG1:
                    g_ps = [psG1.tile([P, T], F32, tag=f"gps{sp}",
                                      name=f"gps{sp}") for sp in range(SP)]
                    for hc in range(HC):
                        for sp in range(SP):
                            c0 = hc * SP * P + sp * P
                            for n in range(2):
                                sl = slice(n * 512, (n + 1) * 512)
                                nc.tensor.matmul(g_ps[sp][:, sl],
                                                 wsg_sb[:, c0:c0 + P],
                                                 xc[hc][:, sl],
                                                 start=(hc == 0),
                                                 stop=(hc == HC - 1))
                    for sp in range(SP):
                        nc.scalar.activation(g_act[sp][:], g_ps[sp][:],
                                             AF.Silu)
                with tc.tile_pool(name="psG2", bufs=1, space="PSUM") as psG2:
                    u_ps = [psG2.tile([P, T], F32, tag=f"ups{sp}",
                                      name=f"ups{sp}") for sp in range(SP)]
                    for hc in range(HC):
                        for sp in range(SP):
                            c0 = hc * SP * P + sp * P
                            for n in range(2):
                                sl = slice(n * 512, (n + 1) * 512)
                                nc.tensor.matmul(u_ps[sp][:, sl],
                                                 wsu_sb[:, c0:c0 + P],
                                                 xc[hc][:, sl],
                                                 start=(hc == 0),
                                                 stop=(hc == HC - 1))
                    for sp in range(SP):
                        nc.vector.tensor_mul(hs[sp][:], g_act[sp][:],
                                             u_ps[sp][:])
                with tc.tile_pool(name="psG3", bufs=6, space="PSUM") as psG3:
                    for tb_ in range(TB):
                        psd = [psG3.tile([P, 512], F32, tag="psGd",
                                         name=f"psGd{n}") for n in range(4)]
                        for sp in range(SP):
                            for n in range(4):
                                nc.tensor.matmul(
                                    psd[n][:],
                                    hs[sp][:, tb_ * P:(tb_ + 1) * P],
                                    wsd_sb[sp][:, n * 512:(n + 1) * 512],
                                    start=(sp == 0), stop=(sp == SP - 1))
                        sbd = pg2.tile([P, H], F16, tag="sbGd", bufs=2)
                        for n in range(4):
                            nc.vector.tensor_copy(
                                sbd[:, n * 512:(n + 1) * 512], psd[n][:])
                        nc.sync.dma_start(rs2_in[tb_ * P:(tb_ + 1) * P, :],
                                          sbd[:])

            psS_cm.__exit__(None, None, None)

            # ---- experts (setup already done above) ----
            for ei in range(2):
                with tc.tile_pool(name=f"pf{ei}", bufs=1) as pf, \
                     tc.tile_pool(name=f"pf2{ei}", bufs=2) as pf2:
                    idx_sb = idx_sb2[ei]
                    gxT = gxT2[ei]
                    wd_res = wd_res2[ei]

                    # merged gate+up pass (8 PSUM banks)
                    g_tm = [pf.tile([P, I], F16, tag=f"gtm{k}", name=f"gtm{k}")
                            for k in range(2)]
                    h_tm = [pf.tile([P, I], F16, tag=f"htm{k}", name=f"htm{k}")
                            for k in range(2)]
                    with tc.tile_pool(name=f"psF2{ei}", bufs=1,
                                      space="PSUM") as psF2:
                        gu_ps = [[psF2.tile([P, 512], F32, tag=f"gups{k}{j}",
                                            name=f"gups{k}{j}")
                                  for j in range(4)] for k in range(2)]
                        for hc in range(HC):
                            wg_c = pf2.tile([P, I], F16, tag="wgF", bufs=3)
                            nc.sync.dma_start(
                                wg_c[:], ex["we_g"][ei, hc * P:(hc + 1) * P, :])
                            wu_c = pf2.tile([P, I], F16, tag="wuF", bufs=3)
                            nc.sync.dma_start(
                                wu_c[:], ex["we_u"][ei, hc * P:(hc + 1) * P, :])
                            for k in range(2):
                                s_ = gxT[:, hc * 2 * P + k * P:
                                         hc * 2 * P + (k + 1) * P]
                                for n in range(2):
                                    nc.tensor.matmul(
                                        gu_ps[k][n][:], s_,
                                        wg_c[:, n * 512:(n + 1) * 512],
                                        start=(hc == 0), stop=(hc == HC - 1))
                                for n in range(2):
                                    nc.tensor.matmul(
                                        gu_ps[k][2 + n][:], s_,
                                        wu_c[:, n * 512:(n + 1) * 512],
                                        start=(hc == 0), stop=(hc == HC - 1))
                        for k in range(2):
                            for n in range(2):
                                sl = slice(n * 512, (n + 1) * 512)
                                nc.scalar.activation(g_tm[k][:, sl],
                                                     gu_ps[k][n][:], AF.Silu)
                                nc.vector.tensor_mul(h_tm[k][:, sl],
                                                     g_tm[k][:, sl],
                                                     gu_ps[k][2 + n][:])
                    h_sb = [pf.tile([P, 2 * P], F16, tag=f"hsb{ip}",
                                    name=f"hsb{ip}") for ip in range(IP)]
                    with tc.tile_pool(name=f"psF4{ei}", bufs=2,
                                      space="PSUM") as psF4:
                        for k in range(2):
                            for ip in range(IP):
                                tp = psF4.tile([P, P], F16, tag="tpF2")
                                nc.tensor.transpose(
                                    tp[:], h_tm[k][:, ip * P:(ip + 1) * P],
                                    identh[:])
                                nc.vector.tensor_copy(
                                    h_sb[ip][:, k * P:(k + 1) * P], tp[:])
                    with tc.tile_pool(name=f"psF5{ei}", bufs=8,
                                      space="PSUM") as psF5:
                        for k in range(2):
                            psd = [psF5.tile([P, 512], F32, tag="psFd",
                                             name=f"psFd{n}")
                                   for n in range(4)]
                            for ip in range(IP):
                                for n in range(4):
                                    nc.tensor.matmul(
                                        psd[n][:],
                                        h_sb[ip][:, k * P:(k + 1) * P],
                                        wd_res[ip][:, n * 512:(n + 1) * 512],
                                        start=(ip == 0), stop=(ip == IP - 1))
                            out_sb = pf.tile([P, H], F16, tag=f"outsb{k}")
                            for n in range(4):
                                nc.vector.tensor_scalar_mul(
                                    out_sb[:, n * 512:(n + 1) * 512],
                                    psd[n][:], wg_own2[ei][k][:, :1])
                            nc.gpsimd.indirect_dma_start(
                                out=rs2_in[:],
                                out_offset=bass.IndirectOffsetOnAxis(
                                    ap=idx_sb[k][:, :1], axis=0),
                                in_=out_sb[:], in_offset=None,
                                bounds_check=T - 1, oob_is_err=False,
                                compute_op=ALU.add)

        nc.gpsimd.collective_compute(
            "ReduceScatter", ALU.add, ins=[rs2_in.opt()], outs=[rs2_out.opt()],
            replica_groups=[list(range(NCN))])
        with tc.tile_pool(name="pz", bufs=2) as pz:
            fin16 = pz.tile([P, H], F16)
            nc.sync.dma_start(fin16[:], rs2_out[:])
            fin = pz.tile([P, H], F32)
            nc.vector.tensor_copy(fin[:], fin16[:])
            nc.sync.dma_start(out_slice[:], fin[:])


_CACHE = {}


def _build():
    key = "nc"
    if key in _CACHE:
        return _CACHE[key]
    nc = bacc.Bacc("TRN2", target_bir_lowering=False, debug=False,
                   num_devices=NCN)
    with tile.TileContext(nc) as tc:
        _emit(nc, tc)
    nc.compile()
    _CACHE[key] = nc
    return nc


def _host_prep(inputs):
    f16 = np.float16
    pos = np.asarray(inputs["positions"]).astype(np.float64)
    hid = np.asarray(inputs["hidden_states"], np.float32)
    w_in = np.asarray(inputs["w_in_ln"], np.float32)
    w_post = np.asarray(inputs["w_post_ln"], np.float32)
    wq = (np.asarray(inputs["wq"], np.float32) * w_in[:, None]).astype(f16)
    wk = (np.asarray(inputs["wk"], np.float32) * w_in[:, None]).astype(f16)
    wv = (np.asarray(inputs["wv"], np.float32) * w_in[:, None]).astype(f16)
    wo = np.asarray(inputs["wo"], np.float32).astype(f16)
    gate_w = np.asarray(inputs["gate_w"], np.float32) * w_post[None, :]
    gate_b = np.asarray(inputs["gate_bias"], np.float32).reshape(1, E)
    we_g = (np.asarray(inputs["we_gate"], np.float32)
            * w_post[None, :, None]).astype(f16)
    we_u = (np.asarray(inputs["we_up"], np.float32)
            * w_post[None, :, None]).astype(f16)
    we_d = np.asarray(inputs["we_down"], np.float32).astype(f16)
    ws_g = (np.asarray(inputs["ws_gate"], np.float32)
            * w_post[:, None]).astype(f16)
    ws_u = (np.asarray(inputs["ws_up"], np.float32)
            * w_post[:, None]).astype(f16)
    ws_d = np.asarray(inputs["ws_down"], np.float32).astype(f16)

    inv_freq = 1.0 / (THETA ** (np.arange(0, D, 2, dtype=np.float64) / D))
    f = pos[None, :] * inv_freq[:, None]
    cos2, sin2 = np.cos(f), np.sin(f)
    cosT = np.repeat(cos2, 2, axis=0).astype(np.float32)
    sinT = np.empty((D, T), np.float32)
    sinT[0::2] = -sin2
    sinT[1::2] = sin2
    s = 1.0 / np.sqrt(D)
    cosq, sinq = (cosT * s).astype(np.float32), (sinT * s).astype(np.float32)

    import ml_dtypes
    bf = ml_dtypes.bfloat16
    ii = np.arange(P)
    diag_mask = np.where(ii[:, None] >= ii[None, :], 0.0, NEG).astype(bf)

    identr_in = np.eye(P, dtype=np.float32)
    identh_in = np.eye(P, dtype=f16)
    ut_in = np.triu(np.ones((P, P), np.float32))
    slb_in = np.zeros((8, TB * P), np.float32)
    for b in range(TB):
        slb_in[:b, b * P:(b + 1) * P] = 1.0
    perm = np.zeros((P, P), np.float32)
    for i in range(0, P, 2):
        perm[i, i + 1] = 1.0
        perm[i + 1, i] = 1.0

    ISC = IS // NCN
    maps = []
    for c in range(NCN):
        g = c // 2
        emask01 = np.zeros((P, 2 * E), np.float32)
        emask01[:, 2 * c] = 1.0          # ei = 0 -> expert 2c
        emask01[:, E + 2 * c + 1] = 1.0  # ei = 1 -> expert 2c+1
        maps.append({
            "hid": hid,
            "hid_slice": np.ascontiguousarray(hid[c * P:(c + 1) * P]),
            "wq_s": np.ascontiguousarray(wq[:, 2 * c * D:(2 * c + 2) * D]),
            "wk_s": np.ascontiguousarray(wk[:, g * D:(g + 1) * D]),
            "wv_s": np.ascontiguousarray(wv[:, g * D:(g + 1) * D]),
            "wo_s": np.ascontiguousarray(wo[2 * c * D:(2 * c + 2) * D, :]),
            "cosq": cosq, "sinq": sinq, "cosk": cosT, "sink": sinT,
            "perm": perm, "diag_mask": diag_mask,
            "identr_in": identr_in, "identh_in": identh_in,
            "ut_in": ut_in, "slb_in": slb_in,
            "gate_wT": np.ascontiguousarray(gate_w.T),
            "gate_b": np.broadcast_to(gate_b, (P, E)).copy(),
            "emask01": emask01,
            "ws_g": np.ascontiguousarray(ws_g[:, c * ISC:(c + 1) * ISC]),
            "ws_u": np.ascontiguousarray(ws_u[:, c * ISC:(c + 1) * ISC]),
            "ws_d": np.ascontiguousarray(ws_d[c * ISC:(c + 1) * ISC, :]),
            "we_g": np.ascontiguousarray(we_g[2 * c:2 * c + 2]),
            "we_u": np.ascontiguousarray(we_u[2 * c:2 * c + 2]),
            "we_d": np.ascontiguousarray(we_d[2 * c:2 * c + 2]),
        })
    return maps


def kernel(trace=False, **inputs):
    nc = _build()
    maps = _host_prep(inputs)
    res = bass_utils.run_bass_kernel_spmd(
        nc, maps, core_ids=list(range(NCN)), trace=trace)
    out = np.concatenate([res.results[c]["out_slice"] for c in range(NCN)], 0)
    resid = np.concatenate([res.results[c]["res_slice"] for c in range(NCN)], 0)
    kernel.last_results = res
    return out, resid


# revision 19
# speedup vs baseline: 1.4756x; 1.0140x over previous
"""Ernie4 decoder layer (RMSNorm + GQA attention + shared expert + 16-expert
top-2 MoE) on 8 Trainium2 NeuronCores.

v2 — fp16 data path everywhere except the router (which must reproduce the
reference top-2 selection exactly; margins are ~3e-5 so it stays fp32 and is
computed locally per core before the AllGather):
  - Attention: head-parallel (2 q-heads + 1 kv-head per core), fp16 QKV /
    scores / probs / o_proj with causal-block skipping; fp16 ReduceScatter.
  - Router: fp32 logits on each core's own 128 tokens; W+sel AllGathered in a
    tiny fp32 collective that precedes the fp16 x AllGather so the token-list
    build overlaps it.
  - Shared expert: intermediate-sharded (IS/8 per core) fp16, output seeds
    the MoE combine buffer.
  - MoE: expert-parallel (2 experts per core), token lists via
    triangular-matmul prefix ranks, indirect-DMA gather/scatter-add in fp16,
    fp16 ReduceScatter for the combine.
"""
import sys
sys.path.insert(0, "/opt/trn_rl_repo")

import numpy as np

import concourse.bass as bass
import concourse.bacc as bacc
import concourse.tile as tile
import concourse.mybir as mybir
from concourse import bass_utils
from concourse.masks import make_identity
from concourse.tile import add_dep_helper

dt = mybir.dt
F32 = dt.float32
F32R = dt.float32r
F16 = dt.float16
I32 = dt.int32
BF16 = dt.bfloat16
AF = mybir.ActivationFunctionType
ALU = mybir.AluOpType
AX = mybir.AxisListType

T, H, NH, NKV, D = 1024, 2048, 16, 4, 128
E, I, IS = 16, 1024, 2048
EPS = 1e-6
THETA = 10000.0
NCN = 8
P = 128
TB = T // P            # 8 token blocks
HC = H // P            # 16 hidden chunks
IP = I // P            # 8 expert-intermediate chunks
SP = IS // NCN // P    # 2 shared-intermediate chunks per core
CAP = 256              # per-expert token capacity
BIG = 1.0e6            # OOB sentinel
NEG = -1e9


def _emit(nc, tc):
    ex = {}
    for name, shape, d in [
        ("hid", [T, H], F32), ("hid_slice", [P, H], F32),
        ("wq_s", [H, 2 * D], F16), ("wk_s", [H, D], F16), ("wv_s", [H, D], F16),
        ("wo_s", [2 * D, H], F16),
        ("cosq", [D, T], F32), ("sinq", [D, T], F32),
        ("cosk", [D, T], F32), ("sink", [D, T], F32),
        ("perm", [P, P], F32),
        ("diag_mask", [P, P], BF16),
        ("gate_wT", [H, E], F32), ("gate_b", [P, E], F32),
        ("emask01", [P, 2 * E], F32),
        ("ws_g", [H, SP * P], F16), ("ws_u", [H, SP * P], F16),
        ("ws_d", [SP * P, H], F16),
        ("we_g", [2, H, I], F16), ("we_u", [2, H, I], F16),
        ("we_d", [2, I, H], F16),
        ("identr_in", [P, P], F32), ("identh_in", [P, P], F16),
        ("ut_in", [P, P], F32),
        ("slb_in", [8, TB * P], F32),
    ]:
        ex[name] = nc.dram_tensor(name, shape, d, kind="ExternalInput").ap()
    out_slice = nc.dram_tensor("out_slice", [P, H], F32, kind="ExternalOutput").ap()
    res_slice = nc.dram_tensor("res_slice", [P, H], F32, kind="ExternalOutput").ap()
    dbg_w = nc.dram_tensor("dbg_w", [P, E], F32, kind="ExternalOutput").ap()

    with tc.tile_pool(name="persist", bufs=1) as pp, \
         tc.tile_pool(name="dram", bufs=1, space="DRAM") as dram:
        rs_in = dram.tile([T, H], F16)
        rs_out = dram.tile([P, H], F16)
        agw_in = dram.tile([P, 2 * E], F32)
        w_all = dram.tile([T, 2 * E], F32, addr_space="Shared")
        agx_in = dram.tile([P, H], F16)
        x_tm = dram.tile([T, H], F16, addr_space="Shared")
        tok_lists = dram.tile([2 * CAP, 1], I32)
        rs2_in = dram.tile([T, H], F16)
        rs2_out = dram.tile([P, H], F16)

        ident = pp.tile([P, P], F32)
        make_identity(nc, ident[:])
        identr = pp.tile([P, P], F32R)
        nc.sync.dma_start(identr[:], ex["identr_in"][:].bitcast(F32R))
        identh = pp.tile([P, P], F16)
        nc.sync.dma_start(identh[:], ex["identh_in"][:])
        hid_sl = pp.tile([P, H], F32)
        nc.sync.dma_start(hid_sl[:], ex["hid_slice"][:])
        eps_t = pp.tile([P, 1], F32)
        nc.vector.memset(eps_t[:], EPS)
        emask01 = pp.tile([P, 2 * E], F32)
        nc.sync.dma_start(emask01[:], ex["emask01"][:])
        sent = pp.tile([P, 1], I32)
        nc.vector.memset(sent[:], 1000000)
        for k in range(2 * CAP // P):
            nc.sync.dma_start(tok_lists[k * P:(k + 1) * P, :], sent[:])
        # shared-expert weights are pure inputs: load them from t=0 so the
        # post-AllGather phase never waits on weight DMAs
        wsg_sb = pp.tile([P, HC * SP * P], F16)
        wsu_sb = pp.tile([P, HC * SP * P], F16)
        for t_, s_ in [(wsg_sb, "ws_g"), (wsu_sb, "ws_u")]:
            nc.sync.dma_start(
                t_[:].rearrange("p (hc m) -> p hc m", hc=HC),
                ex[s_][:].rearrange("(hc p) m -> p hc m", p=P))
        wsd_sb = [pp.tile([P, H], F16, tag=f"wsd{sp}", name=f"wsd{sp}")
                  for sp in range(SP)]
        for sp in range(SP):
            nc.sync.dma_start(wsd_sb[sp][:],
                              ex["ws_d"][sp * P:(sp + 1) * P, :])

        # ======== Phases A-C: attention (fp16) ========
        with tc.tile_pool(name="pab", bufs=1) as pab:
            qT = [pab.tile([P, T], F16, tag=f"qT{j}", name=f"qT{j}")
                  for j in range(2)]
            kT = pab.tile([P, T], F16)
            vT = pab.tile([P, T], F16)
            v_tm = [pab.tile([P, D], F16, tag=f"vtm{b}", name=f"vtm{b}")
                    for b in range(TB)]
            oT = [pab.tile([P, T], F16, tag=f"oT{j}", name=f"oT{j}")
                  for j in range(2)]

            # ---- A: norm + transpose + QKV + rope ----
            with tc.tile_pool(name="pa", bufs=1) as pa, \
                 tc.tile_pool(name="pa2", bufs=3) as pa2:
                cosq = pa.tile([D, T], F32)
                sinq = pa.tile([D, T], F32)
                cosk = pa.tile([D, T], F32)
                sink = pa.tile([D, T], F32)
                for t_, s_ in [(cosq, "cosq"), (sinq, "sinq"),
                               (cosk, "cosk"), (sink, "sink")]:
                    nc.sync.dma_start(t_[:], ex[s_][:])
                permr = pa.tile([P, P], F32R)
                nc.sync.dma_start(permr[:], ex["perm"][:].bitcast(F32R))
                wq_sb = pa.tile([P, HC * 2 * D], F16)
                wk_sb = pa.tile([P, HC * D], F16)
                wv_sb = pa.tile([P, HC * D], F16)
                for t_, s_, m in [(wq_sb, "wq_s", 2 * D), (wk_sb, "wk_s", D),
                                  (wv_sb, "wv_s", D)]:
                    nc.sync.dma_start(
                        t_[:].rearrange("p (hc m) -> p hc m", hc=HC),
                        ex[s_][:].rearrange("(hc p) m -> p hc m", p=P))

                dump = pa.tile([P, H], F32)
                qraw = [pa.tile([P, T], F32R, tag=f"qraw{j}", name=f"qraw{j}")
                        for j in range(2)]
                kraw = pa.tile([P, T], F32R)
                with tc.tile_pool(name="psA1", bufs=2, space="PSUM") as psA1, \
                     tc.tile_pool(name="psA2", bufs=2, space="PSUM") as psA2:
                    for n in range(2):
                        x0T = [pa.tile([P, 512], F16, tag=f"x0T{hc}",
                                       name=f"x0T{hc}_{n}") for hc in range(HC)]
                        for bb in range(TB // 2):
                            b = n * (TB // 2) + bb
                            hidb = pa2.tile([P, H], F32, tag="hidb", bufs=2)
                            nc.sync.dma_start(hidb[:],
                                              ex["hid"][b * P:(b + 1) * P, :])
                            ssum = pa2.tile([P, 1], F32, tag="ssum")
                            nc.scalar.activation(dump[:], hidb[:], AF.Square,
                                                 accum_out=ssum[:, :1])
                            rms = pa2.tile([P, 1], F32, tag="rms")
                            nc.scalar.activation(rms[:], ssum[:],
                                                 AF.Sqrt, bias=eps_t[:, :1],
                                                 scale=1.0 / H)
                            inv = pa2.tile([P, 1], F32, tag="inv")
                            nc.vector.reciprocal(inv[:], rms[:])
                            x0b = pa2.tile([P, H], F16, tag="x0b", bufs=2)
                            nc.vector.tensor_scalar_mul(x0b[:], hidb[:],
                                                        inv[:, :1])
                            for hc in range(HC):
                                tp = psA1.tile([P, P], F16, tag="tpA")
                                nc.tensor.transpose(
                                    tp[:], x0b[:, hc * P:(hc + 1) * P],
                                    identh[:])
                                nc.vector.tensor_copy(
                                    x0T[hc][:, bb * P:(bb + 1) * P], tp[:])

                        def proj(w_sb, m, c0, dst, n=n, x0T=x0T, fp16=False):
                            ps = psA2.tile([P, 512], F32, tag="psQKV",
                                           name="psQKV")
                            for hc in range(HC):
                                nc.tensor.matmul(
                                    ps[:],
                                    w_sb[:, hc * m + c0:hc * m + c0 + P],
                                    x0T[hc][:],
                                    start=(hc == 0), stop=(hc == HC - 1))
                            nc.vector.tensor_copy(
                                dst[:, n * 512:(n + 1) * 512], ps[:])
                        proj(wq_sb, 2 * D, 0, qraw[0])
                        proj(wq_sb, 2 * D, D, qraw[1])
                        proj(wk_sb, D, 0, kraw)
                        proj(wv_sb, D, 0, vT, fp16=True)

                with tc.tile_pool(name="psA3", bufs=2, space="PSUM") as psA3:
                    for src, dst, c_, s_ in [(qraw[0], qT[0], cosq, sinq),
                                             (qraw[1], qT[1], cosq, sinq),
                                             (kraw, kT, cosk, sink)]:
                        for n in range(2):
                            sl = slice(n * 512, (n + 1) * 512)
                            sw = psA3.tile([P, 512], F32, tag="psSW")
                            nc.tensor.matmul(sw[:], permr[:], src[:, sl],
                                             start=True, stop=True)
                            t1 = pa2.tile([P, 512], F32, tag="ropeT1")
                            nc.vector.tensor_mul(t1[:], src[:, sl], c_[:, sl])
                            t2 = pa2.tile([P, 512], F32, tag="ropeT2")
                            nc.vector.tensor_mul(t2[:], sw[:], s_[:, sl])
                            nc.vector.tensor_add(dst[:, sl], t1[:], t2[:])
                    for b in range(TB):
                        tp = psA3.tile([P, P], F16, tag="tpV")
                        nc.tensor.transpose(tp[:], vT[:, b * P:(b + 1) * P],
                                            identh[:])
                        nc.vector.tensor_copy(v_tm[b][:], tp[:])

            # ---- B: attention (causal-block skipped) ----
            with tc.tile_pool(name="pb", bufs=1) as pb, \
                 tc.tile_pool(name="pb2", bufs=3) as pb2:
                dmask = pb.tile([P, P], BF16)
                nc.sync.dma_start(dmask[:], ex["diag_mask"][:])
                wo_sb = [pb.tile([P, H], F16, tag=f"wo{j}", name=f"wo{j}")
                         for j in range(2)]
                nc.sync.dma_start(wo_sb[0][:], ex["wo_s"][0:P, :])
                nc.sync.dma_start(wo_sb[1][:], ex["wo_s"][P:2 * P, :])

                attnT = [pb.tile([P, T], F16, tag=f"attnT{kc}",
                                 name=f"attnT{kc}") for kc in range(TB)]
                for kc in range(1, TB):
                    nc.vector.memset(attnT[kc][:, 0:kc * P], 0.0)
                with tc.tile_pool(name="psB1", bufs=2, space="PSUM") as psB1, \
                     tc.tile_pool(name="psB2", bufs=2, space="PSUM") as psB2, \
                     tc.tile_pool(name="psB3", bufs=2, space="PSUM") as psB3:
                  for h in range(2):
                    for qc in range(TB):
                        cols = (qc + 1) * P
                        prob = pb2.tile([P, T], F32, tag="prob")
                        nsl = (cols + 511) // 512
                        for n in range(nsl):
                            w_ = min(512, cols - n * 512)
                            ps = psB1.tile([P, 512], F32, tag="psSC")
                            nc.tensor.matmul(ps[:, :w_],
                                             qT[h][:, qc * P:(qc + 1) * P],
                                             kT[:, n * 512:n * 512 + w_],
                                             start=True, stop=True)
                            # diagonal block gets the causal mask; the rest
                            # of this slice is fully visible
                            d0 = qc * P - n * 512
                            if 0 <= d0 < w_:
                                if d0 > 0:
                                    nc.vector.tensor_copy(
                                        prob[:, n * 512:n * 512 + d0],
                                        ps[:, :d0])
                                nc.vector.tensor_add(
                                    prob[:, qc * P:qc * P + P],
                                    ps[:, d0:d0 + P], dmask[:])
                            else:
                                nc.vector.tensor_copy(
                                    prob[:, n * 512:n * 512 + w_], ps[:, :w_])
                        mx = pb2.tile([P, 1], F32, tag="mx")
                        nc.vector.reduce_max(mx[:], prob[:, :cols], axis=AX.X)
                        negm = pb2.tile([P, 1], F32, tag="negm")
                        nc.vector.tensor_scalar_mul(negm[:], mx[:], -1.0)
                        ssum = pb2.tile([P, 1], F32, tag="esum")
                        probe_ = pb2.tile([P, T], F32, tag="probe")
                        nc.scalar.activation(probe_[:, :cols], prob[:, :cols],
                                             AF.Exp, bias=negm[:, :1],
                                             accum_out=ssum[:, :1])
                        rec = pb2.tile([P, 1], F32, tag="rec")
                        nc.vector.reciprocal(rec[:], ssum[:])
                        probS = pb2.tile([P, T], F16, tag="probS")
                        nc.vector.tensor_scalar_mul(probS[:, :cols],
                                                    probe_[:, :cols],
                                                    rec[:, :1])
                        for kc in range(qc + 1):
                            tp = psB2.tile([P, P], F16, tag="tpB")
                            nc.tensor.transpose(
                                tp[:], probS[:, kc * P:(kc + 1) * P],
                                identh[:])
                            nc.vector.tensor_copy(
                                attnT[kc][:, qc * P:(qc + 1) * P], tp[:])
                    for n in range(2):
                        sl = slice(n * 512, (n + 1) * 512)
                        kc_hi = 4 * n + 3
                        ps = psB3.tile([P, 512], F32, tag="psAV")
                        for kc in range(kc_hi + 1):
                            nc.tensor.matmul(ps[:], v_tm[kc][:],
                                             attnT[kc][:, sl],
                                             start=(kc == 0),
                                             stop=(kc == kc_hi))
                        nc.vector.tensor_copy(oT[h][:, sl], ps[:])

                # ---- C: o_proj ----
                with tc.tile_pool(name="psC", bufs=8, space="PSUM") as psC:
                    for tb_ in range(TB):
                        pso = [psC.tile([P, 512], F32, tag="psO",
                                        name=f"psO{n}") for n in range(4)]
                        for hp in range(2):
                            for n in range(4):
                                nc.tensor.matmul(
                                    pso[n][:],
                                    oT[hp][:, tb_ * P:(tb_ + 1) * P],
                                    wo_sb[hp][:, n * 512:(n + 1) * 512],
                                    start=(hp == 0), stop=(hp == 1))
                        ob = pb2.tile([P, H], F16, tag="ob", bufs=2)
                        for n in range(4):
                            nc.vector.tensor_copy(
                                ob[:, n * 512:(n + 1) * 512], pso[n][:])
                        nc.sync.dma_start(rs_in[tb_ * P:(tb_ + 1) * P, :],
                                          ob[:])

        nc.gpsimd.collective_compute(
            "ReduceScatter", ALU.add, ins=[rs_in.opt()], outs=[rs_out.opt()],
            replica_groups=[list(range(NCN))])

        # ======== D: residual + norm + local fp32 router + AGs ========
        with tc.tile_pool(name="pd", bufs=1) as pd, \
             tc.tile_pool(name="pd2", bufs=2) as pd2, \
             tc.tile_pool(name="psD", bufs=2, space="PSUM") as psD:
            attn_sl = pd.tile([P, H], F16)
            nc.sync.dma_start(attn_sl[:], rs_out[:])
            res_sb = pd.tile([P, H], F32)
            nc.vector.tensor_add(res_sb[:], hid_sl[:], attn_sl[:])
            nc.sync.dma_start(res_slice[:], res_sb[:])
            dump2 = pd.tile([P, H], F32)
            ssum = pd.tile([P, 1], F32)
            nc.scalar.activation(dump2[:], res_sb[:], AF.Square,
                                 accum_out=ssum[:, :1])
            rms = pd.tile([P, 1], F32)
            nc.scalar.activation(rms[:], ssum[:], AF.Sqrt, bias=eps_t[:, :1],
                                 scale=1.0 / H)
            inv = pd.tile([P, 1], F32)
            nc.vector.reciprocal(inv[:], rms[:])
            x_sl = pd.tile([P, H], F32)
            nc.vector.tensor_scalar_mul(x_sl[:], res_sb[:], inv[:, :1])
            x_sl_h = pd.tile([P, H], F16)
            nc.vector.tensor_copy(x_sl_h[:], x_sl[:])
            nc.sync.dma_start(agx_in[:], x_sl_h[:])

            # fp32 transpose of own slice for exact router logits
            gw_sb = pd.tile([P, HC * E], F32)
            nc.sync.dma_start(
                gw_sb[:].rearrange("p (hc e) -> p hc e", hc=HC),
                ex["gate_wT"][:].rearrange("(hc p) e -> p hc e", p=P))
            gate_b = pd.tile([P, E], F32)
            nc.sync.dma_start(gate_b[:], ex["gate_b"][:])
            x_slT = pd.tile([P, H], F32)
            for hc in range(HC):
                tp = psD.tile([P, P], F32, tag="tpD")
                nc.tensor.transpose(tp[:], x_sl[:, hc * P:(hc + 1) * P],
                                    ident[:])
                nc.vector.tensor_copy(x_slT[:, hc * P:(hc + 1) * P], tp[:])
            lg_ps = psD.tile([P, E], F32, tag="lgps", name="lgps")
            for hc in range(HC):
                nc.tensor.matmul(lg_ps[:], x_slT[:, hc * P:(hc + 1) * P],
                                 gw_sb[:, hc * E:(hc + 1) * E],
                                 start=(hc == 0), stop=(hc == HC - 1))
            sig = pd2.tile([P, E], F32, tag="sig")
            nc.scalar.activation(sig[:], lg_ps[:], AF.Sigmoid)
            sb_ = pd2.tile([P, E], F32, tag="sb_")
            nc.vector.tensor_add(sb_[:], sig[:], gate_b[:])
            mx = pd2.tile([P, 8], F32, tag="mx8")
            nc.vector.max(out=mx[:], in_=sb_[:])
            s1 = pd2.tile([P, E], F32, tag="s1")
            nc.vector.tensor_tensor(out=s1[:], in0=sb_[:],
                                    in1=mx[:, 0:1].to_broadcast([P, E]),
                                    op=ALU.is_equal)
            s2 = pd2.tile([P, E], F32, tag="s2")
            nc.vector.tensor_tensor(out=s2[:], in0=sb_[:],
                                    in1=mx[:, 1:2].to_broadcast([P, E]),
                                    op=ALU.is_equal)
            nc.vector.tensor_add(s1[:], s1[:], s2[:])
            sel_own = pd2.tile([P, E], F32, tag="sel_own")
            nc.vector.tensor_scalar_min(sel_own[:], s1[:], 1.0)
            wa = pd2.tile([P, E], F32, tag="wa")
            nc.vector.tensor_mul(wa[:], sel_own[:], sig[:])
            nrm = pd2.tile([P, 1], F32, tag="nrm")
            nc.vector.reduce_sum(nrm[:], wa[:], axis=AX.X)
            rec = pd2.tile([P, 1], F32, tag="recw")
            nc.vector.reciprocal(rec[:], nrm[:])
            w_tm = pd2.tile([P, E], F32, tag="wtm")
            nc.vector.tensor_scalar_mul(w_tm[:], wa[:], rec[:, :1])
            nc.sync.dma_start(agw_in[:, 0:E], w_tm[:])
            nc.sync.dma_start(agw_in[:, E:2 * E], sel_own[:])
            nc.sync.dma_start(dbg_w[:], w_tm[:])

        cc_w = nc.gpsimd.collective_compute(
            "AllGather", ALU.bypass, ins=[agw_in.opt()], outs=[w_all.opt()],
            replica_groups=[list(range(NCN))])
        cc_x = nc.gpsimd.collective_compute(
            "AllGather", ALU.bypass, ins=[agx_in.opt()], outs=[x_tm.opt()],
            replica_groups=[list(range(NCN))])
        # The tiny router AllGather must run first so the token-list build
        # overlaps the big x AllGather (CC queue executes in trigger order).
        add_dep_helper(cc_x.ins, cc_w.ins, sync=True,
                       reason="AG_W before AG_x")

        # ======== E: token lists from AllGathered router decisions ========
        with tc.tile_pool(name="pe", bufs=1) as pe, \
             tc.tile_pool(name="pe2", bufs=3) as pe2, \
             tc.tile_pool(name="psE", bufs=2, space="PSUM") as psE:
            ut = pe.tile([P, P], F32R)
            nc.sync.dma_start(ut[:], ex["ut_in"][:].bitcast(F32R))
            slb = pe.tile([8, TB * P], F32R)
            nc.sync.dma_start(slb[:], ex["slb_in"][:].bitcast(F32R))
            totals = pe.tile([8, E], F32R)
            pre_sb = [pe.tile([P, E], F32, tag=f"pre{b}", name=f"pre{b}")
                      for b in range(TB)]
            sel_all = [pe.tile([P, E], F32, tag=f"sela{b}", name=f"sela{b}")
                       for b in range(TB)]
            for b in range(TB):
                nc.sync.dma_start(sel_all[b][:],
                                  w_all[b * P:(b + 1) * P, E:2 * E])
                pr_ps = psE.tile([P, E], F32, tag="prps")
                nc.tensor.matmul(pr_ps[:], ut[:],
                                 sel_all[b][:].bitcast(F32R),
                                 start=True, stop=True)
                nc.vector.tensor_copy(pre_sb[b][:], pr_ps[:])
                nc.sync.dma_start(totals[b:b + 1, :],
                                  pre_sb[b][127:128, :].bitcast(F32R))
            for b in range(TB):
                ofs_ps = psE.tile([P, E], F32, tag="ofsps", name="ofsps")
                nc.tensor.matmul(ofs_ps[:], slb[:, b * P:(b + 1) * P],
                                 totals[:], start=True, stop=True)
                grank = pe2.tile([P, E], F32, tag="grank")
                nc.vector.tensor_add(grank[:], pre_sb[b][:], ofs_ps[:])
                nc.vector.tensor_scalar_add(grank[:], grank[:], -1.0)
                gm = pe2.tile([P, E], F32, tag="gm")
                nc.vector.tensor_scalar(out=gm[:], in0=grank[:],
                                        scalar1=float(CAP - 1), scalar2=BIG,
                                        op0=ALU.is_gt, op1=ALU.mult)
                nc.vector.tensor_add(grank[:], grank[:], gm[:])
                um = pe2.tile([P, E], F32, tag="um")
                nc.vector.tensor_scalar(out=um[:], in0=sel_all[b][:],
                                        scalar1=-BIG, scalar2=BIG,
                                        op0=ALU.mult, op1=ALU.add)
                nc.vector.tensor_add(grank[:], grank[:], um[:])
                tok = pe2.tile([P, 1], I32, tag="tok")
                nc.gpsimd.iota(tok[:], pattern=[[0, 1]], base=b * P,
                               channel_multiplier=1)
                for ei in range(2):
                    ge = pe2.tile([P, E], F32, tag="ge")
                    nc.vector.tensor_mul(ge[:], grank[:],
                                         emask01[:, ei * E:(ei + 1) * E])
                    ridx = pe2.tile([P, 1], F32, tag="ridx")
                    nc.vector.reduce_sum(ridx[:], ge[:], axis=AX.X)
                    nc.vector.tensor_scalar_add(ridx[:], ridx[:],
                                                float(ei * CAP))
                    ridx_i = pe2.tile([P, 1], I32, tag="ridxi")
                    nc.vector.tensor_copy(ridx_i[:], ridx[:])
                    nc.gpsimd.indirect_dma_start(
                        out=tok_lists[:],
                        out_offset=bass.IndirectOffsetOnAxis(
                            ap=ridx_i[:, :1], axis=0),
                        in_=tok[:], in_offset=None,
                        bounds_check=2 * CAP - 1, oob_is_err=False)

        # ======== F: xT + shared expert + experts (fp16) ========
        with tc.tile_pool(name="pxt", bufs=1) as pxt, \
             tc.tile_pool(name="pfs", bufs=1) as pfs, \
             tc.tile_pool(name="pfs2", bufs=2) as pfs2:
            xc = [pxt.tile([P, T], F16, tag=f"xc{hc}", name=f"xc{hc}")
                  for hc in range(HC)]
            with tc.tile_pool(name="pxt2", bufs=3) as pxt2, \
                 tc.tile_pool(name="psX", bufs=2, space="PSUM") as psX:
                for b in range(TB):
                    xb = pxt2.tile([P, H], F16, tag="xb", bufs=2)
                    nc.sync.dma_start(xb[:], x_tm[b * P:(b + 1) * P, :])
                    for hc in range(HC):
                        tp = psX.tile([P, P], F16, tag="tpX")
                        nc.tensor.transpose(tp[:], xb[:, hc * P:(hc + 1) * P],
                                            identh[:])
                        nc.vector.tensor_copy(xc[hc][:, b * P:(b + 1) * P],
                                              tp[:])

            # ---- both experts' setup: token lists, gathers, gxT, weights,
            # per-token gate weights — overlaps the shared expert below ----
            idx_sb2 = [[pfs.tile([P, 1], I32, tag=f"idx{ei}{k}",
                                 name=f"idx{ei}{k}") for k in range(2)]
                       for ei in range(2)]
            gxT2 = [pfs.tile([P, HC * 2 * P], F16, tag=f"gxT{ei}",
                             name=f"gxT{ei}") for ei in range(2)]
            wd_res2 = [[pfs.tile([P, H], F16, tag=f"wd{ei}{ip}",
                                 name=f"wd{ei}{ip}") for ip in range(IP)]
                       for ei in range(2)]
            wg_own2 = [[pfs.tile([P, 1], F32, tag=f"wgo{ei}{k}",
                                 name=f"wgo{ei}{k}") for k in range(2)]
                       for ei in range(2)]
            psS_cm = tc.tile_pool(name="psS", bufs=2, space="PSUM")
            psS = psS_cm.__enter__()
            for ei in range(2):
                for k in range(2):
                    nc.sync.dma_start(
                        idx_sb2[ei][k][:],
                        tok_lists[ei * CAP + k * P:ei * CAP + (k + 1) * P, :])
                    gx = pfs2.tile([P, H], F16, tag="gx")
                    nc.vector.memset(gx[:], 0.0)
                    nc.gpsimd.indirect_dma_start(
                        out=gx[:], out_offset=None,
                        in_=x_tm[:],
                        in_offset=bass.IndirectOffsetOnAxis(
                            ap=idx_sb2[ei][k][:, :1], axis=0),
                        bounds_check=T - 1, oob_is_err=False)
                    for hc in range(HC):
                        tp = psS.tile([P, P], F16, tag="tpS")
                        nc.tensor.transpose(tp[:], gx[:, hc * P:(hc + 1) * P],
                                            identh[:])
                        nc.vector.tensor_copy(
                            gxT2[ei][:, hc * 2 * P + k * P:
                                  hc * 2 * P + (k + 1) * P], tp[:])
                    wrow = pfs2.tile([P, 2 * E], F32, tag="wrow")
                    nc.vector.memset(wrow[:], 0.0)
                    nc.gpsimd.indirect_dma_start(
                        out=wrow[:], out_offset=None, in_=w_all[:],
                        in_offset=bass.IndirectOffsetOnAxis(
                            ap=idx_sb2[ei][k][:, :1], axis=0),
                        bounds_check=T - 1, oob_is_err=False)
                    we_ = pfs2.tile([P, E], F32, tag="we_")
                    nc.vector.tensor_mul(we_[:], wrow[:, 0:E],
                                         emask01[:, ei * E:(ei + 1) * E])
                    nc.vector.reduce_sum(wg_own2[ei][k][:], we_[:], axis=AX.X)

            # ---- shared expert ----
            with tc.tile_pool(name="pg", bufs=1) as pg, \
                 tc.tile_pool(name="pg2", bufs=3) as pg2:
                g_act = [pg.tile([P, T], F16, tag=f"gact{sp}", name=f"gact{sp}")
                         for sp in range(SP)]
                hs = [pg.tile([P, T], F16, tag=f"hs{sp}", name=f"hs{sp}")
                      for sp in range(SP)]
                with tc.tile_pool(name="psG1", bufs=1, space="PSUM") as psG1:
                    g_ps = [psG1.tile([P, T], F32, tag=f"gps{sp}",
                                      name=f"gps{sp}") for sp in range(SP)]
                    for hc in range(HC):
                        for sp in range(SP):
                            c0 = hc * SP * P + sp * P
                            for n in range(2):
                                sl = slice(n * 512, (n + 1) * 512)
                                nc.tensor.matmul(g_ps[sp][:, sl],
                                                 wsg_sb[:, c0:c0 + P],
                                                 xc[hc][:, sl],
                                                 start=(hc == 0),
                                                 stop=(hc == HC - 1))
                    for sp in range(SP):
                        nc.scalar.activation(g_act[sp][:], g_ps[sp][:],
                                             AF.Silu)
                with tc.tile_pool(name="psG2", bufs=1, space="PSUM") as psG2:
                    u_ps = [psG2.tile([P, T], F32, tag=f"ups{sp}",
                                      name=f"ups{sp}") for sp in range(SP)]
                    for hc in range(HC):
                        for sp in range(SP):
                            c0 = hc * SP * P + sp * P
                            for n in range(2):
                                sl = slice(n * 512, (n + 1) * 512)
                                nc.tensor.matmul(u_ps[sp][:, sl],
                                                 wsu_sb[:, c0:c0 + P],
                                                 xc[hc][:, sl],
                                                 start=(hc == 0),
                                                 stop=(hc == HC - 1))
                    for sp in range(SP):
                        nc.vector.tensor_mul(hs[sp][:], g_act[sp][:],
                                             u_ps[sp][:])
                with tc.tile_pool(name="psG3", bufs=6, space="PSUM") as psG3:
                    for tb_ in range(TB):
                        psd = [psG3.tile([P, 512], F32, tag="psGd",
                                         name=f"psGd{n}") for n in range(4)]
                        for sp in range(SP):
                            for n in range(4):
                                nc.tensor.matmul(
                                    psd[n][:],
                                    hs[sp][:, tb_ * P:(tb_ + 1) * P],
                                    wsd_sb[sp][:, n * 512:(n + 1) * 512],
                                    start=(sp == 0), stop=(sp == SP - 1))
                        sbd = pg2.tile([P, H], F16, tag="sbGd", bufs=2)
                        for n in range(4):
                            nc.vector.tensor_copy(
                                sbd[:, n * 512:(n + 1) * 512], psd[n][:])
                        nc.sync.dma_start(rs2_in[tb_ * P:(tb_ + 1) * P, :],
                                          sbd[:])

            psS_cm.__exit__(None, None, None)

            # expert down-proj weights: emitted late so these 8 MB of DMAs
            # sit behind the x-block/gather traffic in queue priority, but
            # they still have ~100us of slack before first use
            for ei in range(2):
                for ip in range(IP):
                    nc.sync.dma_start(wd_res2[ei][ip][:],
                                      ex["we_d"][ei, ip * P:(ip + 1) * P, :])

            # ---- experts (setup already done above) ----
            for ei in range(2):
                with tc.tile_pool(name=f"pf{ei}", bufs=1) as pf, \
                     tc.tile_pool(name=f"pf2{ei}", bufs=2) as pf2:
                    idx_sb = idx_sb2[ei]
                    gxT = gxT2[ei]
                    wd_res = wd_res2[ei]

                    # merged gate+up pass (8 PSUM banks)
                    g_tm = [pf.tile([P, I], F16, tag=f"gtm{k}", name=f"gtm{k}")
                            for k in range(2)]
                    h_tm = [pf.tile([P, I], F16, tag=f"htm{k}", name=f"htm{k}")
                            for k in range(2)]
                    with tc.tile_pool(name=f"psF2{ei}", bufs=1,
                                      space="PSUM") as psF2:
                        gu_ps = [[psF2.tile([P, 512], F32, tag=f"gups{k}{j}",
                                            name=f"gups{k}{j}")
                                  for j in range(4)] for k in range(2)]
                        for hc in range(HC):
                            wg_c = pf2.tile([P, I], F16, tag="wgF", bufs=3)
                            nc.sync.dma_start(
                                wg_c[:], ex["we_g"][ei, hc * P:(hc + 1) * P, :])
                            wu_c = pf2.tile([P, I], F16, tag="wuF", bufs=3)
                            nc.sync.dma_start(
                                wu_c[:], ex["we_u"][ei, hc * P:(hc + 1) * P, :])
                            for k in range(2):
                                s_ = gxT[:, hc * 2 * P + k * P:
                                         hc * 2 * P + (k + 1) * P]
                                for n in range(2):
                                    nc.tensor.matmul(
                                        gu_ps[k][n][:], s_,
                                        wg_c[:, n * 512:(n + 1) * 512],
                                        start=(hc == 0), stop=(hc == HC - 1))
                                for n in range(2):
                                    nc.tensor.matmul(
                                        gu_ps[k][2 + n][:], s_,
                                        wu_c[:, n * 512:(n + 1) * 512],
                                        start=(hc == 0), stop=(hc == HC - 1))
                        for k in range(2):
                            for n in range(2):
                                sl = slice(n * 512, (n + 1) * 512)
                                nc.scalar.activation(g_tm[k][:, sl],
                                                     gu_ps[k][n][:], AF.Silu)
                                nc.vector.tensor_mul(h_tm[k][:, sl],
                                                     g_tm[k][:, sl],
                                                     gu_ps[k][2 + n][:])
                    h_sb = [pf.tile([P, 2 * P], F16, tag=f"hsb{ip}",
                                    name=f"hsb{ip}") for ip in range(IP)]
                    with tc.tile_pool(name=f"psF4{ei}", bufs=2,
                                      space="PSUM") as psF4:
                        for k in range(2):
                            for ip in range(IP):
                                tp = psF4.tile([P, P], F16, tag="tpF2")
                                nc.tensor.transpose(
                                    tp[:], h_tm[k][:, ip * P:(ip + 1) * P],
                                    identh[:])
                                nc.vector.tensor_copy(
                                    h_sb[ip][:, k * P:(k + 1) * P], tp[:])
                    with tc.tile_pool(name=f"psF5{ei}", bufs=8,
                                      space="PSUM") as psF5:
                        for k in range(2):
                            psd = [psF5.tile([P, 512], F32, tag="psFd",
                                             name=f"psFd{n}")
                                   for n in range(4)]
                            for ip in range(IP):
                                for n in range(4):
                                    nc.tensor.matmul(
                                        psd[n][:],
                                        h_sb[ip][:, k * P:(k + 1) * P],
                                        wd_res[ip][:, n * 512:(n + 1) * 512],
                                        start=(ip == 0), stop=(ip == IP - 1))
                            out_sb = pf.tile([P, H], F16, tag=f"outsb{k}")
                            for n in range(4):
                                nc.vector.tensor_scalar_mul(
                                    out_sb[:, n * 512:(n + 1) * 512],
                                    psd[n][:], wg_own2[ei][k][:, :1])
                            nc.gpsimd.indirect_dma_start(
                                out=rs2_in[:],
                                out_offset=bass.IndirectOffsetOnAxis(
                                    ap=idx_sb[k][:, :1], axis=0),
                                in_=out_sb[:], in_offset=None,
                                bounds_check=T - 1, oob_is_err=False,
                                compute_op=ALU.add)

        nc.gpsimd.collective_compute(
            "ReduceScatter", ALU.add, ins=[rs2_in.opt()], outs=[rs2_out.opt()],
            replica_groups=[list(range(NCN))])
        with tc.tile_pool(name="pz", bufs=2) as pz:
            fin16 = pz.tile([P, H], F16)
            nc.sync.dma_start(fin16[:], rs2_out[:])
            fin = pz.tile([P, H], F32)
            nc.vector.tensor_copy(fin[:], fin16[:])
            nc.sync.dma_start(out_slice[:], fin[:])


_CACHE = {}


def _build():
    key = "nc"
    if key in _CACHE:
        return _CACHE[key]
    nc = bacc.Bacc("TRN2", target_bir_lowering=False, debug=False,
                   num_devices=NCN)
    with tile.TileContext(nc) as tc:
        _emit(nc, tc)
    nc.compile()
    _CACHE[key] = nc
    return nc


def _host_prep(inputs):
    f16 = np.float16
    pos = np.asarray(inputs["positions"]).astype(np.float64)
    hid = np.asarray(inputs["hidden_states"], np.float32)
    w_in = np.asarray(inputs["w_in_ln"], np.float32)
    w_post = np.asarray(inputs["w_post_ln"], np.float32)
    wq = (np.asarray(inputs["wq"], np.float32) * w_in[:, None]).astype(f16)
    wk = (np.asarray(inputs["wk"], np.float32) * w_in[:, None]).astype(f16)
    wv = (np.asarray(inputs["wv"], np.float32) * w_in[:, None]).astype(f16)
    wo = np.asarray(inputs["wo"], np.float32).astype(f16)
    gate_w = np.asarray(inputs["gate_w"], np.float32) * w_post[None, :]
    gate_b = np.asarray(inputs["gate_bias"], np.float32).reshape(1, E)
    we_g = (np.asarray(inputs["we_gate"], np.float32)
            * w_post[None, :, None]).astype(f16)
    we_u = (np.asarray(inputs["we_up"], np.float32)
            * w_post[None, :, None]).astype(f16)
    we_d = np.asarray(inputs["we_down"], np.float32).astype(f16)
    ws_g = (np.asarray(inputs["ws_gate"], np.float32)
            * w_post[:, None]).astype(f16)
    ws_u = (np.asarray(inputs["ws_up"], np.float32)
            * w_post[:, None]).astype(f16)
    ws_d = np.asarray(inputs["ws_down"], np.float32).astype(f16)

    inv_freq = 1.0 / (THETA ** (np.arange(0, D, 2, dtype=np.float64) / D))
    f = pos[None, :] * inv_freq[:, None]
    cos2, sin2 = np.cos(f), np.sin(f)
    cosT = np.repeat(cos2, 2, axis=0).astype(np.float32)
    sinT = np.empty((D, T), np.float32)
    sinT[0::2] = -sin2
    sinT[1::2] = sin2
    s = 1.0 / np.sqrt(D)
    cosq, sinq = (cosT * s).astype(np.float32), (sinT * s).astype(np.float32)

    import ml_dtypes
    bf = ml_dtypes.bfloat16
    ii = np.arange(P)
    diag_mask = np.where(ii[:, None] >= ii[None, :], 0.0, NEG).astype(bf)

    identr_in = np.eye(P, dtype=np.float32)
    identh_in = np.eye(P, dtype=f16)
    ut_in = np.triu(np.ones((P, P), np.float32))
    slb_in = np.zeros((8, TB * P), np.float32)
    for b in range(TB):
        slb_in[:b, b * P:(b + 1) * P] = 1.0
    perm = np.zeros((P, P), np.float32)
    for i in range(0, P, 2):
        perm[i, i + 1] = 1.0
        perm[i + 1, i] = 1.0

    ISC = IS // NCN
    maps = []
    for c in range(NCN):
        g = c // 2
        emask01 = np.zeros((P, 2 * E), np.float32)
        emask01[:, 2 * c] = 1.0          # ei = 0 -> expert 2c
        emask01[:, E + 2 * c + 1] = 1.0  # ei = 1 -> expert 2c+1
        maps.append({
            "hid": hid,
            "hid_slice": np.ascontiguousarray(hid[c * P:(c + 1) * P]),
            "wq_s": np.ascontiguousarray(wq[:, 2 * c * D:(2 * c + 2) * D]),
            "wk_s": np.ascontiguousarray(wk[:, g * D:(g + 1) * D]),
            "wv_s": np.ascontiguousarray(wv[:, g * D:(g + 1) * D]),
            "wo_s": np.ascontiguousarray(wo[2 * c * D:(2 * c + 2) * D, :]),
            "cosq": cosq, "sinq": sinq, "cosk": cosT, "sink": sinT,
            "perm": perm, "diag_mask": diag_mask,
            "identr_in": identr_in, "identh_in": identh_in,
            "ut_in": ut_in, "slb_in": slb_in,
            "gate_wT": np.ascontiguousarray(gate_w.T),
            "gate_b": np.broadcast_to(gate_b, (P, E)).copy(),
            "emask01": emask01,
            "ws_g": np.ascontiguousarray(ws_g[:, c * ISC:(c + 1) * ISC]),
            "ws_u": np.ascontiguousarray(ws_u[:, c * ISC:(c + 1) * ISC]),
            "ws_d": np.ascontiguousarray(ws_d[c * ISC:(c + 1) * ISC, :]),
            "we_g": np.ascontiguousarray(we_g[2 * c:2 * c + 2]),
            "we_u": np.ascontiguousarray(we_u[2 * c:2 * c + 2]),
            "we_d": np.ascontiguousarray(we_d[2 * c:2 * c + 2]),
        })
    return maps


def kernel(trace=False, **inputs):
    nc = _build()
    maps = _host_prep(inputs)
    res = bass_utils.run_bass_kernel_spmd(
        nc, maps, core_ids=list(range(NCN)), trace=trace)
    out = np.concatenate([res.results[c]["out_slice"] for c in range(NCN)], 0)
    resid = np.concatenate([res.results[c]["res_slice"] for c in range(NCN)], 0)
    kernel.last_results = res
    return out, resid


# revision 25
# speedup vs baseline: 1.5177x; 1.0285x over previous
"""Ernie4 decoder layer (RMSNorm + GQA attention + shared expert + 16-expert
top-2 MoE) on 8 Trainium2 NeuronCores.

v2 — fp16 data path everywhere except the router (which must reproduce the
reference top-2 selection exactly; margins are ~3e-5 so it stays fp32 and is
computed locally per core before the AllGather):
  - Attention: head-parallel (2 q-heads + 1 kv-head per core), fp16 QKV /
    scores / probs / o_proj with causal-block skipping; fp16 ReduceScatter.
  - Router: fp32 logits on each core's own 128 tokens; W+sel AllGathered in a
    tiny fp32 collective that precedes the fp16 x AllGather so the token-list
    build overlaps it.
  - Shared expert: intermediate-sharded (IS/8 per core) fp16, output seeds
    the MoE combine buffer.
  - MoE: expert-parallel (2 experts per core), token lists via
    triangular-matmul prefix ranks, indirect-DMA gather/scatter-add in fp16,
    fp16 ReduceScatter for the combine.
"""
import sys
sys.path.insert(0, "/opt/trn_rl_repo")

import numpy as np

import concourse.bass as bass
import concourse.bacc as bacc
import concourse.tile as tile
import concourse.mybir as mybir
from concourse import bass_utils
from concourse.masks import make_identity
from concourse.tile import add_dep_helper

dt = mybir.dt
F32 = dt.float32
F32R = dt.float32r
F16 = dt.float16
I32 = dt.int32
BF16 = dt.bfloat16
AF = mybir.ActivationFunctionType
ALU = mybir.AluOpType
AX = mybir.AxisListType

T, H, NH, NKV, D = 1024, 2048, 16, 4, 128
E, I, IS = 16, 1024, 2048
EPS = 1e-6
THETA = 10000.0
NCN = 8
P = 128
TB = T // P            # 8 token blocks
HC = H // P            # 16 hidden chunks
IP = I // P            # 8 expert-intermediate chunks
SP = IS // NCN // P    # 2 shared-intermediate chunks per core
CAP = 256              # per-expert token capacity
BIG = 1.0e6            # OOB sentinel
NEG = -1e9


def _emit(nc, tc):
    ex = {}
    for name, shape, d in [
        ("hid", [T, H], F32), ("hid_slice", [P, H], F32),
        ("wq_s", [H, 2 * D], F16), ("wk_s", [H, D], F16), ("wv_s", [H, D], F16),
        ("wo_s", [2 * D, H], F16),
        ("cosq", [D, T], F32), ("sinq", [D, T], F32),
        ("cosk", [D, T], F32), ("sink", [D, T], F32),
        ("perm", [P, P], F32),
        ("diag_mask", [P, P], BF16),
        ("gate_wT", [H, E], F32), ("gate_b", [P, E], F32),
        ("emask01", [P, 2 * E], F32),
        ("ws_g", [H, SP * P], F16), ("ws_u", [H, SP * P], F16),
        ("ws_d", [SP * P, H], F16),
        ("we_g", [2, H, I], F16), ("we_u", [2, H, I], F16),
        ("we_d", [2, I, H], F16),
        ("identr_in", [P, P], F32), ("identh_in", [P, P], F16),
        ("ut_in", [P, P], F32),
        ("slb_in", [8, TB * P], F32),
        ("slot_iota", [P, CAP], F32), ("tokid2", [P, 2 * TB], F16),
    ]:
        ex[name] = nc.dram_tensor(name, shape, d, kind="ExternalInput").ap()
    out_slice = nc.dram_tensor("out_slice", [P, H], F32, kind="ExternalOutput").ap()
    res_slice = nc.dram_tensor("res_slice", [P, H], F32, kind="ExternalOutput").ap()
    dbg_w = nc.dram_tensor("dbg_w", [P, E], F32, kind="ExternalOutput").ap()

    with tc.tile_pool(name="persist", bufs=1) as pp, \
         tc.tile_pool(name="dram", bufs=1, space="DRAM") as dram:
        rs_in = dram.tile([T, H], F16)
        rs_out = dram.tile([P, H], F16)
        agw_in = dram.tile([P, 2 * E], F32)
        w_all = dram.tile([T, 2 * E], F32, addr_space="Shared")
        agx_in = dram.tile([P, H], F16)
        x_tm = dram.tile([T, H], F16, addr_space="Shared")
        tok_lists = dram.tile([2 * CAP, 1], I32)
        rs2_in = dram.tile([T, H], F16)
        rs2_out = dram.tile([P, H], F16)

        ident = pp.tile([P, P], F32)
        make_identity(nc, ident[:])
        identr = pp.tile([P, P], F32R)
        nc.sync.dma_start(identr[:], ex["identr_in"][:].bitcast(F32R))
        identh = pp.tile([P, P], F16)
        nc.sync.dma_start(identh[:], ex["identh_in"][:])
        hid_sl = pp.tile([P, H], F32)
        nc.sync.dma_start(hid_sl[:], ex["hid_slice"][:])
        eps_t = pp.tile([P, 1], F32)
        nc.vector.memset(eps_t[:], EPS)
        emask01 = pp.tile([P, 2 * E], F32)
        nc.sync.dma_start(emask01[:], ex["emask01"][:])
        # shared-expert weights are pure inputs: load them from t=0 so the
        # post-AllGather phase never waits on weight DMAs
        wsg_sb = pp.tile([P, HC * SP * P], F16)
        wsu_sb = pp.tile([P, HC * SP * P], F16)
        for t_, s_ in [(wsg_sb, "ws_g"), (wsu_sb, "ws_u")]:
            nc.sync.dma_start(
                t_[:].rearrange("p (hc m) -> p hc m", hc=HC),
                ex[s_][:].rearrange("(hc p) m -> p hc m", p=P))
        wsd_sb = [pp.tile([P, H], F16, tag=f"wsd{sp}", name=f"wsd{sp}")
                  for sp in range(SP)]
        for sp in range(SP):
            nc.sync.dma_start(wsd_sb[sp][:],
                              ex["ws_d"][sp * P:(sp + 1) * P, :])

        # ======== Phases A-C: attention (fp16) ========
        with tc.tile_pool(name="pab", bufs=1) as pab:
            qT = [pab.tile([P, T], F16, tag=f"qT{j}", name=f"qT{j}")
                  for j in range(2)]
            kT = pab.tile([P, T], F16)
            vT = pab.tile([P, T], F16)
            v_tm = [pab.tile([P, D], F16, tag=f"vtm{b}", name=f"vtm{b}")
                    for b in range(TB)]
            oT = [pab.tile([P, T], F16, tag=f"oT{j}", name=f"oT{j}")
                  for j in range(2)]

            # ---- A: norm + transpose + QKV + rope ----
            with tc.tile_pool(name="pa", bufs=1) as pa, \
                 tc.tile_pool(name="pa2", bufs=3) as pa2:
                cosq = pa.tile([D, T], F32)
                sinq = pa.tile([D, T], F32)
                cosk = pa.tile([D, T], F32)
                sink = pa.tile([D, T], F32)
                for t_, s_ in [(cosq, "cosq"), (sinq, "sinq"),
                               (cosk, "cosk"), (sink, "sink")]:
                    nc.sync.dma_start(t_[:], ex[s_][:])
                permr = pa.tile([P, P], F32R)
                nc.sync.dma_start(permr[:], ex["perm"][:].bitcast(F32R))
                wq_sb = pa.tile([P, HC * 2 * D], F16)
                wk_sb = pa.tile([P, HC * D], F16)
                wv_sb = pa.tile([P, HC * D], F16)
                for t_, s_, m in [(wq_sb, "wq_s", 2 * D), (wk_sb, "wk_s", D),
                                  (wv_sb, "wv_s", D)]:
                    nc.sync.dma_start(
                        t_[:].rearrange("p (hc m) -> p hc m", hc=HC),
                        ex[s_][:].rearrange("(hc p) m -> p hc m", p=P))

                dump = pa.tile([P, H], F32)
                qraw = [pa.tile([P, T], F32R, tag=f"qraw{j}", name=f"qraw{j}")
                        for j in range(2)]
                kraw = pa.tile([P, T], F32R)
                with tc.tile_pool(name="psA1", bufs=2, space="PSUM") as psA1, \
                     tc.tile_pool(name="psA2", bufs=2, space="PSUM") as psA2:
                    for n in range(2):
                        x0T = [pa.tile([P, 512], F16, tag=f"x0T{hc}",
                                       name=f"x0T{hc}_{n}") for hc in range(HC)]
                        for bb in range(TB // 2):
                            b = n * (TB // 2) + bb
                            hidb = pa2.tile([P, H], F32, tag="hidb", bufs=2)
                            nc.sync.dma_start(hidb[:],
                                              ex["hid"][b * P:(b + 1) * P, :])
                            ssum = pa2.tile([P, 1], F32, tag="ssum")
                            nc.scalar.activation(dump[:], hidb[:], AF.Square,
                                                 accum_out=ssum[:, :1])
                            rms = pa2.tile([P, 1], F32, tag="rms")
                            nc.scalar.activation(rms[:], ssum[:],
                                                 AF.Sqrt, bias=eps_t[:, :1],
                                                 scale=1.0 / H)
                            inv = pa2.tile([P, 1], F32, tag="inv")
                            nc.vector.reciprocal(inv[:], rms[:])
                            x0b = pa2.tile([P, H], F16, tag="x0b", bufs=2)
                            nc.vector.tensor_scalar_mul(x0b[:], hidb[:],
                                                        inv[:, :1])
                            for hc in range(HC):
                                tp = psA1.tile([P, P], F16, tag="tpA")
                                nc.tensor.transpose(
                                    tp[:], x0b[:, hc * P:(hc + 1) * P],
                                    identh[:])
                                nc.vector.tensor_copy(
                                    x0T[hc][:, bb * P:(bb + 1) * P], tp[:])

                        def proj(w_sb, m, c0, dst, n=n, x0T=x0T, fp16=False):
                            ps = psA2.tile([P, 512], F32, tag="psQKV",
                                           name="psQKV")
                            for hc in range(HC):
                                nc.tensor.matmul(
                                    ps[:],
                                    w_sb[:, hc * m + c0:hc * m + c0 + P],
                                    x0T[hc][:],
                                    start=(hc == 0), stop=(hc == HC - 1))
                            nc.vector.tensor_copy(
                                dst[:, n * 512:(n + 1) * 512], ps[:])
                        proj(wq_sb, 2 * D, 0, qraw[0])
                        proj(wq_sb, 2 * D, D, qraw[1])
                        proj(wk_sb, D, 0, kraw)
                        proj(wv_sb, D, 0, vT, fp16=True)

                with tc.tile_pool(name="psA3", bufs=2, space="PSUM") as psA3:
                    for src, dst, c_, s_ in [(qraw[0], qT[0], cosq, sinq),
                                             (qraw[1], qT[1], cosq, sinq),
                                             (kraw, kT, cosk, sink)]:
                        for n in range(2):
                            sl = slice(n * 512, (n + 1) * 512)
                            sw = psA3.tile([P, 512], F32, tag="psSW")
                            nc.tensor.matmul(sw[:], permr[:], src[:, sl],
                                             start=True, stop=True)
                            t1 = pa2.tile([P, 512], F32, tag="ropeT1")
                            nc.vector.tensor_mul(t1[:], src[:, sl], c_[:, sl])
                            t2 = pa2.tile([P, 512], F32, tag="ropeT2")
                            nc.vector.tensor_mul(t2[:], sw[:], s_[:, sl])
                            nc.vector.tensor_add(dst[:, sl], t1[:], t2[:])
                    for b in range(TB):
                        tp = psA3.tile([P, P], F16, tag="tpV")
                        nc.tensor.transpose(tp[:], vT[:, b * P:(b + 1) * P],
                                            identh[:])
                        nc.vector.tensor_copy(v_tm[b][:], tp[:])

            # ---- B: attention (causal-block skipped) ----
            with tc.tile_pool(name="pb", bufs=1) as pb, \
                 tc.tile_pool(name="pb2", bufs=3) as pb2:
                dmask = pb.tile([P, P], BF16)
                nc.sync.dma_start(dmask[:], ex["diag_mask"][:])
                wo_sb = [pb.tile([P, H], F16, tag=f"wo{j}", name=f"wo{j}")
                         for j in range(2)]
                nc.sync.dma_start(wo_sb[0][:], ex["wo_s"][0:P, :])
                nc.sync.dma_start(wo_sb[1][:], ex["wo_s"][P:2 * P, :])

                attnT = [pb.tile([P, T], F16, tag=f"attnT{kc}",
                                 name=f"attnT{kc}") for kc in range(TB)]
                for kc in range(1, TB):
                    nc.vector.memset(attnT[kc][:, 0:kc * P], 0.0)
                with tc.tile_pool(name="psB1", bufs=2, space="PSUM") as psB1, \
                     tc.tile_pool(name="psB2", bufs=2, space="PSUM") as psB2, \
                     tc.tile_pool(name="psB3", bufs=2, space="PSUM") as psB3:
                  for h in range(2):
                    for qc in range(TB):
                        cols = (qc + 1) * P
                        prob = pb2.tile([P, T], F32, tag="prob")
                        nsl = (cols + 511) // 512
                        for n in range(nsl):
                            w_ = min(512, cols - n * 512)
                            ps = psB1.tile([P, 512], F32, tag="psSC")
                            nc.tensor.matmul(ps[:, :w_],
                                             qT[h][:, qc * P:(qc + 1) * P],
                                             kT[:, n * 512:n * 512 + w_],
                                             start=True, stop=True)
                            # diagonal block gets the causal mask; the rest
                            # of this slice is fully visible
                            d0 = qc * P - n * 512
                            if 0 <= d0 < w_:
                                if d0 > 0:
                                    nc.vector.tensor_copy(
                                        prob[:, n * 512:n * 512 + d0],
                                        ps[:, :d0])
                                nc.vector.tensor_add(
                                    prob[:, qc * P:qc * P + P],
                                    ps[:, d0:d0 + P], dmask[:])
                            else:
                                nc.vector.tensor_copy(
                                    prob[:, n * 512:n * 512 + w_], ps[:, :w_])
                        mx = pb2.tile([P, 1], F32, tag="mx")
                        nc.vector.reduce_max(mx[:], prob[:, :cols], axis=AX.X)
                        negm = pb2.tile([P, 1], F32, tag="negm")
                        nc.vector.tensor_scalar_mul(negm[:], mx[:], -1.0)
                        ssum = pb2.tile([P, 1], F32, tag="esum")
                        probe_ = pb2.tile([P, T], F32, tag="probe")
                        nc.scalar.activation(probe_[:, :cols], prob[:, :cols],
                                             AF.Exp, bias=negm[:, :1],
                                             accum_out=ssum[:, :1])
                        rec = pb2.tile([P, 1], F32, tag="rec")
                        nc.vector.reciprocal(rec[:], ssum[:])
                        probS = pb2.tile([P, T], F16, tag="probS")
                        nc.vector.tensor_scalar_mul(probS[:, :cols],
                                                    probe_[:, :cols],
                                                    rec[:, :1])
                        for kc in range(qc + 1):
                            tp = psB2.tile([P, P], F16, tag="tpB")
                            nc.tensor.transpose(
                                tp[:], probS[:, kc * P:(kc + 1) * P],
                                identh[:])
                            nc.vector.tensor_copy(
                                attnT[kc][:, qc * P:(qc + 1) * P], tp[:])
                    for n in range(2):
                        sl = slice(n * 512, (n + 1) * 512)
                        kc_hi = 4 * n + 3
                        ps = psB3.tile([P, 512], F32, tag="psAV")
                        for kc in range(kc_hi + 1):
                            nc.tensor.matmul(ps[:], v_tm[kc][:],
                                             attnT[kc][:, sl],
                                             start=(kc == 0),
                                             stop=(kc == kc_hi))
                        nc.vector.tensor_copy(oT[h][:, sl], ps[:])

                # ---- C: o_proj ----
                with tc.tile_pool(name="psC", bufs=8, space="PSUM") as psC:
                    for tb_ in range(TB):
                        pso = [psC.tile([P, 512], F32, tag="psO",
                                        name=f"psO{n}") for n in range(4)]
                        for hp in range(2):
                            for n in range(4):
                                nc.tensor.matmul(
                                    pso[n][:],
                                    oT[hp][:, tb_ * P:(tb_ + 1) * P],
                                    wo_sb[hp][:, n * 512:(n + 1) * 512],
                                    start=(hp == 0), stop=(hp == 1))
                        ob = pb2.tile([P, H], F16, tag="ob", bufs=2)
                        for n in range(4):
                            nc.vector.tensor_copy(
                                ob[:, n * 512:(n + 1) * 512], pso[n][:])
                        nc.sync.dma_start(rs_in[tb_ * P:(tb_ + 1) * P, :],
                                          ob[:])

        nc.gpsimd.collective_compute(
            "ReduceScatter", ALU.add, ins=[rs_in.opt()], outs=[rs_out.opt()],
            replica_groups=[list(range(NCN))])

        # ======== D: residual + norm + local fp32 router + AGs ========
        with tc.tile_pool(name="pd", bufs=1) as pd, \
             tc.tile_pool(name="pd2", bufs=2) as pd2, \
             tc.tile_pool(name="psD", bufs=2, space="PSUM") as psD:
            attn_sl = pd.tile([P, H], F16)
            nc.sync.dma_start(attn_sl[:], rs_out[:])
            res_sb = pd.tile([P, H], F32)
            nc.vector.tensor_add(res_sb[:], hid_sl[:], attn_sl[:])
            nc.sync.dma_start(res_slice[:], res_sb[:])
            dump2 = pd.tile([P, H], F32)
            ssum = pd.tile([P, 1], F32)
            nc.scalar.activation(dump2[:], res_sb[:], AF.Square,
                                 accum_out=ssum[:, :1])
            rms = pd.tile([P, 1], F32)
            nc.scalar.activation(rms[:], ssum[:], AF.Sqrt, bias=eps_t[:, :1],
                                 scale=1.0 / H)
            inv = pd.tile([P, 1], F32)
            nc.vector.reciprocal(inv[:], rms[:])
            x_sl = pd.tile([P, H], F32)
            nc.vector.tensor_scalar_mul(x_sl[:], res_sb[:], inv[:, :1])
            x_sl_h = pd.tile([P, H], F16)
            nc.vector.tensor_copy(x_sl_h[:], x_sl[:])
            nc.sync.dma_start(agx_in[:], x_sl_h[:])

            # exact fp32 router on the un-normalized residual: transposes and
            # logit matmuls run in parallel with the rmsnorm stats, and the
            # 1/rms scale folds into the sigmoid's per-token scale operand
            gw_sb = pd.tile([P, HC * E], F32)
            nc.sync.dma_start(
                gw_sb[:].rearrange("p (hc e) -> p hc e", hc=HC),
                ex["gate_wT"][:].rearrange("(hc p) e -> p hc e", p=P))
            gate_b = pd.tile([P, E], F32)
            nc.sync.dma_start(gate_b[:], ex["gate_b"][:])
            resT = pd.tile([P, H], F32)
            for hc in range(HC):
                tp = psD.tile([P, P], F32, tag="tpD")
                nc.tensor.transpose(tp[:], res_sb[:, hc * P:(hc + 1) * P],
                                    ident[:])
                nc.vector.tensor_copy(resT[:, hc * P:(hc + 1) * P], tp[:])
            lg_ps = psD.tile([P, E], F32, tag="lgps", name="lgps")
            for hc in range(HC):
                nc.tensor.matmul(lg_ps[:], resT[:, hc * P:(hc + 1) * P],
                                 gw_sb[:, hc * E:(hc + 1) * E],
                                 start=(hc == 0), stop=(hc == HC - 1))
            sig = pd2.tile([P, E], F32, tag="sig")
            nc.scalar.activation(sig[:], lg_ps[:], AF.Sigmoid,
                                 scale=inv[:, :1])
            sb_ = pd2.tile([P, E], F32, tag="sb_")
            nc.vector.tensor_add(sb_[:], sig[:], gate_b[:])
            mx = pd2.tile([P, 8], F32, tag="mx8")
            nc.vector.max(out=mx[:], in_=sb_[:])
            s1 = pd2.tile([P, E], F32, tag="s1")
            nc.vector.tensor_tensor(out=s1[:], in0=sb_[:],
                                    in1=mx[:, 0:1].to_broadcast([P, E]),
                                    op=ALU.is_equal)
            s2 = pd2.tile([P, E], F32, tag="s2")
            nc.vector.tensor_tensor(out=s2[:], in0=sb_[:],
                                    in1=mx[:, 1:2].to_broadcast([P, E]),
                                    op=ALU.is_equal)
            nc.vector.tensor_add(s1[:], s1[:], s2[:])
            sel_own = pd2.tile([P, E], F32, tag="sel_own")
            nc.vector.tensor_scalar_min(sel_own[:], s1[:], 1.0)
            wa = pd2.tile([P, E], F32, tag="wa")
            nc.vector.tensor_mul(wa[:], sel_own[:], sig[:])
            nrm = pd2.tile([P, 1], F32, tag="nrm")
            nc.vector.reduce_sum(nrm[:], wa[:], axis=AX.X)
            rec = pd2.tile([P, 1], F32, tag="recw")
            nc.vector.reciprocal(rec[:], nrm[:])
            w_tm = pd2.tile([P, E], F32, tag="wtm")
            nc.vector.tensor_scalar_mul(w_tm[:], wa[:], rec[:, :1])
            nc.sync.dma_start(agw_in[:, 0:E], w_tm[:])
            nc.sync.dma_start(agw_in[:, E:2 * E], sel_own[:])
            nc.sync.dma_start(dbg_w[:], w_tm[:])

        cc_w = nc.gpsimd.collective_compute(
            "AllGather", ALU.bypass, ins=[agw_in.opt()], outs=[w_all.opt()],
            replica_groups=[list(range(NCN))])
        cc_x = nc.gpsimd.collective_compute(
            "AllGather", ALU.bypass, ins=[agx_in.opt()], outs=[x_tm.opt()],
            replica_groups=[list(range(NCN))])
        # The tiny router AllGather must run first so the token-list build
        # overlaps the big x AllGather (CC queue executes in trigger order).
        add_dep_helper(cc_x.ins, cc_w.ins, sync=True,
                       reason="AG_W before AG_x")

        # ======== E: token lists from AllGathered router decisions ========
        # Inverse permutation (slot -> token id) built with matmuls instead of
        # 16 serialized indirect scatters: M[token, slot] = (rank == slot),
        # tok_list[slot] = sum_t M[t, slot] * t, with +BIG for empty slots.
        with tc.tile_pool(name="pe", bufs=1) as pe, \
             tc.tile_pool(name="pe2", bufs=3) as pe2, \
             tc.tile_pool(name="psE", bufs=2, space="PSUM") as psE, \
             tc.tile_pool(name="psE2", bufs=1, space="PSUM") as psE2:
            ut = pe.tile([P, P], F32R)
            nc.sync.dma_start(ut[:], ex["ut_in"][:].bitcast(F32R))
            slb = pe.tile([8, TB * P], F32R)
            nc.sync.dma_start(slb[:], ex["slb_in"][:].bitcast(F32R))
            s_iota = pe.tile([P, CAP], F32)
            nc.sync.dma_start(s_iota[:], ex["slot_iota"][:])
            tokid2 = pe.tile([P, 2 * TB], F16)
            nc.sync.dma_start(tokid2[:], ex["tokid2"][:])
            totals = pe.tile([8, E], F32R)
            pre_sb = [pe.tile([P, E], F32, tag=f"pre{b}", name=f"pre{b}")
                      for b in range(TB)]
            sel_all = [pe.tile([P, E], F32, tag=f"sela{b}", name=f"sela{b}")
                       for b in range(TB)]
            for b in range(TB):
                nc.sync.dma_start(sel_all[b][:],
                                  w_all[b * P:(b + 1) * P, E:2 * E])
                pr_ps = psE.tile([P, E], F32, tag="prps")
                nc.tensor.matmul(pr_ps[:], ut[:],
                                 sel_all[b][:].bitcast(F32R),
                                 start=True, stop=True)
                nc.vector.tensor_copy(pre_sb[b][:], pr_ps[:])
                nc.sync.dma_start(totals[b:b + 1, :],
                                  pre_sb[b][127:128, :].bitcast(F32R))
            tl_ps = [[psE2.tile([P, 2], F32, tag=f"tl{ei}{ch}",
                                name=f"tl{ei}{ch}") for ch in range(2)]
                     for ei in range(2)]
            for b in range(TB):
                ofs_ps = psE.tile([P, E], F32, tag="ofsps", name="ofsps")
                nc.tensor.matmul(ofs_ps[:], slb[:, b * P:(b + 1) * P],
                                 totals[:], start=True, stop=True)
                grank = pe2.tile([P, E], F32, tag="grank")
                nc.vector.tensor_add(grank[:], pre_sb[b][:], ofs_ps[:])
                nc.vector.tensor_scalar_add(grank[:], grank[:], -1.0)
                gm = pe2.tile([P, E], F32, tag="gm")
                nc.vector.tensor_scalar(out=gm[:], in0=grank[:],
                                        scalar1=float(CAP - 1), scalar2=BIG,
                                        op0=ALU.is_gt, op1=ALU.mult)
                nc.vector.tensor_add(grank[:], grank[:], gm[:])
                um = pe2.tile([P, E], F32, tag="um")
                nc.vector.tensor_scalar(out=um[:], in0=sel_all[b][:],
                                        scalar1=-BIG, scalar2=BIG,
                                        op0=ALU.mult, op1=ALU.add)
                nc.vector.tensor_add(grank[:], grank[:], um[:])
                for ei in range(2):
                    ge = pe2.tile([P, E], F32, tag="ge")
                    nc.vector.tensor_mul(ge[:], grank[:],
                                         emask01[:, ei * E:(ei + 1) * E])
                    ridx = pe2.tile([P, 1], F32, tag="ridx")
                    nc.vector.reduce_sum(ridx[:], ge[:], axis=AX.X)
                    mb = pe2.tile([P, CAP], F16, tag="mb")
                    nc.vector.tensor_tensor(
                        out=mb[:], in0=s_iota[:],
                        in1=ridx[:, 0:1].to_broadcast([P, CAP]),
                        op=ALU.is_equal)
                    for ch in range(2):
                        nc.tensor.matmul(tl_ps[ei][ch][:],
                                         mb[:, ch * P:(ch + 1) * P],
                                         tokid2[:, 2 * b:2 * b + 2],
                                         start=(b == 0), stop=(b == TB - 1))
            for ei in range(2):
                for ch in range(2):
                    tl = pe2.tile([P, 2], F32, tag="tlsb")
                    nc.vector.tensor_copy(tl[:], tl_ps[ei][ch][:])
                    pad = pe2.tile([P, 1], F32, tag="pad")
                    nc.vector.tensor_scalar(out=pad[:], in0=tl[:, 1:2],
                                            scalar1=-BIG, scalar2=BIG,
                                            op0=ALU.mult, op1=ALU.add)
                    tok_f = pe2.tile([P, 1], F32, tag="tokf")
                    nc.vector.tensor_add(tok_f[:], tl[:, 0:1], pad[:])
                    tok_i = pe2.tile([P, 1], I32, tag="toki")
                    nc.vector.tensor_copy(tok_i[:], tok_f[:])
                    nc.sync.dma_start(
                        tok_lists[ei * CAP + ch * P:
                                  ei * CAP + (ch + 1) * P, :], tok_i[:])

        # ======== F: xT + shared expert + experts (fp16) ========
        with tc.tile_pool(name="pxt", bufs=1) as pxt, \
             tc.tile_pool(name="pfs", bufs=1) as pfs, \
             tc.tile_pool(name="pfs2", bufs=2) as pfs2:
            xc = [pxt.tile([P, T], F16, tag=f"xc{hc}", name=f"xc{hc}")
                  for hc in range(HC)]
            with tc.tile_pool(name="pxt2", bufs=3) as pxt2, \
                 tc.tile_pool(name="psX", bufs=2, space="PSUM") as psX:
                for b in range(TB):
                    xb = pxt2.tile([P, H], F16, tag="xb", bufs=2)
                    nc.sync.dma_start(xb[:], x_tm[b * P:(b + 1) * P, :])
                    for hc in range(HC):
                        tp = psX.tile([P, P], F16, tag="tpX")
                        nc.tensor.transpose(tp[:], xb[:, hc * P:(hc + 1) * P],
                                            identh[:])
                        nc.vector.tensor_copy(xc[hc][:, b * P:(b + 1) * P],
                                              tp[:])

            # ---- both experts' setup: token lists, gathers, gxT, weights,
            # per-token gate weights — overlaps the shared expert below ----
            idx_sb2 = [[pfs.tile([P, 1], I32, tag=f"idx{ei}{k}",
                                 name=f"idx{ei}{k}") for k in range(2)]
                       for ei in range(2)]
            gxT2 = [pfs.tile([P, HC * 2 * P], F16, tag=f"gxT{ei}",
                             name=f"gxT{ei}") for ei in range(2)]
            wd_res2 = [[pfs.tile([P, H], F16, tag=f"wd{ei}{ip}",
                                 name=f"wd{ei}{ip}") for ip in range(IP)]
                       for ei in range(2)]
            wg_own2 = [[pfs.tile([P, 1], F32, tag=f"wgo{ei}{k}",
                                 name=f"wgo{ei}{k}") for k in range(2)]
                       for ei in range(2)]
            psS_cm = tc.tile_pool(name="psS", bufs=2, space="PSUM")
            psS = psS_cm.__enter__()
            for ei in range(2):
                for k in range(2):
                    nc.sync.dma_start(
                        idx_sb2[ei][k][:],
                        tok_lists[ei * CAP + k * P:ei * CAP + (k + 1) * P, :])
                    gx = pfs2.tile([P, H], F16, tag="gx")
                    nc.vector.memset(gx[:], 0.0)
                    nc.gpsimd.indirect_dma_start(
                        out=gx[:], out_offset=None,
                        in_=x_tm[:],
                        in_offset=bass.IndirectOffsetOnAxis(
                            ap=idx_sb2[ei][k][:, :1], axis=0),
                        bounds_check=T - 1, oob_is_err=False)
                    for hc in range(HC):
                        tp = psS.tile([P, P], F16, tag="tpS")
                        nc.tensor.transpose(tp[:], gx[:, hc * P:(hc + 1) * P],
                                            identh[:])
                        nc.vector.tensor_copy(
                            gxT2[ei][:, hc * 2 * P + k * P:
                                  hc * 2 * P + (k + 1) * P], tp[:])
                    wrow = pfs2.tile([P, 2 * E], F32, tag="wrow")
                    nc.vector.memset(wrow[:], 0.0)
                    nc.gpsimd.indirect_dma_start(
                        out=wrow[:], out_offset=None, in_=w_all[:],
                        in_offset=bass.IndirectOffsetOnAxis(
                            ap=idx_sb2[ei][k][:, :1], axis=0),
                        bounds_check=T - 1, oob_is_err=False)
                    we_ = pfs2.tile([P, E], F32, tag="we_")
                    nc.vector.tensor_mul(we_[:], wrow[:, 0:E],
                                         emask01[:, ei * E:(ei + 1) * E])
                    nc.vector.reduce_sum(wg_own2[ei][k][:], we_[:], axis=AX.X)

            # ---- shared expert ----
            with tc.tile_pool(name="pg", bufs=1) as pg, \
                 tc.tile_pool(name="pg2", bufs=3) as pg2:
                g_act = [pg.tile([P, T], F16, tag=f"gact{sp}", name=f"gact{sp}")
                         for sp in range(SP)]
                hs = [pg.tile([P, T], F16, tag=f"hs{sp}", name=f"hs{sp}")
                      for sp in range(SP)]
                with tc.tile_pool(name="psG1", bufs=1, space="PSUM") as psG1:
                    g_ps = [psG1.tile([P, T], F32, tag=f"gps{sp}",
                                      name=f"gps{sp}") for sp in range(SP)]
                    for hc in range(HC):
                        for sp in range(SP):
                            c0 = hc * SP * P + sp * P
                            for n in range(2):
                                sl = slice(n * 512, (n + 1) * 512)
                                nc.tensor.matmul(g_ps[sp][:, sl],
                                                 wsg_sb[:, c0:c0 + P],
                                                 xc[hc][:, sl],
                                                 start=(hc == 0),
                                                 stop=(hc == HC - 1))
                    for sp in range(SP):
                        nc.scalar.activation(g_act[sp][:], g_ps[sp][:],
                                             AF.Silu)
                with tc.tile_pool(name="psG2", bufs=1, space="PSUM") as psG2:
                    u_ps = [psG2.tile([P, T], F32, tag=f"ups{sp}",
                                      name=f"ups{sp}") for sp in range(SP)]
                    for hc in range(HC):
                        for sp in range(SP):
                            c0 = hc * SP * P + sp * P
                            for n in range(2):
                                sl = slice(n * 512, (n + 1) * 512)
                                nc.tensor.matmul(u_ps[sp][:, sl],
                                                 wsu_sb[:, c0:c0 + P],
                                                 xc[hc][:, sl],
                                                 start=(hc == 0),
                                                 stop=(hc == HC - 1))
                    for sp in range(SP):
                        nc.vector.tensor_mul(hs[sp][:], g_act[sp][:],
                                             u_ps[sp][:])
                with tc.tile_pool(name="psG3", bufs=6, space="PSUM") as psG3:
                    for tb_ in range(TB):
                        psd = [psG3.tile([P, 512], F32, tag="psGd",
                                         name=f"psGd{n}") for n in range(4)]
                        for sp in range(SP):
                            for n in range(4):
                                nc.tensor.matmul(
                                    psd[n][:],
                                    hs[sp][:, tb_ * P:(tb_ + 1) * P],
                                    wsd_sb[sp][:, n * 512:(n + 1) * 512],
                                    start=(sp == 0), stop=(sp == SP - 1))
                        sbd = pg2.tile([P, H], F16, tag="sbGd", bufs=2)
                        for n in range(4):
                            nc.vector.tensor_copy(
                                sbd[:, n * 512:(n + 1) * 512], psd[n][:])
                        nc.sync.dma_start(rs2_in[tb_ * P:(tb_ + 1) * P, :],
                                          sbd[:])

            psS_cm.__exit__(None, None, None)

            # expert down-proj weights: emitted late so these 8 MB of DMAs
            # sit behind the x-block/gather traffic in queue priority, but
            # they still have ~100us of slack before first use
            for ei in range(2):
                for ip in range(IP):
                    nc.sync.dma_start(wd_res2[ei][ip][:],
                                      ex["we_d"][ei, ip * P:(ip + 1) * P, :])

            # ---- experts (setup already done above) ----
            for ei in range(2):
                with tc.tile_pool(name=f"pf{ei}", bufs=1) as pf, \
                     tc.tile_pool(name=f"pf2{ei}", bufs=2) as pf2:
                    idx_sb = idx_sb2[ei]
                    gxT = gxT2[ei]
                    wd_res = wd_res2[ei]

                    # merged gate+up pass (8 PSUM banks)
                    g_tm = [pf.tile([P, I], F16, tag=f"gtm{k}", name=f"gtm{k}")
                            for k in range(2)]
                    h_tm = [pf.tile([P, I], F16, tag=f"htm{k}", name=f"htm{k}")
                            for k in range(2)]
                    with tc.tile_pool(name=f"psF2{ei}", bufs=1,
                                      space="PSUM") as psF2:
                        gu_ps = [[psF2.tile([P, 512], F32, tag=f"gups{k}{j}",
                                            name=f"gups{k}{j}")
                                  for j in range(4)] for k in range(2)]
                        for hc in range(HC):
                            wg_c = pf2.tile([P, I], F16, tag="wgF", bufs=3)
                            nc.sync.dma_start(
                                wg_c[:], ex["we_g"][ei, hc * P:(hc + 1) * P, :])
                            wu_c = pf2.tile([P, I], F16, tag="wuF", bufs=3)
                            nc.sync.dma_start(
                                wu_c[:], ex["we_u"][ei, hc * P:(hc + 1) * P, :])
                            for k in range(2):
                                s_ = gxT[:, hc * 2 * P + k * P:
                                         hc * 2 * P + (k + 1) * P]
                                for n in range(2):
                                    nc.tensor.matmul(
                                        gu_ps[k][n][:], s_,
                                        wg_c[:, n * 512:(n + 1) * 512],
                                        start=(hc == 0), stop=(hc == HC - 1))
                                for n in range(2):
                                    nc.tensor.matmul(
                                        gu_ps[k][2 + n][:], s_,
                                        wu_c[:, n * 512:(n + 1) * 512],
                                        start=(hc == 0), stop=(hc == HC - 1))
                        for k in range(2):
                            for n in range(2):
                                sl = slice(n * 512, (n + 1) * 512)
                                nc.scalar.activation(g_tm[k][:, sl],
                                                     gu_ps[k][n][:], AF.Silu)
                                nc.vector.tensor_mul(h_tm[k][:, sl],
                                                     g_tm[k][:, sl],
                                                     gu_ps[k][2 + n][:])
                    h_sb = [pf.tile([P, 2 * P], F16, tag=f"hsb{ip}",
                                    name=f"hsb{ip}") for ip in range(IP)]
                    with tc.tile_pool(name=f"psF4{ei}", bufs=2,
                                      space="PSUM") as psF4:
                        for k in range(2):
                            for ip in range(IP):
                                tp = psF4.tile([P, P], F16, tag="tpF2")
                                nc.tensor.transpose(
                                    tp[:], h_tm[k][:, ip * P:(ip + 1) * P],
                                    identh[:])
                                nc.vector.tensor_copy(
                                    h_sb[ip][:, k * P:(k + 1) * P], tp[:])
                    with tc.tile_pool(name=f"psF5{ei}", bufs=8,
                                      space="PSUM") as psF5:
                        for k in range(2):
                            psd = [psF5.tile([P, 512], F32, tag="psFd",
                                             name=f"psFd{n}")
                                   for n in range(4)]
                            for ip in range(IP):
                                for n in range(4):
                                    nc.tensor.matmul(
                                        psd[n][:],
                                        h_sb[ip][:, k * P:(k + 1) * P],
                                        wd_res[ip][:, n * 512:(n + 1) * 512],
                                        start=(ip == 0), stop=(ip == IP - 1))
                            out_sb = pf.tile([P, H], F16, tag=f"outsb{k}")
                            for n in range(4):
                                nc.vector.tensor_scalar_mul(
                                    out_sb[:, n * 512:(n + 1) * 512],
                                    psd[n][:], wg_own2[ei][k][:, :1])
                            nc.gpsimd.indirect_dma_start(
                                out=rs2_in[:],
                                out_offset=bass.IndirectOffsetOnAxis(
                                    ap=idx_sb[k][:, :1], axis=0),
                                in_=out_sb[:], in_offset=None,
                                bounds_check=T - 1, oob_is_err=False,
                                compute_op=ALU.add)

        nc.gpsimd.collective_compute(
            "ReduceScatter", ALU.add, ins=[rs2_in.opt()], outs=[rs2_out.opt()],
            replica_groups=[list(range(NCN))])
        with tc.tile_pool(name="pz", bufs=2) as pz:
            fin16 = pz.tile([P, H], F16)
            nc.sync.dma_start(fin16[:], rs2_out[:])
            fin = pz.tile([P, H], F32)
            nc.vector.tensor_copy(fin[:], fin16[:])
            nc.sync.dma_start(out_slice[:], fin[:])


_CACHE = {}


def _build():
    key = "nc"
    if key in _CACHE:
        return _CACHE[key]
    nc = bacc.Bacc("TRN2", target_bir_lowering=False, debug=False,
                   num_devices=NCN)
    with tile.TileContext(nc) as tc:
        _emit(nc, tc)
    nc.compile()
    _CACHE[key] = nc
    return nc


def _host_prep(inputs):
    f16 = np.float16
    pos = np.asarray(inputs["positions"]).astype(np.float64)
    hid = np.asarray(inputs["hidden_states"], np.float32)
    w_in = np.asarray(inputs["w_in_ln"], np.float32)
    w_post = np.asarray(inputs["w_post_ln"], np.float32)
    wq = (np.asarray(inputs["wq"], np.float32) * w_in[:, None]).astype(f16)
    wk = (np.asarray(inputs["wk"], np.float32) * w_in[:, None]).astype(f16)
    wv = (np.asarray(inputs["wv"], np.float32) * w_in[:, None]).astype(f16)
    wo = np.asarray(inputs["wo"], np.float32).astype(f16)
    gate_w = np.asarray(inputs["gate_w"], np.float32) * w_post[None, :]
    gate_b = np.asarray(inputs["gate_bias"], np.float32).reshape(1, E)
    we_g = (np.asarray(inputs["we_gate"], np.float32)
            * w_post[None, :, None]).astype(f16)
    we_u = (np.asarray(inputs["we_up"], np.float32)
            * w_post[None, :, None]).astype(f16)
    we_d = np.asarray(inputs["we_down"], np.float32).astype(f16)
    ws_g = (np.asarray(inputs["ws_gate"], np.float32)
            * w_post[:, None]).astype(f16)
    ws_u = (np.asarray(inputs["ws_up"], np.float32)
            * w_post[:, None]).astype(f16)
    ws_d = np.asarray(inputs["ws_down"], np.float32).astype(f16)

    inv_freq = 1.0 / (THETA ** (np.arange(0, D, 2, dtype=np.float64) / D))
    f = pos[None, :] * inv_freq[:, None]
    cos2, sin2 = np.cos(f), np.sin(f)
    cosT = np.repeat(cos2, 2, axis=0).astype(np.float32)
    sinT = np.empty((D, T), np.float32)
    sinT[0::2] = -sin2
    sinT[1::2] = sin2
    s = 1.0 / np.sqrt(D)
    cosq, sinq = (cosT * s).astype(np.float32), (sinT * s).astype(np.float32)

    import ml_dtypes
    bf = ml_dtypes.bfloat16
    ii = np.arange(P)
    diag_mask = np.where(ii[:, None] >= ii[None, :], 0.0, NEG).astype(bf)

    identr_in = np.eye(P, dtype=np.float32)
    identh_in = np.eye(P, dtype=f16)
    ut_in = np.triu(np.ones((P, P), np.float32))
    slb_in = np.zeros((8, TB * P), np.float32)
    for b in range(TB):
        slb_in[:b, b * P:(b + 1) * P] = 1.0
    perm = np.zeros((P, P), np.float32)
    for i in range(0, P, 2):
        perm[i, i + 1] = 1.0
        perm[i + 1, i] = 1.0
    slot_iota = np.broadcast_to(np.arange(CAP, dtype=np.float32),
                                (P, CAP)).copy()
    tokid2 = np.zeros((P, 2 * TB), f16)
    for b in range(TB):
        tokid2[:, 2 * b] = (b * P + np.arange(P)).astype(f16)
        tokid2[:, 2 * b + 1] = 1.0

    ISC = IS // NCN
    maps = []
    for c in range(NCN):
        g = c // 2
        emask01 = np.zeros((P, 2 * E), np.float32)
        emask01[:, 2 * c] = 1.0          # ei = 0 -> expert 2c
        emask01[:, E + 2 * c + 1] = 1.0  # ei = 1 -> expert 2c+1
        maps.append({
            "hid": hid,
            "hid_slice": np.ascontiguousarray(hid[c * P:(c + 1) * P]),
            "wq_s": np.ascontiguousarray(wq[:, 2 * c * D:(2 * c + 2) * D]),
            "wk_s": np.ascontiguousarray(wk[:, g * D:(g + 1) * D]),
            "wv_s": np.ascontiguousarray(wv[:, g * D:(g + 1) * D]),
            "wo_s": np.ascontiguousarray(wo[2 * c * D:(2 * c + 2) * D, :]),
            "cosq": cosq, "sinq": sinq, "cosk": cosT, "sink": sinT,
            "perm": perm, "diag_mask": diag_mask,
            "identr_in": identr_in, "identh_in": identh_in,
            "ut_in": ut_in, "slb_in": slb_in,
            "slot_iota": slot_iota, "tokid2": tokid2,
            "gate_wT": np.ascontiguousarray(gate_w.T),
            "gate_b": np.broadcast_to(gate_b, (P, E)).copy(),
            "emask01": emask01,
            "ws_g": np.ascontiguousarray(ws_g[:, c * ISC:(c + 1) * ISC]),
            "ws_u": np.ascontiguousarray(ws_u[:, c * ISC:(c + 1) * ISC]),
            "ws_d": np.ascontiguousarray(ws_d[c * ISC:(c + 1) * ISC, :]),
            "we_g": np.ascontiguousarray(we_g[2 * c:2 * c + 2]),
            "we_u": np.ascontiguousarray(we_u[2 * c:2 * c + 2]),
            "we_d": np.ascontiguousarray(we_d[2 * c:2 * c + 2]),
        })
    return maps


def kernel(trace=False, **inputs):
    nc = _build()
    maps = _host_prep(inputs)
    res = bass_utils.run_bass_kernel_spmd(
        nc, maps, core_ids=list(range(NCN)), trace=trace)
    out = np.concatenate([res.results[c]["out_slice"] for c in range(NCN)], 0)
    resid = np.concatenate([res.results[c]["res_slice"] for c in range(NCN)], 0)
    kernel.last_results = res
    return out, resid


# revision 30
# speedup vs baseline: 1.6050x; 1.0575x over previous
"""Ernie4 decoder layer (RMSNorm + GQA attention + shared expert + 16-expert
top-2 MoE) on 8 Trainium2 NeuronCores.

v2 — fp16 data path everywhere except the router (which must reproduce the
reference top-2 selection exactly; margins are ~3e-5 so it stays fp32 and is
computed locally per core before the AllGather):
  - Attention: head-parallel (2 q-heads + 1 kv-head per core), fp16 QKV /
    scores / probs / o_proj with causal-block skipping; fp16 ReduceScatter.
  - Router: fp32 logits on each core's own 128 tokens; W+sel AllGathered in a
    tiny fp32 collective that precedes the fp16 x AllGather so the token-list
    build overlaps it.
  - Shared expert: intermediate-sharded (IS/8 per core) fp16, output seeds
    the MoE combine buffer.
  - MoE: expert-parallel (2 experts per core), token lists via
    triangular-matmul prefix ranks, indirect-DMA gather/scatter-add in fp16,
    fp16 ReduceScatter for the combine.
"""
import sys
sys.path.insert(0, "/opt/trn_rl_repo")

import numpy as np

import concourse.bass as bass
import concourse.bacc as bacc
import concourse.tile as tile
import concourse.mybir as mybir
from concourse import bass_utils
from concourse.masks import make_identity
from concourse.tile import add_dep_helper

dt = mybir.dt
F32 = dt.float32
F32R = dt.float32r
F16 = dt.float16
I32 = dt.int32
BF16 = dt.bfloat16
AF = mybir.ActivationFunctionType
ALU = mybir.AluOpType
AX = mybir.AxisListType

T, H, NH, NKV, D = 1024, 2048, 16, 4, 128
E, I, IS = 16, 1024, 2048
EPS = 1e-6
THETA = 10000.0
NCN = 8
P = 128
TB = T // P            # 8 token blocks
HC = H // P            # 16 hidden chunks
IP = I // P            # 8 expert-intermediate chunks
SP = IS // NCN // P    # 2 shared-intermediate chunks per core
CAP = 256              # per-expert token capacity
BIG = 1.0e6            # OOB sentinel
NEG = -1e9


def _emit(nc, tc):
    ex = {}
    for name, shape, d in [
        ("hid", [T, H], F32), ("hid_slice", [P, H], F32),
        ("wq_s", [H, 2 * D], F16), ("wk_s", [H, D], F16), ("wv_s", [H, D], F16),
        ("wo_s", [2 * D, H], F16),
        ("cosq", [D, T], F32), ("sinq", [D, T], F32),
        ("cosk", [D, T], F32), ("sink", [D, T], F32),
        ("perm", [P, P], F32),
        ("diag_mask", [P, P], BF16),
        ("gate_wT", [H, E], F32), ("gate_b", [P, E], F32),
        ("emask01", [P, 2 * E], F32),
        ("ws_g", [H, SP * P], F16), ("ws_u", [H, SP * P], F16),
        ("ws_d", [SP * P, H], F16),
        ("we_g", [2, H, I], F16), ("we_u", [2, H, I], F16),
        ("we_d", [2, I, H], F16),
        ("identr_in", [P, P], F32), ("identh_in", [P, P], F16),
        ("ut_in", [P, P], F32),
        ("slb_in", [8, TB * P], F32),
        ("slot_iota", [P, CAP], F32), ("tokid2", [P, 2 * TB], F16),
    ]:
        ex[name] = nc.dram_tensor(name, shape, d, kind="ExternalInput").ap()
    out_slice = nc.dram_tensor("out_slice", [P, H], F32, kind="ExternalOutput").ap()
    res_slice = nc.dram_tensor("res_slice", [P, H], F32, kind="ExternalOutput").ap()
    dbg_w = nc.dram_tensor("dbg_w", [P, E], F32, kind="ExternalOutput").ap()

    with tc.tile_pool(name="persist", bufs=1) as pp, \
         tc.tile_pool(name="dram", bufs=1, space="DRAM") as dram:
        rs_in = dram.tile([T, H], F16)
        rs_out = dram.tile([P, H], F16)
        agw_in = dram.tile([P, 2 * E], F32)
        w_all = dram.tile([T, 2 * E], F32, addr_space="Shared")
        agx_in = dram.tile([P, H], F16)
        x_tm = dram.tile([T, H], F16, addr_space="Shared")
        rs2_in = dram.tile([T, H], F16)
        rs2_out = dram.tile([P, H], F16)

        ident = pp.tile([P, P], F32)
        make_identity(nc, ident[:])
        identr = pp.tile([P, P], F32R)
        nc.sync.dma_start(identr[:], ex["identr_in"][:].bitcast(F32R))
        identh = pp.tile([P, P], F16)
        nc.sync.dma_start(identh[:], ex["identh_in"][:])
        hid_sl = pp.tile([P, H], F32)
        nc.sync.dma_start(hid_sl[:], ex["hid_slice"][:])
        eps_t = pp.tile([P, 1], F32)
        nc.vector.memset(eps_t[:], EPS)
        emask01 = pp.tile([P, 2 * E], F32)
        nc.sync.dma_start(emask01[:], ex["emask01"][:])
        # per-expert token lists live in SBUF end-to-end (built by the
        # matmul-based inverse permutation in phase E, consumed in F)
        idx_sb2 = [[pp.tile([P, 1], I32, tag=f"idx{ei}{k}",
                            name=f"idx{ei}{k}") for k in range(2)]
                   for ei in range(2)]
        # shared-expert weights are pure inputs: load them from t=0 so the
        # post-AllGather phase never waits on weight DMAs
        wsg_sb = pp.tile([P, HC * SP * P], F16)
        wsu_sb = pp.tile([P, HC * SP * P], F16)
        for t_, s_ in [(wsg_sb, "ws_g"), (wsu_sb, "ws_u")]:
            nc.sync.dma_start(
                t_[:].rearrange("p (hc m) -> p hc m", hc=HC),
                ex[s_][:].rearrange("(hc p) m -> p hc m", p=P))
        wsd_sb = [pp.tile([P, H], F16, tag=f"wsd{sp}", name=f"wsd{sp}")
                  for sp in range(SP)]
        for sp in range(SP):
            nc.sync.dma_start(wsd_sb[sp][:],
                              ex["ws_d"][sp * P:(sp + 1) * P, :])

        # ======== Phases A-C: attention (fp16) ========
        with tc.tile_pool(name="pab", bufs=1) as pab:
            qT = [pab.tile([P, T], F16, tag=f"qT{j}", name=f"qT{j}")
                  for j in range(2)]
            kT = pab.tile([P, T], F16)
            vT = pab.tile([P, T], F16)
            v_tm = [pab.tile([P, D], F16, tag=f"vtm{b}", name=f"vtm{b}")
                    for b in range(TB)]
            oT = [pab.tile([P, T], F16, tag=f"oT{j}", name=f"oT{j}")
                  for j in range(2)]

            # ---- A: norm + transpose + QKV + rope ----
            with tc.tile_pool(name="pa", bufs=1) as pa, \
                 tc.tile_pool(name="pa2", bufs=3) as pa2:
                cosq = pa.tile([D, T], F32)
                sinq = pa.tile([D, T], F32)
                cosk = pa.tile([D, T], F32)
                sink = pa.tile([D, T], F32)
                for t_, s_ in [(cosq, "cosq"), (sinq, "sinq"),
                               (cosk, "cosk"), (sink, "sink")]:
                    nc.sync.dma_start(t_[:], ex[s_][:])
                permr = pa.tile([P, P], F32R)
                nc.sync.dma_start(permr[:], ex["perm"][:].bitcast(F32R))
                wq_sb = pa.tile([P, HC * 2 * D], F16)
                wk_sb = pa.tile([P, HC * D], F16)
                wv_sb = pa.tile([P, HC * D], F16)
                for t_, s_, m in [(wq_sb, "wq_s", 2 * D), (wk_sb, "wk_s", D),
                                  (wv_sb, "wv_s", D)]:
                    nc.sync.dma_start(
                        t_[:].rearrange("p (hc m) -> p hc m", hc=HC),
                        ex[s_][:].rearrange("(hc p) m -> p hc m", p=P))

                dump = pa.tile([P, H], F32)
                qraw = [pa.tile([P, T], F32R, tag=f"qraw{j}", name=f"qraw{j}")
                        for j in range(2)]
                kraw = pa.tile([P, T], F32R)
                with tc.tile_pool(name="psA1", bufs=2, space="PSUM") as psA1, \
                     tc.tile_pool(name="psA2", bufs=2, space="PSUM") as psA2:
                    for n in range(2):
                        x0T = [pa.tile([P, 512], F16, tag=f"x0T{hc}",
                                       name=f"x0T{hc}_{n}") for hc in range(HC)]
                        for bb in range(TB // 2):
                            b = n * (TB // 2) + bb
                            hidb = pa2.tile([P, H], F32, tag="hidb", bufs=2)
                            nc.sync.dma_start(hidb[:],
                                              ex["hid"][b * P:(b + 1) * P, :])
                            ssum = pa2.tile([P, 1], F32, tag="ssum")
                            nc.scalar.activation(dump[:], hidb[:], AF.Square,
                                                 accum_out=ssum[:, :1])
                            rms = pa2.tile([P, 1], F32, tag="rms")
                            nc.scalar.activation(rms[:], ssum[:],
                                                 AF.Sqrt, bias=eps_t[:, :1],
                                                 scale=1.0 / H)
                            inv = pa2.tile([P, 1], F32, tag="inv")
                            nc.vector.reciprocal(inv[:], rms[:])
                            x0b = pa2.tile([P, H], F16, tag="x0b", bufs=2)
                            nc.vector.tensor_scalar_mul(x0b[:], hidb[:],
                                                        inv[:, :1])
                            for hc in range(HC):
                                tp = psA1.tile([P, P], F16, tag="tpA")
                                nc.tensor.transpose(
                                    tp[:], x0b[:, hc * P:(hc + 1) * P],
                                    identh[:])
                                nc.vector.tensor_copy(
                                    x0T[hc][:, bb * P:(bb + 1) * P], tp[:])

                        def proj(w_sb, m, c0, dst, n=n, x0T=x0T, fp16=False):
                            ps = psA2.tile([P, 512], F32, tag="psQKV",
                                           name="psQKV")
                            for hc in range(HC):
                                nc.tensor.matmul(
                                    ps[:],
                                    w_sb[:, hc * m + c0:hc * m + c0 + P],
                                    x0T[hc][:],
                                    start=(hc == 0), stop=(hc == HC - 1))
                            nc.vector.tensor_copy(
                                dst[:, n * 512:(n + 1) * 512], ps[:])
                        proj(wq_sb, 2 * D, 0, qraw[0])
                        proj(wq_sb, 2 * D, D, qraw[1])
                        proj(wk_sb, D, 0, kraw)
                        proj(wv_sb, D, 0, vT, fp16=True)

                with tc.tile_pool(name="psA3", bufs=2, space="PSUM") as psA3:
                    for src, dst, c_, s_ in [(qraw[0], qT[0], cosq, sinq),
                                             (qraw[1], qT[1], cosq, sinq),
                                             (kraw, kT, cosk, sink)]:
                        for n in range(2):
                            sl = slice(n * 512, (n + 1) * 512)
                            sw = psA3.tile([P, 512], F32, tag="psSW")
                            nc.tensor.matmul(sw[:], permr[:], src[:, sl],
                                             start=True, stop=True)
                            t1 = pa2.tile([P, 512], F32, tag="ropeT1")
                            nc.vector.tensor_mul(t1[:], src[:, sl], c_[:, sl])
                            t2 = pa2.tile([P, 512], F32, tag="ropeT2")
                            nc.vector.tensor_mul(t2[:], sw[:], s_[:, sl])
                            nc.vector.tensor_add(dst[:, sl], t1[:], t2[:])
                    for b in range(TB):
                        tp = psA3.tile([P, P], F16, tag="tpV")
                        nc.tensor.transpose(tp[:], vT[:, b * P:(b + 1) * P],
                                            identh[:])
                        nc.vector.tensor_copy(v_tm[b][:], tp[:])

            # ---- B: attention (causal-block skipped) ----
            with tc.tile_pool(name="pb", bufs=1) as pb, \
                 tc.tile_pool(name="pb2", bufs=3) as pb2:
                dmask = pb.tile([P, P], BF16)
                nc.sync.dma_start(dmask[:], ex["diag_mask"][:])
                wo_sb = [pb.tile([P, H], F16, tag=f"wo{j}", name=f"wo{j}")
                         for j in range(2)]
                nc.sync.dma_start(wo_sb[0][:], ex["wo_s"][0:P, :])
                nc.sync.dma_start(wo_sb[1][:], ex["wo_s"][P:2 * P, :])

                attnT = [pb.tile([P, T], F16, tag=f"attnT{kc}",
                                 name=f"attnT{kc}") for kc in range(TB)]
                for kc in range(1, TB):
                    nc.vector.memset(attnT[kc][:, 0:kc * P], 0.0)
                with tc.tile_pool(name="psB1", bufs=2, space="PSUM") as psB1, \
                     tc.tile_pool(name="psB2", bufs=2, space="PSUM") as psB2, \
                     tc.tile_pool(name="psB3", bufs=2, space="PSUM") as psB3:
                  for h in range(2):
                    for qc in range(TB):
                        cols = (qc + 1) * P
                        prob = pb2.tile([P, T], F32, tag="prob")
                        nsl = (cols + 511) // 512
                        for n in range(nsl):
                            w_ = min(512, cols - n * 512)
                            ps = psB1.tile([P, 512], F32, tag="psSC")
                            nc.tensor.matmul(ps[:, :w_],
                                             qT[h][:, qc * P:(qc + 1) * P],
                                             kT[:, n * 512:n * 512 + w_],
                                             start=True, stop=True)
                            # diagonal block gets the causal mask; the rest
                            # of this slice is fully visible
                            d0 = qc * P - n * 512
                            if 0 <= d0 < w_:
                                if d0 > 0:
                                    nc.vector.tensor_copy(
                                        prob[:, n * 512:n * 512 + d0],
                                        ps[:, :d0])
                                nc.vector.tensor_add(
                                    prob[:, qc * P:qc * P + P],
                                    ps[:, d0:d0 + P], dmask[:])
                            else:
                                nc.vector.tensor_copy(
                                    prob[:, n * 512:n * 512 + w_], ps[:, :w_])
                        mx = pb2.tile([P, 1], F32, tag="mx")
                        nc.vector.reduce_max(mx[:], prob[:, :cols], axis=AX.X)
                        negm = pb2.tile([P, 1], F32, tag="negm")
                        nc.vector.tensor_scalar_mul(negm[:], mx[:], -1.0)
                        ssum = pb2.tile([P, 1], F32, tag="esum")
                        probe_ = pb2.tile([P, T], F32, tag="probe")
                        nc.scalar.activation(probe_[:, :cols], prob[:, :cols],
                                             AF.Exp, bias=negm[:, :1],
                                             accum_out=ssum[:, :1])
                        rec = pb2.tile([P, 1], F32, tag="rec")
                        nc.vector.reciprocal(rec[:], ssum[:])
                        probS = pb2.tile([P, T], F16, tag="probS")
                        nc.vector.tensor_scalar_mul(probS[:, :cols],
                                                    probe_[:, :cols],
                                                    rec[:, :1])
                        for kc in range(qc + 1):
                            tp = psB2.tile([P, P], F16, tag="tpB")
                            nc.tensor.transpose(
                                tp[:], probS[:, kc * P:(kc + 1) * P],
                                identh[:])
                            nc.vector.tensor_copy(
                                attnT[kc][:, qc * P:(qc + 1) * P], tp[:])
                    for n in range(2):
                        sl = slice(n * 512, (n + 1) * 512)
                        kc_hi = 4 * n + 3
                        ps = psB3.tile([P, 512], F32, tag="psAV")
                        for kc in range(kc_hi + 1):
                            nc.tensor.matmul(ps[:], v_tm[kc][:],
                                             attnT[kc][:, sl],
                                             start=(kc == 0),
                                             stop=(kc == kc_hi))
                        nc.vector.tensor_copy(oT[h][:, sl], ps[:])

                # ---- C: o_proj ----
                with tc.tile_pool(name="psC", bufs=8, space="PSUM") as psC:
                    for tb_ in range(TB):
                        pso = [psC.tile([P, 512], F32, tag="psO",
                                        name=f"psO{n}") for n in range(4)]
                        for hp in range(2):
                            for n in range(4):
                                nc.tensor.matmul(
                                    pso[n][:],
                                    oT[hp][:, tb_ * P:(tb_ + 1) * P],
                                    wo_sb[hp][:, n * 512:(n + 1) * 512],
                                    start=(hp == 0), stop=(hp == 1))
                        ob = pb2.tile([P, H], F16, tag="ob", bufs=2)
                        for n in range(4):
                            nc.vector.tensor_copy(
                                ob[:, n * 512:(n + 1) * 512], pso[n][:])
                        nc.sync.dma_start(rs_in[tb_ * P:(tb_ + 1) * P, :],
                                          ob[:])

        nc.gpsimd.collective_compute(
            "ReduceScatter", ALU.add, ins=[rs_in.opt()], outs=[rs_out.opt()],
            replica_groups=[list(range(NCN))])

        # ======== D: residual + norm + local fp32 router + AGs ========
        with tc.tile_pool(name="pd", bufs=1) as pd, \
             tc.tile_pool(name="pd2", bufs=2) as pd2, \
             tc.tile_pool(name="psD", bufs=2, space="PSUM") as psD:
            attn_sl = pd.tile([P, H], F16)
            nc.sync.dma_start(attn_sl[:], rs_out[:])
            res_sb = pd.tile([P, H], F32)
            nc.vector.tensor_add(res_sb[:], hid_sl[:], attn_sl[:])
            nc.sync.dma_start(res_slice[:], res_sb[:])
            dump2 = pd.tile([P, H], F32)
            ssum = pd.tile([P, 1], F32)
            nc.scalar.activation(dump2[:], res_sb[:], AF.Square,
                                 accum_out=ssum[:, :1])
            rms = pd.tile([P, 1], F32)
            nc.scalar.activation(rms[:], ssum[:], AF.Sqrt, bias=eps_t[:, :1],
                                 scale=1.0 / H)
            inv = pd.tile([P, 1], F32)
            nc.vector.reciprocal(inv[:], rms[:])
            x_sl = pd.tile([P, H], F32)
            nc.vector.tensor_scalar_mul(x_sl[:], res_sb[:], inv[:, :1])
            x_sl_h = pd.tile([P, H], F16)
            nc.vector.tensor_copy(x_sl_h[:], x_sl[:])
            nc.sync.dma_start(agx_in[:], x_sl_h[:])

            # exact fp32 router on the un-normalized residual: transposes and
            # logit matmuls run in parallel with the rmsnorm stats, and the
            # 1/rms scale folds into the sigmoid's per-token scale operand
            gw_sb = pd.tile([P, HC * E], F32)
            nc.sync.dma_start(
                gw_sb[:].rearrange("p (hc e) -> p hc e", hc=HC),
                ex["gate_wT"][:].rearrange("(hc p) e -> p hc e", p=P))
            gate_b = pd.tile([P, E], F32)
            nc.sync.dma_start(gate_b[:], ex["gate_b"][:])
            resT = pd.tile([P, H], F32)
            for hc in range(HC):
                tp = psD.tile([P, P], F32, tag="tpD")
                nc.tensor.transpose(tp[:], res_sb[:, hc * P:(hc + 1) * P],
                                    ident[:])
                nc.vector.tensor_copy(resT[:, hc * P:(hc + 1) * P], tp[:])
            lg_ps = psD.tile([P, E], F32, tag="lgps", name="lgps")
            for hc in range(HC):
                nc.tensor.matmul(lg_ps[:], resT[:, hc * P:(hc + 1) * P],
                                 gw_sb[:, hc * E:(hc + 1) * E],
                                 start=(hc == 0), stop=(hc == HC - 1))
            sig = pd2.tile([P, E], F32, tag="sig")
            nc.scalar.activation(sig[:], lg_ps[:], AF.Sigmoid,
                                 scale=inv[:, :1])
            sb_ = pd2.tile([P, E], F32, tag="sb_")
            nc.vector.tensor_add(sb_[:], sig[:], gate_b[:])
            mx = pd2.tile([P, 8], F32, tag="mx8")
            nc.vector.max(out=mx[:], in_=sb_[:])
            s1 = pd2.tile([P, E], F32, tag="s1")
            nc.vector.tensor_tensor(out=s1[:], in0=sb_[:],
                                    in1=mx[:, 0:1].to_broadcast([P, E]),
                                    op=ALU.is_equal)
            s2 = pd2.tile([P, E], F32, tag="s2")
            nc.vector.tensor_tensor(out=s2[:], in0=sb_[:],
                                    in1=mx[:, 1:2].to_broadcast([P, E]),
                                    op=ALU.is_equal)
            nc.vector.tensor_add(s1[:], s1[:], s2[:])
            sel_own = pd2.tile([P, E], F32, tag="sel_own")
            nc.vector.tensor_scalar_min(sel_own[:], s1[:], 1.0)
            wa = pd2.tile([P, E], F32, tag="wa")
            nc.vector.tensor_mul(wa[:], sel_own[:], sig[:])
            nrm = pd2.tile([P, 1], F32, tag="nrm")
            nc.vector.reduce_sum(nrm[:], wa[:], axis=AX.X)
            rec = pd2.tile([P, 1], F32, tag="recw")
            nc.vector.reciprocal(rec[:], nrm[:])
            w_tm = pd2.tile([P, E], F32, tag="wtm")
            nc.vector.tensor_scalar_mul(w_tm[:], wa[:], rec[:, :1])
            nc.sync.dma_start(agw_in[:, 0:E], w_tm[:])
            nc.sync.dma_start(agw_in[:, E:2 * E], sel_own[:])
            nc.sync.dma_start(dbg_w[:], w_tm[:])

        cc_w = nc.gpsimd.collective_compute(
            "AllGather", ALU.bypass, ins=[agw_in.opt()], outs=[w_all.opt()],
            replica_groups=[list(range(NCN))])
        cc_x = nc.gpsimd.collective_compute(
            "AllGather", ALU.bypass, ins=[agx_in.opt()], outs=[x_tm.opt()],
            replica_groups=[list(range(NCN))])
        # The tiny router AllGather must run first so the token-list build
        # overlaps the big x AllGather (CC queue executes in trigger order).
        add_dep_helper(cc_x.ins, cc_w.ins, sync=True,
                       reason="AG_W before AG_x")

        # ======== E: token lists from AllGathered router decisions ========
        # Inverse permutation (slot -> token id) built with matmuls instead of
        # 16 serialized indirect scatters: M[token, slot] = (rank == slot),
        # tok_list[slot] = sum_t M[t, slot] * t, with +BIG for empty slots.
        with tc.tile_pool(name="pe", bufs=1) as pe, \
             tc.tile_pool(name="pe2", bufs=3) as pe2, \
             tc.tile_pool(name="psE", bufs=2, space="PSUM") as psE, \
             tc.tile_pool(name="psE2", bufs=1, space="PSUM") as psE2:
            ut = pe.tile([P, P], F32R)
            nc.sync.dma_start(ut[:], ex["ut_in"][:].bitcast(F32R))
            slb = pe.tile([8, TB * P], F32R)
            nc.sync.dma_start(slb[:], ex["slb_in"][:].bitcast(F32R))
            s_iota = pe.tile([P, CAP], F32)
            nc.sync.dma_start(s_iota[:], ex["slot_iota"][:])
            tokid2 = pe.tile([P, 2 * TB], F16)
            nc.sync.dma_start(tokid2[:], ex["tokid2"][:])
            totals = pe.tile([8, E], F32R)
            pre_sb = [pe.tile([P, E], F32, tag=f"pre{b}", name=f"pre{b}")
                      for b in range(TB)]
            sel_all = [pe.tile([P, E], F32, tag=f"sela{b}", name=f"sela{b}")
                       for b in range(TB)]
            for b in range(TB):
                nc.sync.dma_start(sel_all[b][:],
                                  w_all[b * P:(b + 1) * P, E:2 * E])
                pr_ps = psE.tile([P, E], F32, tag="prps")
                nc.tensor.matmul(pr_ps[:], ut[:],
                                 sel_all[b][:].bitcast(F32R),
                                 start=True, stop=True)
                nc.vector.tensor_copy(pre_sb[b][:], pr_ps[:])
                nc.sync.dma_start(totals[b:b + 1, :],
                                  pre_sb[b][127:128, :].bitcast(F32R))
            tl_ps = [[psE2.tile([P, 2], F32, tag=f"tl{ei}{ch}",
                                name=f"tl{ei}{ch}") for ch in range(2)]
                     for ei in range(2)]
            for b in range(TB):
                ofs_ps = psE.tile([P, E], F32, tag="ofsps", name="ofsps")
                nc.tensor.matmul(ofs_ps[:], slb[:, b * P:(b + 1) * P],
                                 totals[:], start=True, stop=True)
                grank = pe2.tile([P, E], F32, tag="grank")
                nc.vector.tensor_add(grank[:], pre_sb[b][:], ofs_ps[:])
                nc.vector.tensor_scalar_add(grank[:], grank[:], -1.0)
                gm = pe2.tile([P, E], F32, tag="gm")
                nc.vector.tensor_scalar(out=gm[:], in0=grank[:],
                                        scalar1=float(CAP - 1), scalar2=BIG,
                                        op0=ALU.is_gt, op1=ALU.mult)
                nc.vector.tensor_add(grank[:], grank[:], gm[:])
                um = pe2.tile([P, E], F32, tag="um")
                nc.vector.tensor_scalar(out=um[:], in0=sel_all[b][:],
                                        scalar1=-BIG, scalar2=BIG,
                                        op0=ALU.mult, op1=ALU.add)
                nc.vector.tensor_add(grank[:], grank[:], um[:])
                for ei in range(2):
                    ge = pe2.tile([P, E], F32, tag="ge")
                    nc.vector.tensor_mul(ge[:], grank[:],
                                         emask01[:, ei * E:(ei + 1) * E])
                    ridx = pe2.tile([P, 1], F32, tag="ridx")
                    nc.vector.reduce_sum(ridx[:], ge[:], axis=AX.X)
                    mb = pe2.tile([P, CAP], F16, tag="mb")
                    nc.vector.tensor_tensor(
                        out=mb[:], in0=s_iota[:],
                        in1=ridx[:, 0:1].to_broadcast([P, CAP]),
                        op=ALU.is_equal)
                    for ch in range(2):
                        nc.tensor.matmul(tl_ps[ei][ch][:],
                                         mb[:, ch * P:(ch + 1) * P],
                                         tokid2[:, 2 * b:2 * b + 2],
                                         start=(b == 0), stop=(b == TB - 1))
            for ei in range(2):
                for ch in range(2):
                    tl = pe2.tile([P, 2], F32, tag="tlsb")
                    nc.vector.tensor_copy(tl[:], tl_ps[ei][ch][:])
                    pad = pe2.tile([P, 1], F32, tag="pad")
                    nc.vector.tensor_scalar(out=pad[:], in0=tl[:, 1:2],
                                            scalar1=-BIG, scalar2=BIG,
                                            op0=ALU.mult, op1=ALU.add)
                    tok_f = pe2.tile([P, 1], F32, tag="tokf")
                    nc.vector.tensor_add(tok_f[:], tl[:, 0:1], pad[:])
                    nc.vector.tensor_copy(idx_sb2[ei][ch][:], tok_f[:])

        # ======== F: xT + shared expert + experts (fp16) ========
        with tc.tile_pool(name="pxt", bufs=1) as pxt, \
             tc.tile_pool(name="pfs", bufs=1) as pfs, \
             tc.tile_pool(name="pfs2", bufs=2) as pfs2:
            xc = [pxt.tile([P, T], F16, tag=f"xc{hc}", name=f"xc{hc}")
                  for hc in range(HC)]
            with tc.tile_pool(name="pxt2", bufs=3) as pxt2, \
                 tc.tile_pool(name="psX", bufs=2, space="PSUM") as psX:
                for b in range(TB):
                    xb = pxt2.tile([P, H], F16, tag="xb", bufs=2)
                    nc.sync.dma_start(xb[:], x_tm[b * P:(b + 1) * P, :])
                    for hc in range(HC):
                        tp = psX.tile([P, P], F16, tag="tpX")
                        nc.tensor.transpose(tp[:], xb[:, hc * P:(hc + 1) * P],
                                            identh[:])
                        nc.vector.tensor_copy(xc[hc][:, b * P:(b + 1) * P],
                                              tp[:])

            # ---- both experts' setup: gathers, gxT, weights,
            # per-token gate weights — overlaps the shared expert below ----
            gxT2 = [pfs.tile([P, HC * 2 * P], F16, tag=f"gxT{ei}",
                             name=f"gxT{ei}") for ei in range(2)]
            wd_res2 = [[pfs.tile([P, H], F16, tag=f"wd{ei}{ip}",
                                 name=f"wd{ei}{ip}") for ip in range(IP)]
                       for ei in range(2)]
            wg_own2 = [[pfs.tile([P, 1], F32, tag=f"wgo{ei}{k}",
                                 name=f"wgo{ei}{k}") for k in range(2)]
                       for ei in range(2)]
            psS_cm = tc.tile_pool(name="psS", bufs=2, space="PSUM")
            psS = psS_cm.__enter__()
            for ei in range(2):
                for k in range(2):
                    gx = pfs2.tile([P, H], F16, tag="gx")
                    nc.vector.memset(gx[:], 0.0)
                    nc.gpsimd.indirect_dma_start(
                        out=gx[:], out_offset=None,
                        in_=x_tm[:],
                        in_offset=bass.IndirectOffsetOnAxis(
                            ap=idx_sb2[ei][k][:, :1], axis=0),
                        bounds_check=T - 1, oob_is_err=False)
                    for hc in range(HC):
                        tp = psS.tile([P, P], F16, tag="tpS")
                        nc.tensor.transpose(tp[:], gx[:, hc * P:(hc + 1) * P],
                                            identh[:])
                        nc.vector.tensor_copy(
                            gxT2[ei][:, hc * 2 * P + k * P:
                                  hc * 2 * P + (k + 1) * P], tp[:])
                    wrow = pfs2.tile([P, 2 * E], F32, tag="wrow")
                    nc.vector.memset(wrow[:], 0.0)
                    nc.gpsimd.indirect_dma_start(
                        out=wrow[:], out_offset=None, in_=w_all[:],
                        in_offset=bass.IndirectOffsetOnAxis(
                            ap=idx_sb2[ei][k][:, :1], axis=0),
                        bounds_check=T - 1, oob_is_err=False)
                    we_ = pfs2.tile([P, E], F32, tag="we_")
                    nc.vector.tensor_mul(we_[:], wrow[:, 0:E],
                                         emask01[:, ei * E:(ei + 1) * E])
                    nc.vector.reduce_sum(wg_own2[ei][k][:], we_[:], axis=AX.X)

            # ---- shared expert ----
            with tc.tile_pool(name="pg", bufs=1) as pg, \
                 tc.tile_pool(name="pg2", bufs=3) as pg2:
                g_act = [pg.tile([P, T], F16, tag=f"gact{sp}", name=f"gact{sp}")
                         for sp in range(SP)]
                hs = [pg.tile([P, T], F16, tag=f"hs{sp}", name=f"hs{sp}")
                      for sp in range(SP)]
                with tc.tile_pool(name="psG1", bufs=1, space="PSUM") as psG1:
                    g_ps = [psG1.tile([P, T], F32, tag=f"gps{sp}",
                                      name=f"gps{sp}") for sp in range(SP)]
                    for hc in range(HC):
                        for sp in range(SP):
                            c0 = hc * SP * P + sp * P
                            for n in range(2):
                                sl = slice(n * 512, (n + 1) * 512)
                                nc.tensor.matmul(g_ps[sp][:, sl],
                                                 wsg_sb[:, c0:c0 + P],
                                                 xc[hc][:, sl],
                                                 start=(hc == 0),
                                                 stop=(hc == HC - 1))
                    for sp in range(SP):
                        nc.scalar.activation(g_act[sp][:], g_ps[sp][:],
                                             AF.Silu)
                with tc.tile_pool(name="psG2", bufs=1, space="PSUM") as psG2:
                    u_ps = [psG2.tile([P, T], F32, tag=f"ups{sp}",
                                      name=f"ups{sp}") for sp in range(SP)]
                    for hc in range(HC):
                        for sp in range(SP):
                            c0 = hc * SP * P + sp * P
                            for n in range(2):
                                sl = slice(n * 512, (n + 1) * 512)
                                nc.tensor.matmul(u_ps[sp][:, sl],
                                                 wsu_sb[:, c0:c0 + P],
                                                 xc[hc][:, sl],
                                                 start=(hc == 0),
                                                 stop=(hc == HC - 1))
                    for sp in range(SP):
                        nc.vector.tensor_mul(hs[sp][:], g_act[sp][:],
                                             u_ps[sp][:])
                with tc.tile_pool(name="psG3", bufs=6, space="PSUM") as psG3:
                    for tb_ in range(TB):
                        psd = [psG3.tile([P, 512], F32, tag="psGd",
                                         name=f"psGd{n}") for n in range(4)]
                        for sp in range(SP):
                            for n in range(4):
                                nc.tensor.matmul(
                                    psd[n][:],
                                    hs[sp][:, tb_ * P:(tb_ + 1) * P],
                                    wsd_sb[sp][:, n * 512:(n + 1) * 512],
                                    start=(sp == 0), stop=(sp == SP - 1))
                        sbd = pg2.tile([P, H], F16, tag="sbGd", bufs=2)
                        for n in range(4):
                            nc.vector.tensor_copy(
                                sbd[:, n * 512:(n + 1) * 512], psd[n][:])
                        nc.sync.dma_start(rs2_in[tb_ * P:(tb_ + 1) * P, :],
                                          sbd[:])

            psS_cm.__exit__(None, None, None)

            # expert down-proj weights: emitted late so these 8 MB of DMAs
            # sit behind the x-block/gather traffic in queue priority, but
            # they still have ~100us of slack before first use
            for ei in range(2):
                for ip in range(IP):
                    nc.sync.dma_start(wd_res2[ei][ip][:],
                                      ex["we_d"][ei, ip * P:(ip + 1) * P, :])

            # ---- experts (setup already done above) ----
            for ei in range(2):
                with tc.tile_pool(name=f"pf{ei}", bufs=1) as pf, \
                     tc.tile_pool(name=f"pf2{ei}", bufs=2) as pf2:
                    idx_sb = idx_sb2[ei]
                    gxT = gxT2[ei]
                    wd_res = wd_res2[ei]

                    # merged gate+up pass (8 PSUM banks)
                    g_tm = [pf.tile([P, I], F16, tag=f"gtm{k}", name=f"gtm{k}")
                            for k in range(2)]
                    h_tm = [pf.tile([P, I], F16, tag=f"htm{k}", name=f"htm{k}")
                            for k in range(2)]
                    with tc.tile_pool(name=f"psF2{ei}", bufs=1,
                                      space="PSUM") as psF2:
                        gu_ps = [[psF2.tile([P, 512], F32, tag=f"gups{k}{j}",
                                            name=f"gups{k}{j}")
                                  for j in range(4)] for k in range(2)]
                        for hc in range(HC):
                            wg_c = pf2.tile([P, I], F16, tag="wgF", bufs=3)
                            nc.sync.dma_start(
                                wg_c[:], ex["we_g"][ei, hc * P:(hc + 1) * P, :])
                            wu_c = pf2.tile([P, I], F16, tag="wuF", bufs=3)
                            nc.sync.dma_start(
                                wu_c[:], ex["we_u"][ei, hc * P:(hc + 1) * P, :])
                            for k in range(2):
                                s_ = gxT[:, hc * 2 * P + k * P:
                                         hc * 2 * P + (k + 1) * P]
                                for n in range(2):
                                    nc.tensor.matmul(
                                        gu_ps[k][n][:], s_,
                                        wg_c[:, n * 512:(n + 1) * 512],
                                        start=(hc == 0), stop=(hc == HC - 1))
                                for n in range(2):
                                    nc.tensor.matmul(
                                        gu_ps[k][2 + n][:], s_,
                                        wu_c[:, n * 512:(n + 1) * 512],
                                        start=(hc == 0), stop=(hc == HC - 1))
                        for k in range(2):
                            for n in range(2):
                                sl = slice(n * 512, (n + 1) * 512)
                                nc.scalar.activation(g_tm[k][:, sl],
                                                     gu_ps[k][n][:], AF.Silu)
                                nc.vector.tensor_mul(h_tm[k][:, sl],
                                                     g_tm[k][:, sl],
                                                     gu_ps[k][2 + n][:])
                    h_sb = [pf.tile([P, 2 * P], F16, tag=f"hsb{ip}",
                                    name=f"hsb{ip}") for ip in range(IP)]
                    with tc.tile_pool(name=f"psF4{ei}", bufs=2,
                                      space="PSUM") as psF4:
                        for k in range(2):
                            for ip in range(IP):
                                tp = psF4.tile([P, P], F16, tag="tpF2")
                                nc.tensor.transpose(
                                    tp[:], h_tm[k][:, ip * P:(ip + 1) * P],
                                    identh[:])
                                nc.vector.tensor_copy(
                                    h_sb[ip][:, k * P:(k + 1) * P], tp[:])
                    with tc.tile_pool(name=f"psF5{ei}", bufs=8,
                                      space="PSUM") as psF5:
                        for k in range(2):
                            psd = [psF5.tile([P, 512], F32, tag="psFd",
                                             name=f"psFd{n}")
                                   for n in range(4)]
                            for ip in range(IP):
                                for n in range(4):
                                    nc.tensor.matmul(
                                        psd[n][:],
                                        h_sb[ip][:, k * P:(k + 1) * P],
                                        wd_res[ip][:, n * 512:(n + 1) * 512],
                                        start=(ip == 0), stop=(ip == IP - 1))
                            out_sb = pf.tile([P, H], F16, tag=f"outsb{k}")
                            for n in range(4):
                                nc.vector.tensor_scalar_mul(
                                    out_sb[:, n * 512:(n + 1) * 512],
                                    psd[n][:], wg_own2[ei][k][:, :1])
                            nc.gpsimd.indirect_dma_start(
                                out=rs2_in[:],
                                out_offset=bass.IndirectOffsetOnAxis(
                                    ap=idx_sb[k][:, :1], axis=0),
                                in_=out_sb[:], in_offset=None,
                                bounds_check=T - 1, oob_is_err=False,
                                compute_op=ALU.add)

        nc.gpsimd.collective_compute(
            "ReduceScatter", ALU.add, ins=[rs2_in.opt()], outs=[rs2_out.opt()],
            replica_groups=[list(range(NCN))])
        with tc.tile_pool(name="pz", bufs=2) as pz:
            fin16 = pz.tile([P, H], F16)
            nc.sync.dma_start(fin16[:], rs2_out[:])
            fin = pz.tile([P, H], F32)
            nc.vector.tensor_copy(fin[:], fin16[:])
            nc.sync.dma_start(out_slice[:], fin[:])


_CACHE = {}


def _build():
    key = "nc"
    if key in _CACHE:
        return _CACHE[key]
    nc = bacc.Bacc("TRN2", target_bir_lowering=False, debug=False,
                   num_devices=NCN)
    with tile.TileContext(nc) as tc:
        _emit(nc, tc)
    nc.compile()
    _CACHE[key] = nc
    return nc


def _host_prep(inputs):
    f16 = np.float16
    pos = np.asarray(inputs["positions"]).astype(np.float64)
    hid = np.asarray(inputs["hidden_states"], np.float32)
    w_in = np.asarray(inputs["w_in_ln"], np.float32)
    w_post = np.asarray(inputs["w_post_ln"], np.float32)
    wq = (np.asarray(inputs["wq"], np.float32) * w_in[:, None]).astype(f16)
    wk = (np.asarray(inputs["wk"], np.float32) * w_in[:, None]).astype(f16)
    wv = (np.asarray(inputs["wv"], np.float32) * w_in[:, None]).astype(f16)
    wo = np.asarray(inputs["wo"], np.float32).astype(f16)
    gate_w = np.asarray(inputs["gate_w"], np.float32) * w_post[None, :]
    gate_b = np.asarray(inputs["gate_bias"], np.float32).reshape(1, E)
    we_g = (np.asarray(inputs["we_gate"], np.float32)
            * w_post[None, :, None]).astype(f16)
    we_u = (np.asarray(inputs["we_up"], np.float32)
            * w_post[None, :, None]).astype(f16)
    we_d = np.asarray(inputs["we_down"], np.float32).astype(f16)
    ws_g = (np.asarray(inputs["ws_gate"], np.float32)
            * w_post[:, None]).astype(f16)
    ws_u = (np.asarray(inputs["ws_up"], np.float32)
            * w_post[:, None]).astype(f16)
    ws_d = np.asarray(inputs["ws_down"], np.float32).astype(f16)

    inv_freq = 1.0 / (THETA ** (np.arange(0, D, 2, dtype=np.float64) / D))
    f = pos[None, :] * inv_freq[:, None]
    cos2, sin2 = np.cos(f), np.sin(f)
    cosT = np.repeat(cos2, 2, axis=0).astype(np.float32)
    sinT = np.empty((D, T), np.float32)
    sinT[0::2] = -sin2
    sinT[1::2] = sin2
    s = 1.0 / np.sqrt(D)
    cosq, sinq = (cosT * s).astype(np.float32), (sinT * s).astype(np.float32)

    import ml_dtypes
    bf = ml_dtypes.bfloat16
    ii = np.arange(P)
    diag_mask = np.where(ii[:, None] >= ii[None, :], 0.0, NEG).astype(bf)

    identr_in = np.eye(P, dtype=np.float32)
    identh_in = np.eye(P, dtype=f16)
    ut_in = np.triu(np.ones((P, P), np.float32))
    slb_in = np.zeros((8, TB * P), np.float32)
    for b in range(TB):
        slb_in[:b, b * P:(b + 1) * P] = 1.0
    perm = np.zeros((P, P), np.float32)
    for i in range(0, P, 2):
        perm[i, i + 1] = 1.0
        perm[i + 1, i] = 1.0
    slot_iota = np.broadcast_to(np.arange(CAP, dtype=np.float32),
                                (P, CAP)).copy()
    tokid2 = np.zeros((P, 2 * TB), f16)
    for b in range(TB):
        tokid2[:, 2 * b] = (b * P + np.arange(P)).astype(f16)
        tokid2[:, 2 * b + 1] = 1.0

    ISC = IS // NCN
    maps = []
    for c in range(NCN):
        g = c // 2
        emask01 = np.zeros((P, 2 * E), np.float32)
        emask01[:, 2 * c] = 1.0          # ei = 0 -> expert 2c
        emask01[:, E + 2 * c + 1] = 1.0  # ei = 1 -> expert 2c+1
        maps.append({
            "hid": hid,
            "hid_slice": np.ascontiguousarray(hid[c * P:(c + 1) * P]),
            "wq_s": np.ascontiguousarray(wq[:, 2 * c * D:(2 * c + 2) * D]),
            "wk_s": np.ascontiguousarray(wk[:, g * D:(g + 1) * D]),
            "wv_s": np.ascontiguousarray(wv[:, g * D:(g + 1) * D]),
            "wo_s": np.ascontiguousarray(wo[2 * c * D:(2 * c + 2) * D, :]),
            "cosq": cosq, "sinq": sinq, "cosk": cosT, "sink": sinT,
            "perm": perm, "diag_mask": diag_mask,
            "identr_in": identr_in, "identh_in": identh_in,
            "ut_in": ut_in, "slb_in": slb_in,
            "slot_iota": slot_iota, "tokid2": tokid2,
            "gate_wT": np.ascontiguousarray(gate_w.T),
            "gate_b": np.broadcast_to(gate_b, (P, E)).copy(),
            "emask01": emask01,
            "ws_g": np.ascontiguousarray(ws_g[:, c * ISC:(c + 1) * ISC]),
            "ws_u": np.ascontiguousarray(ws_u[:, c * ISC:(c + 1) * ISC]),
            "ws_d": np.ascontiguousarray(ws_d[c * ISC:(c + 1) * ISC, :]),
            "we_g": np.ascontiguousarray(we_g[2 * c:2 * c + 2]),
            "we_u": np.ascontiguousarray(we_u[2 * c:2 * c + 2]),
            "we_d": np.ascontiguousarray(we_d[2 * c:2 * c + 2]),
        })
    return maps


def kernel(trace=False, **inputs):
    nc = _build()
    maps = _host_prep(inputs)
    res = bass_utils.run_bass_kernel_spmd(
        nc, maps, core_ids=list(range(NCN)), trace=trace)
    out = np.concatenate([res.results[c]["out_slice"] for c in range(NCN)], 0)
    resid = np.concatenate([res.results[c]["res_slice"] for c in range(NCN)], 0)
    kernel.last_results = res
    return out, resid


# revision 32
# speedup vs baseline: 1.6134x; 1.0053x over previous
"""Ernie4 decoder layer (RMSNorm + GQA attention + shared expert + 16-expert
top-2 MoE) on 8 Trainium2 NeuronCores.

v2 — fp16 data path everywhere except the router (which must reproduce the
reference top-2 selection exactly; margins are ~3e-5 so it stays fp32 and is
computed locally per core before the AllGather):
  - Attention: head-parallel (2 q-heads + 1 kv-head per core), fp16 QKV /
    scores / probs / o_proj with causal-block skipping; fp16 ReduceScatter.
  - Router: fp32 logits on each core's own 128 tokens; W+sel AllGathered in a
    tiny fp32 collective that precedes the fp16 x AllGather so the token-list
    build overlaps it.
  - Shared expert: intermediate-sharded (IS/8 per core) fp16, output seeds
    the MoE combine buffer.
  - MoE: expert-parallel (2 experts per core), token lists via
    triangular-matmul prefix ranks, indirect-DMA gather/scatter-add in fp16,
    fp16 ReduceScatter for the combine.
"""
import sys
sys.path.insert(0, "/opt/trn_rl_repo")

import numpy as np

import concourse.bass as bass
import concourse.bacc as bacc
import concourse.tile as tile
import concourse.mybir as mybir
from concourse import bass_utils
from concourse.masks import make_identity
from concourse.tile import add_dep_helper

dt = mybir.dt
F32 = dt.float32
F32R = dt.float32r
F16 = dt.float16
I32 = dt.int32
BF16 = dt.bfloat16
AF = mybir.ActivationFunctionType
ALU = mybir.AluOpType
AX = mybir.AxisListType

T, H, NH, NKV, D = 1024, 2048, 16, 4, 128
E, I, IS = 16, 1024, 2048
EPS = 1e-6
THETA = 10000.0
NCN = 8
P = 128
TB = T // P            # 8 token blocks
HC = H // P            # 16 hidden chunks
IP = I // P            # 8 expert-intermediate chunks
SP = IS // NCN // P    # 2 shared-intermediate chunks per core
CAP = 256              # per-expert token capacity
BIG = 1.0e6            # OOB sentinel
NEG = -1e9


def _emit(nc, tc):
    ex = {}
    for name, shape, d in [
        ("hid", [T, H], F32), ("hid_slice", [P, H], F32),
        ("wq_s", [H, 2 * D], F16), ("wk_s", [H, D], F16), ("wv_s", [H, D], F16),
        ("wo_s", [2 * D, H], F16),
        ("cosq", [D, T], F32), ("sinq", [D, T], F32),
        ("cosk", [D, T], F32), ("sink", [D, T], F32),
        ("perm", [P, P], F32),
        ("diag_mask", [P, P], BF16),
        ("gate_wT", [H, E], F32), ("gate_b", [P, E], F32),
        ("emask01", [P, 2 * E], F32),
        ("ws_g", [H, SP * P], F16), ("ws_u", [H, SP * P], F16),
        ("ws_d", [SP * P, H], F16),
        ("we_g", [2, H, I], F16), ("we_u", [2, H, I], F16),
        ("we_d", [2, I, H], F16),
        ("identr_in", [P, P], F32), ("identh_in", [P, P], F16),
        ("ut_in", [P, P], F32), ("ut_h", [P, P], F16),
        ("slb_in", [8, TB * P], F32),
        ("slot_iota", [P, CAP], F32), ("tokid2", [P, 2 * TB], F16),
    ]:
        ex[name] = nc.dram_tensor(name, shape, d, kind="ExternalInput").ap()
    out_slice = nc.dram_tensor("out_slice", [P, H], F32, kind="ExternalOutput").ap()
    res_slice = nc.dram_tensor("res_slice", [P, H], F32, kind="ExternalOutput").ap()
    dbg_w = nc.dram_tensor("dbg_w", [P, E], F32, kind="ExternalOutput").ap()

    with tc.tile_pool(name="persist", bufs=1) as pp, \
         tc.tile_pool(name="dram", bufs=1, space="DRAM") as dram:
        rs_in = dram.tile([T, H], F16)
        rs_out = dram.tile([P, H], F16)
        agx_in = dram.tile([P, H + 2 * E], F16)
        xw_all = dram.tile([T, H + 2 * E], F16, addr_space="Shared")
        warm_in = dram.tile([8, 8], F16)
        warm_out = dram.tile([64, 8], F16, addr_space="Shared")
        rs2_in = dram.tile([T, H], F16)
        rs2_out = dram.tile([P, H], F16)

        ident = pp.tile([P, P], F32)
        make_identity(nc, ident[:])
        identr = pp.tile([P, P], F32R)
        nc.sync.dma_start(identr[:], ex["identr_in"][:].bitcast(F32R))
        identh = pp.tile([P, P], F16)
        nc.sync.dma_start(identh[:], ex["identh_in"][:])
        hid_sl = pp.tile([P, H], F32)
        nc.sync.dma_start(hid_sl[:], ex["hid_slice"][:])
        eps_t = pp.tile([P, 1], F32)
        nc.vector.memset(eps_t[:], EPS)
        emask01 = pp.tile([P, 2 * E], F32)
        nc.sync.dma_start(emask01[:], ex["emask01"][:])
        wz = pp.tile([8, 8], F16)
        nc.vector.memset(wz[:], 0.0)
        nc.sync.dma_start(warm_in[:], wz[:])
        nc.gpsimd.collective_compute(
            "AllGather", ALU.bypass, ins=[warm_in.opt()],
            outs=[warm_out.opt()], replica_groups=[list(range(NCN))])
        # per-expert token lists live in SBUF end-to-end (built by the
        # matmul-based inverse permutation in phase E, consumed in F)
        idx_sb2 = [[pp.tile([P, 1], I32, tag=f"idx{ei}{k}",
                            name=f"idx{ei}{k}") for k in range(2)]
                   for ei in range(2)]
        # shared-expert weights are pure inputs: load them from t=0 so the
        # post-AllGather phase never waits on weight DMAs
        wsg_sb = pp.tile([P, HC * SP * P], F16)
        wsu_sb = pp.tile([P, HC * SP * P], F16)
        for t_, s_ in [(wsg_sb, "ws_g"), (wsu_sb, "ws_u")]:
            nc.sync.dma_start(
                t_[:].rearrange("p (hc m) -> p hc m", hc=HC),
                ex[s_][:].rearrange("(hc p) m -> p hc m", p=P))
        wsd_sb = [pp.tile([P, H], F16, tag=f"wsd{sp}", name=f"wsd{sp}")
                  for sp in range(SP)]
        for sp in range(SP):
            nc.sync.dma_start(wsd_sb[sp][:],
                              ex["ws_d"][sp * P:(sp + 1) * P, :])

        # ======== Phases A-C: attention (fp16) ========
        with tc.tile_pool(name="pab", bufs=1) as pab:
            qT = [pab.tile([P, T], F16, tag=f"qT{j}", name=f"qT{j}")
                  for j in range(2)]
            kT = pab.tile([P, T], F16)
            vT = pab.tile([P, T], F16)
            v_tm = [pab.tile([P, D], F16, tag=f"vtm{b}", name=f"vtm{b}")
                    for b in range(TB)]
            oT = [pab.tile([P, T], F16, tag=f"oT{j}", name=f"oT{j}")
                  for j in range(2)]

            # ---- A: norm + transpose + QKV + rope ----
            with tc.tile_pool(name="pa", bufs=1) as pa, \
                 tc.tile_pool(name="pa2", bufs=3) as pa2:
                cosq = pa.tile([D, T], F32)
                sinq = pa.tile([D, T], F32)
                cosk = pa.tile([D, T], F32)
                sink = pa.tile([D, T], F32)
                for t_, s_ in [(cosq, "cosq"), (sinq, "sinq"),
                               (cosk, "cosk"), (sink, "sink")]:
                    nc.sync.dma_start(t_[:], ex[s_][:])
                permr = pa.tile([P, P], F32R)
                nc.sync.dma_start(permr[:], ex["perm"][:].bitcast(F32R))
                wq_sb = pa.tile([P, HC * 2 * D], F16)
                wk_sb = pa.tile([P, HC * D], F16)
                wv_sb = pa.tile([P, HC * D], F16)
                for t_, s_, m in [(wq_sb, "wq_s", 2 * D), (wk_sb, "wk_s", D),
                                  (wv_sb, "wv_s", D)]:
                    nc.sync.dma_start(
                        t_[:].rearrange("p (hc m) -> p hc m", hc=HC),
                        ex[s_][:].rearrange("(hc p) m -> p hc m", p=P))

                dump = pa.tile([P, H], F32)
                qraw = [pa.tile([P, T], F32R, tag=f"qraw{j}", name=f"qraw{j}")
                        for j in range(2)]
                kraw = pa.tile([P, T], F32R)
                with tc.tile_pool(name="psA1", bufs=2, space="PSUM") as psA1, \
                     tc.tile_pool(name="psA2", bufs=2, space="PSUM") as psA2:
                    for n in range(2):
                        x0T = [pa.tile([P, 512], F16, tag=f"x0T{hc}",
                                       name=f"x0T{hc}_{n}") for hc in range(HC)]
                        for bb in range(TB // 2):
                            b = n * (TB // 2) + bb
                            hidb = pa2.tile([P, H], F32, tag="hidb", bufs=2)
                            nc.sync.dma_start(hidb[:],
                                              ex["hid"][b * P:(b + 1) * P, :])
                            ssum = pa2.tile([P, 1], F32, tag="ssum")
                            nc.scalar.activation(dump[:], hidb[:], AF.Square,
                                                 accum_out=ssum[:, :1])
                            rms = pa2.tile([P, 1], F32, tag="rms")
                            nc.scalar.activation(rms[:], ssum[:],
                                                 AF.Sqrt, bias=eps_t[:, :1],
                                                 scale=1.0 / H)
                            inv = pa2.tile([P, 1], F32, tag="inv")
                            nc.vector.reciprocal(inv[:], rms[:])
                            x0b = pa2.tile([P, H], F16, tag="x0b", bufs=2)
                            nc.vector.tensor_scalar_mul(x0b[:], hidb[:],
                                                        inv[:, :1])
                            for hc in range(HC):
                                tp = psA1.tile([P, P], F16, tag="tpA")
                                nc.tensor.transpose(
                                    tp[:], x0b[:, hc * P:(hc + 1) * P],
                                    identh[:])
                                nc.vector.tensor_copy(
                                    x0T[hc][:, bb * P:(bb + 1) * P], tp[:])

                        def proj(w_sb, m, c0, dst, n=n, x0T=x0T, fp16=False):
                            ps = psA2.tile([P, 512], F32, tag="psQKV",
                                           name="psQKV")
                            for hc in range(HC):
                                nc.tensor.matmul(
                                    ps[:],
                                    w_sb[:, hc * m + c0:hc * m + c0 + P],
                                    x0T[hc][:],
                                    start=(hc == 0), stop=(hc == HC - 1))
                            nc.vector.tensor_copy(
                                dst[:, n * 512:(n + 1) * 512], ps[:])
                        proj(wq_sb, 2 * D, 0, qraw[0])
                        proj(wq_sb, 2 * D, D, qraw[1])
                        proj(wk_sb, D, 0, kraw)
                        proj(wv_sb, D, 0, vT, fp16=True)

                with tc.tile_pool(name="psA3", bufs=2, space="PSUM") as psA3:
                    for src, dst, c_, s_ in [(qraw[0], qT[0], cosq, sinq),
                                             (qraw[1], qT[1], cosq, sinq),
                                             (kraw, kT, cosk, sink)]:
                        for n in range(2):
                            sl = slice(n * 512, (n + 1) * 512)
                            sw = psA3.tile([P, 512], F32, tag="psSW")
                            nc.tensor.matmul(sw[:], permr[:], src[:, sl],
                                             start=True, stop=True)
                            t1 = pa2.tile([P, 512], F32, tag="ropeT1")
                            nc.vector.tensor_mul(t1[:], src[:, sl], c_[:, sl])
                            t2 = pa2.tile([P, 512], F32, tag="ropeT2")
                            nc.vector.tensor_mul(t2[:], sw[:], s_[:, sl])
                            nc.vector.tensor_add(dst[:, sl], t1[:], t2[:])
                    for b in range(TB):
                        tp = psA3.tile([P, P], F16, tag="tpV")
                        nc.tensor.transpose(tp[:], vT[:, b * P:(b + 1) * P],
                                            identh[:])
                        nc.vector.tensor_copy(v_tm[b][:], tp[:])

            # ---- B: attention (causal-block skipped) ----
            with tc.tile_pool(name="pb", bufs=1) as pb, \
                 tc.tile_pool(name="pb2", bufs=3) as pb2:
                dmask = pb.tile([P, P], BF16)
                nc.sync.dma_start(dmask[:], ex["diag_mask"][:])
                wo_sb = [pb.tile([P, H], F16, tag=f"wo{j}", name=f"wo{j}")
                         for j in range(2)]
                nc.sync.dma_start(wo_sb[0][:], ex["wo_s"][0:P, :])
                nc.sync.dma_start(wo_sb[1][:], ex["wo_s"][P:2 * P, :])

                attnT = [pb.tile([P, T], F16, tag=f"attnT{kc}",
                                 name=f"attnT{kc}") for kc in range(TB)]
                for kc in range(1, TB):
                    nc.vector.memset(attnT[kc][:, 0:kc * P], 0.0)
                with tc.tile_pool(name="psB1", bufs=2, space="PSUM") as psB1, \
                     tc.tile_pool(name="psB2", bufs=2, space="PSUM") as psB2, \
                     tc.tile_pool(name="psB3", bufs=2, space="PSUM") as psB3:
                  for h in range(2):
                    for qc in range(TB):
                        cols = (qc + 1) * P
                        prob = pb2.tile([P, T], F32, tag="prob")
                        nsl = (cols + 511) // 512
                        for n in range(nsl):
                            w_ = min(512, cols - n * 512)
                            ps = psB1.tile([P, 512], F32, tag="psSC")
                            nc.tensor.matmul(ps[:, :w_],
                                             qT[h][:, qc * P:(qc + 1) * P],
                                             kT[:, n * 512:n * 512 + w_],
                                             start=True, stop=True)
                            # diagonal block gets the causal mask; the rest
                            # of this slice is fully visible
                            d0 = qc * P - n * 512
                            if 0 <= d0 < w_:
                                if d0 > 0:
                                    nc.vector.tensor_copy(
                                        prob[:, n * 512:n * 512 + d0],
                                        ps[:, :d0])
                                nc.vector.tensor_add(
                                    prob[:, qc * P:qc * P + P],
                                    ps[:, d0:d0 + P], dmask[:])
                            else:
                                nc.vector.tensor_copy(
                                    prob[:, n * 512:n * 512 + w_], ps[:, :w_])
                        mx = pb2.tile([P, 1], F32, tag="mx")
                        nc.vector.reduce_max(mx[:], prob[:, :cols], axis=AX.X)
                        negm = pb2.tile([P, 1], F32, tag="negm")
                        nc.vector.tensor_scalar_mul(negm[:], mx[:], -1.0)
                        ssum = pb2.tile([P, 1], F32, tag="esum")
                        probe_ = pb2.tile([P, T], F32, tag="probe")
                        nc.scalar.activation(probe_[:, :cols], prob[:, :cols],
                                             AF.Exp, bias=negm[:, :1],
                                             accum_out=ssum[:, :1])
                        rec = pb2.tile([P, 1], F32, tag="rec")
                        nc.vector.reciprocal(rec[:], ssum[:])
                        probS = pb2.tile([P, T], F16, tag="probS")
                        nc.vector.tensor_scalar_mul(probS[:, :cols],
                                                    probe_[:, :cols],
                                                    rec[:, :1])
                        for kc in range(qc + 1):
                            tp = psB2.tile([P, P], F16, tag="tpB")
                            nc.tensor.transpose(
                                tp[:], probS[:, kc * P:(kc + 1) * P],
                                identh[:])
                            nc.vector.tensor_copy(
                                attnT[kc][:, qc * P:(qc + 1) * P], tp[:])
                    for n in range(2):
                        sl = slice(n * 512, (n + 1) * 512)
                        kc_hi = 4 * n + 3
                        ps = psB3.tile([P, 512], F32, tag="psAV")
                        for kc in range(kc_hi + 1):
                            nc.tensor.matmul(ps[:], v_tm[kc][:],
                                             attnT[kc][:, sl],
                                             start=(kc == 0),
                                             stop=(kc == kc_hi))
                        nc.vector.tensor_copy(oT[h][:, sl], ps[:])

                # ---- C: o_proj ----
                with tc.tile_pool(name="psC", bufs=8, space="PSUM") as psC:
                    for tb_ in range(TB):
                        pso = [psC.tile([P, 512], F32, tag="psO",
                                        name=f"psO{n}") for n in range(4)]
                        for hp in range(2):
                            for n in range(4):
                                nc.tensor.matmul(
                                    pso[n][:],
                                    oT[hp][:, tb_ * P:(tb_ + 1) * P],
                                    wo_sb[hp][:, n * 512:(n + 1) * 512],
                                    start=(hp == 0), stop=(hp == 1))
                        ob = pb2.tile([P, H], F16, tag="ob", bufs=2)
                        for n in range(4):
                            nc.vector.tensor_copy(
                                ob[:, n * 512:(n + 1) * 512], pso[n][:])
                        nc.sync.dma_start(rs_in[tb_ * P:(tb_ + 1) * P, :],
                                          ob[:])

        nc.gpsimd.collective_compute(
            "ReduceScatter", ALU.add, ins=[rs_in.opt()], outs=[rs_out.opt()],
            replica_groups=[list(range(NCN))])

        # ======== D: residual + norm + local fp32 router + AGs ========
        with tc.tile_pool(name="pd", bufs=1) as pd, \
             tc.tile_pool(name="pd2", bufs=2) as pd2, \
             tc.tile_pool(name="psD", bufs=2, space="PSUM") as psD:
            attn_sl = pd.tile([P, H], F16)
            nc.sync.dma_start(attn_sl[:], rs_out[:])
            res_sb = pd.tile([P, H], F32)
            nc.vector.tensor_add(res_sb[:], hid_sl[:], attn_sl[:])
            nc.sync.dma_start(res_slice[:], res_sb[:])
            dump2 = pd.tile([P, H], F32)
            ssum = pd.tile([P, 1], F32)
            nc.scalar.activation(dump2[:], res_sb[:], AF.Square,
                                 accum_out=ssum[:, :1])
            rms = pd.tile([P, 1], F32)
            nc.scalar.activation(rms[:], ssum[:], AF.Sqrt, bias=eps_t[:, :1],
                                 scale=1.0 / H)
            inv = pd.tile([P, 1], F32)
            nc.vector.reciprocal(inv[:], rms[:])
            x_sl = pd.tile([P, H], F32)
            nc.vector.tensor_scalar_mul(x_sl[:], res_sb[:], inv[:, :1])
            x_sl_h = pd.tile([P, H], F16)
            nc.vector.tensor_copy(x_sl_h[:], x_sl[:])
            nc.sync.dma_start(agx_in[:, 0:H], x_sl_h[:])

            # exact fp32 router on the un-normalized residual: transposes and
            # logit matmuls run in parallel with the rmsnorm stats, and the
            # 1/rms scale folds into the sigmoid's per-token scale operand
            gw_sb = pd.tile([P, HC * E], F32)
            nc.sync.dma_start(
                gw_sb[:].rearrange("p (hc e) -> p hc e", hc=HC),
                ex["gate_wT"][:].rearrange("(hc p) e -> p hc e", p=P))
            gate_b = pd.tile([P, E], F32)
            nc.sync.dma_start(gate_b[:], ex["gate_b"][:])
            resT = pd.tile([P, H], F32)
            for hc in range(HC):
                tp = psD.tile([P, P], F32, tag="tpD")
                nc.tensor.transpose(tp[:], res_sb[:, hc * P:(hc + 1) * P],
                                    ident[:])
                nc.vector.tensor_copy(resT[:, hc * P:(hc + 1) * P], tp[:])
            lg_ps = psD.tile([P, E], F32, tag="lgps", name="lgps")
            for hc in range(HC):
                nc.tensor.matmul(lg_ps[:], resT[:, hc * P:(hc + 1) * P],
                                 gw_sb[:, hc * E:(hc + 1) * E],
                                 start=(hc == 0), stop=(hc == HC - 1))
            sig = pd2.tile([P, E], F32, tag="sig")
            nc.scalar.activation(sig[:], lg_ps[:], AF.Sigmoid,
                                 scale=inv[:, :1])
            sb_ = pd2.tile([P, E], F32, tag="sb_")
            nc.vector.tensor_add(sb_[:], sig[:], gate_b[:])
            mx = pd2.tile([P, 8], F32, tag="mx8")
            nc.vector.max(out=mx[:], in_=sb_[:])
            s1 = pd2.tile([P, E], F32, tag="s1")
            nc.vector.tensor_tensor(out=s1[:], in0=sb_[:],
                                    in1=mx[:, 0:1].to_broadcast([P, E]),
                                    op=ALU.is_equal)
            s2 = pd2.tile([P, E], F32, tag="s2")
            nc.vector.tensor_tensor(out=s2[:], in0=sb_[:],
                                    in1=mx[:, 1:2].to_broadcast([P, E]),
                                    op=ALU.is_equal)
            nc.vector.tensor_add(s1[:], s1[:], s2[:])
            sel_own = pd2.tile([P, E], F32, tag="sel_own")
            nc.vector.tensor_scalar_min(sel_own[:], s1[:], 1.0)
            wa = pd2.tile([P, E], F32, tag="wa")
            nc.vector.tensor_mul(wa[:], sel_own[:], sig[:])
            nrm = pd2.tile([P, 1], F32, tag="nrm")
            nc.vector.reduce_sum(nrm[:], wa[:], axis=AX.X)
            rec = pd2.tile([P, 1], F32, tag="recw")
            nc.vector.reciprocal(rec[:], nrm[:])
            w_tm = pd2.tile([P, E], F32, tag="wtm")
            nc.vector.tensor_scalar_mul(w_tm[:], wa[:], rec[:, :1])
            wsel_h = pd2.tile([P, 2 * E], F16, tag="wselh")
            nc.vector.tensor_copy(wsel_h[:, 0:E], w_tm[:])
            nc.vector.tensor_copy(wsel_h[:, E:2 * E], sel_own[:])
            nc.sync.dma_start(agx_in[:, H:H + 2 * E], wsel_h[:])
            nc.sync.dma_start(dbg_w[:], w_tm[:])

        nc.gpsimd.collective_compute(
            "AllGather", ALU.bypass, ins=[agx_in.opt()], outs=[xw_all.opt()],
            replica_groups=[list(range(NCN))])

        # ======== E: token lists from AllGathered router decisions ========
        # Inverse permutation (slot -> token id) built with matmuls instead of
        # 16 serialized indirect scatters: M[token, slot] = (rank == slot),
        # tok_list[slot] = sum_t M[t, slot] * t, with +BIG for empty slots.
        with tc.tile_pool(name="pe", bufs=1) as pe, \
             tc.tile_pool(name="pe2", bufs=3) as pe2, \
             tc.tile_pool(name="psE", bufs=2, space="PSUM") as psE, \
             tc.tile_pool(name="psE2", bufs=1, space="PSUM") as psE2:
            ut = pe.tile([P, P], F16)
            nc.sync.dma_start(ut[:], ex["ut_h"][:])
            slb = pe.tile([8, TB * P], F32R)
            nc.sync.dma_start(slb[:], ex["slb_in"][:].bitcast(F32R))
            s_iota = pe.tile([P, CAP], F32)
            nc.sync.dma_start(s_iota[:], ex["slot_iota"][:])
            tokid2 = pe.tile([P, 2 * TB], F16)
            nc.sync.dma_start(tokid2[:], ex["tokid2"][:])
            totals = pe.tile([8, E], F32R)
            pre_sb = [pe.tile([P, E], F32, tag=f"pre{b}", name=f"pre{b}")
                      for b in range(TB)]
            sel_all = [pe.tile([P, E], F16, tag=f"sela{b}", name=f"sela{b}")
                       for b in range(TB)]
            for b in range(TB):
                nc.sync.dma_start(
                    sel_all[b][:],
                    xw_all[b * P:(b + 1) * P, H + E:H + 2 * E])
                pr_ps = psE.tile([P, E], F32, tag="prps")
                nc.tensor.matmul(pr_ps[:], ut[:], sel_all[b][:],
                                 start=True, stop=True)
                nc.vector.tensor_copy(pre_sb[b][:], pr_ps[:])
                nc.sync.dma_start(totals[b:b + 1, :],
                                  pre_sb[b][127:128, :].bitcast(F32R))
            tl_ps = [[psE2.tile([P, 2], F32, tag=f"tl{ei}{ch}",
                                name=f"tl{ei}{ch}") for ch in range(2)]
                     for ei in range(2)]
            for b in range(TB):
                ofs_ps = psE.tile([P, E], F32, tag="ofsps", name="ofsps")
                nc.tensor.matmul(ofs_ps[:], slb[:, b * P:(b + 1) * P],
                                 totals[:], start=True, stop=True)
                grank = pe2.tile([P, E], F32, tag="grank")
                nc.vector.tensor_add(grank[:], pre_sb[b][:], ofs_ps[:])
                nc.vector.tensor_scalar_add(grank[:], grank[:], -1.0)
                gm = pe2.tile([P, E], F32, tag="gm")
                nc.vector.tensor_scalar(out=gm[:], in0=grank[:],
                                        scalar1=float(CAP - 1), scalar2=BIG,
                                        op0=ALU.is_gt, op1=ALU.mult)
                nc.vector.tensor_add(grank[:], grank[:], gm[:])
                um = pe2.tile([P, E], F32, tag="um")
                nc.vector.tensor_scalar(out=um[:], in0=sel_all[b][:],
                                        scalar1=-BIG, scalar2=BIG,
                                        op0=ALU.mult, op1=ALU.add)
                nc.vector.tensor_add(grank[:], grank[:], um[:])
                for ei in range(2):
                    ge = pe2.tile([P, E], F32, tag="ge")
                    nc.vector.tensor_mul(ge[:], grank[:],
                                         emask01[:, ei * E:(ei + 1) * E])
                    ridx = pe2.tile([P, 1], F32, tag="ridx")
                    nc.vector.reduce_sum(ridx[:], ge[:], axis=AX.X)
                    mb = pe2.tile([P, CAP], F16, tag="mb")
                    nc.vector.tensor_tensor(
                        out=mb[:], in0=s_iota[:],
                        in1=ridx[:, 0:1].to_broadcast([P, CAP]),
                        op=ALU.is_equal)
                    for ch in range(2):
                        nc.tensor.matmul(tl_ps[ei][ch][:],
                                         mb[:, ch * P:(ch + 1) * P],
                                         tokid2[:, 2 * b:2 * b + 2],
                                         start=(b == 0), stop=(b == TB - 1))
            for ei in range(2):
                for ch in range(2):
                    tl = pe2.tile([P, 2], F32, tag="tlsb")
                    nc.vector.tensor_copy(tl[:], tl_ps[ei][ch][:])
                    pad = pe2.tile([P, 1], F32, tag="pad")
                    nc.vector.tensor_scalar(out=pad[:], in0=tl[:, 1:2],
                                            scalar1=-BIG, scalar2=BIG,
                                            op0=ALU.mult, op1=ALU.add)
                    tok_f = pe2.tile([P, 1], F32, tag="tokf")
                    nc.vector.tensor_add(tok_f[:], tl[:, 0:1], pad[:])
                    nc.vector.tensor_copy(idx_sb2[ei][ch][:], tok_f[:])

        # ======== F: xT + shared expert + experts (fp16) ========
        with tc.tile_pool(name="pxt", bufs=1) as pxt, \
             tc.tile_pool(name="pfs", bufs=1) as pfs, \
             tc.tile_pool(name="pfs2", bufs=2) as pfs2:
            xc = [pxt.tile([P, T], F16, tag=f"xc{hc}", name=f"xc{hc}")
                  for hc in range(HC)]
            with tc.tile_pool(name="pxt2", bufs=3) as pxt2, \
                 tc.tile_pool(name="psX", bufs=2, space="PSUM") as psX:
                for b in range(TB):
                    xb = pxt2.tile([P, H], F16, tag="xb", bufs=2)
                    nc.sync.dma_start(xb[:], xw_all[b * P:(b + 1) * P, 0:H])
                    for hc in range(HC):
                        tp = psX.tile([P, P], F16, tag="tpX")
                        nc.tensor.transpose(tp[:], xb[:, hc * P:(hc + 1) * P],
                                            identh[:])
                        nc.vector.tensor_copy(xc[hc][:, b * P:(b + 1) * P],
                                              tp[:])

            # ---- both experts' setup: gathers, gxT, weights,
            # per-token gate weights — overlaps the shared expert below ----
            gxT2 = [pfs.tile([P, HC * 2 * P], F16, tag=f"gxT{ei}",
                             name=f"gxT{ei}") for ei in range(2)]
            wd_res2 = [[pfs.tile([P, H], F16, tag=f"wd{ei}{ip}",
                                 name=f"wd{ei}{ip}") for ip in range(IP)]
                       for ei in range(2)]
            wg_own2 = [[pfs.tile([P, 1], F32, tag=f"wgo{ei}{k}",
                                 name=f"wgo{ei}{k}") for k in range(2)]
                       for ei in range(2)]
            psS_cm = tc.tile_pool(name="psS", bufs=2, space="PSUM")
            psS = psS_cm.__enter__()
            for ei in range(2):
                for k in range(2):
                    # gather full rows: x plus the 32 W/sel columns ride along
                    gx = pfs2.tile([P, H + 2 * E], F16, tag="gx")
                    nc.vector.memset(gx[:], 0.0)
                    nc.gpsimd.indirect_dma_start(
                        out=gx[:], out_offset=None,
                        in_=xw_all[:],
                        in_offset=bass.IndirectOffsetOnAxis(
                            ap=idx_sb2[ei][k][:, :1], axis=0),
                        bounds_check=T - 1, oob_is_err=False)
                    for hc in range(HC):
                        tp = psS.tile([P, P], F16, tag="tpS")
                        nc.tensor.transpose(tp[:], gx[:, hc * P:(hc + 1) * P],
                                            identh[:])
                        nc.vector.tensor_copy(
                            gxT2[ei][:, hc * 2 * P + k * P:
                                  hc * 2 * P + (k + 1) * P], tp[:])
                    we_ = pfs2.tile([P, E], F32, tag="we_")
                    nc.vector.tensor_mul(we_[:], gx[:, H:H + E],
                                         emask01[:, ei * E:(ei + 1) * E])
                    nc.vector.reduce_sum(wg_own2[ei][k][:], we_[:], axis=AX.X)

            # ---- shared expert ----
            with tc.tile_pool(name="pg", bufs=1) as pg, \
                 tc.tile_pool(name="pg2", bufs=3) as pg2:
                g_act = [pg.tile([P, T], F16, tag=f"gact{sp}", name=f"gact{sp}")
                         for sp in range(SP)]
                hs = [pg.tile([P, T], F16, tag=f"hs{sp}", name=f"hs{sp}")
                      for sp in range(SP)]
                with tc.tile_pool(name="psG1", bufs=1, space="PSUM") as psG1:
                    g_ps = [psG1.tile([P, T], F32, tag=f"gps{sp}",
                                      name=f"gps{sp}") for sp in range(SP)]
                    for hc in range(HC):
                        for sp in range(SP):
                            c0 = hc * SP * P + sp * P
                            for n in range(2):
                                sl = slice(n * 512, (n + 1) * 512)
                                nc.tensor.matmul(g_ps[sp][:, sl],
                                                 wsg_sb[:, c0:c0 + P],
                                                 xc[hc][:, sl],
                                                 start=(hc == 0),
                                                 stop=(hc == HC - 1))
                    for sp in range(SP):
                        nc.scalar.activation(g_act[sp][:], g_ps[sp][:],
                                             AF.Silu)
                with tc.tile_pool(name="psG2", bufs=1, space="PSUM") as psG2:
                    u_ps = [psG2.tile([P, T], F32, tag=f"ups{sp}",
                                      name=f"ups{sp}") for sp in range(SP)]
                    for hc in range(HC):
                        for sp in range(SP):
                            c0 = hc * SP * P + sp * P
                            for n in range(2):
                                sl = slice(n * 512, (n + 1) * 512)
                                nc.tensor.matmul(u_ps[sp][:, sl],
                                                 wsu_sb[:, c0:c0 + P],
                                                 xc[hc][:, sl],
                                                 start=(hc == 0),
                                                 stop=(hc == HC - 1))
                    for sp in range(SP):
                        nc.vector.tensor_mul(hs[sp][:], g_act[sp][:],
                                             u_ps[sp][:])
                with tc.tile_pool(name="psG3", bufs=6, space="PSUM") as psG3:
                    for tb_ in range(TB):
                        psd = [psG3.tile([P, 512], F32, tag="psGd",
                                         name=f"psGd{n}") for n in range(4)]
                        for sp in range(SP):
                            for n in range(4):
                                nc.tensor.matmul(
                                    psd[n][:],
                                    hs[sp][:, tb_ * P:(tb_ + 1) * P],
                                    wsd_sb[sp][:, n * 512:(n + 1) * 512],
                                    start=(sp == 0), stop=(sp == SP - 1))
                        sbd = pg2.tile([P, H], F16, tag="sbGd", bufs=2)
                        for n in range(4):
                            nc.vector.tensor_copy(
                                sbd[:, n * 512:(n + 1) * 512], psd[n][:])
                        nc.sync.dma_start(rs2_in[tb_ * P:(tb_ + 1) * P, :],
                                          sbd[:])

            psS_cm.__exit__(None, None, None)

            # expert down-proj weights: emitted late so these 8 MB of DMAs
            # sit behind the x-block/gather traffic in queue priority, but
            # they still have ~100us of slack before first use
            for ei in range(2):
                for ip in range(IP):
                    nc.sync.dma_start(wd_res2[ei][ip][:],
                                      ex["we_d"][ei, ip * P:(ip + 1) * P, :])

            # ---- experts (setup already done above) ----
            for ei in range(2):
                with tc.tile_pool(name=f"pf{ei}", bufs=1) as pf, \
                     tc.tile_pool(name=f"pf2{ei}", bufs=2) as pf2:
                    idx_sb = idx_sb2[ei]
                    gxT = gxT2[ei]
                    wd_res = wd_res2[ei]

                    # merged gate+up pass (8 PSUM banks)
                    g_tm = [pf.tile([P, I], F16, tag=f"gtm{k}", name=f"gtm{k}")
                            for k in range(2)]
                    h_tm = [pf.tile([P, I], F16, tag=f"htm{k}", name=f"htm{k}")
                            for k in range(2)]
                    with tc.tile_pool(name=f"psF2{ei}", bufs=1,
                                      space="PSUM") as psF2:
                        gu_ps = [[psF2.tile([P, 512], F32, tag=f"gups{k}{j}",
                                            name=f"gups{k}{j}")
                                  for j in range(4)] for k in range(2)]
                        for hc in range(HC):
                            wg_c = pf2.tile([P, I], F16, tag="wgF", bufs=3)
                            nc.sync.dma_start(
                                wg_c[:], ex["we_g"][ei, hc * P:(hc + 1) * P, :])
                            wu_c = pf2.tile([P, I], F16, tag="wuF", bufs=3)
                            nc.sync.dma_start(
                                wu_c[:], ex["we_u"][ei, hc * P:(hc + 1) * P, :])
                            for k in range(2):
                                s_ = gxT[:, hc * 2 * P + k * P:
                                         hc * 2 * P + (k + 1) * P]
                                for n in range(2):
                                    nc.tensor.matmul(
                                        gu_ps[k][n][:], s_,
                                        wg_c[:, n * 512:(n + 1) * 512],
                                        start=(hc == 0), stop=(hc == HC - 1))
                                for n in range(2):
                                    nc.tensor.matmul(
                                        gu_ps[k][2 + n][:], s_,
                                        wu_c[:, n * 512:(n + 1) * 512],
                                        start=(hc == 0), stop=(hc == HC - 1))
                        for k in range(2):
                            for n in range(2):
                                sl = slice(n * 512, (n + 1) * 512)
                                nc.scalar.activation(g_tm[k][:, sl],
                                                     gu_ps[k][n][:], AF.Silu)
                                nc.vector.tensor_mul(h_tm[k][:, sl],
                                                     g_tm[k][:, sl],
                                                     gu_ps[k][2 + n][:])
                    h_sb = [pf.tile([P, 2 * P], F16, tag=f"hsb{ip}",
                                    name=f"hsb{ip}") for ip in range(IP)]
                    with tc.tile_pool(name=f"psF4{ei}", bufs=2,
                                      space="PSUM") as psF4:
                        for k in range(2):
                            for ip in range(IP):
                                tp = psF4.tile([P, P], F16, tag="tpF2")
                                nc.tensor.transpose(
                                    tp[:], h_tm[k][:, ip * P:(ip + 1) * P],
                                    identh[:])
                                nc.vector.tensor_copy(
                                    h_sb[ip][:, k * P:(k + 1) * P], tp[:])
                    with tc.tile_pool(name=f"psF5{ei}", bufs=8,
                                      space="PSUM") as psF5:
                        for k in range(2):
                            psd = [psF5.tile([P, 512], F32, tag="psFd",
                                             name=f"psFd{n}")
                                   for n in range(4)]
                            for ip in range(IP):
                                for n in range(4):
                                    nc.tensor.matmul(
                                        psd[n][:],
                                        h_sb[ip][:, k * P:(k + 1) * P],
                                        wd_res[ip][:, n * 512:(n + 1) * 512],
                                        start=(ip == 0), stop=(ip == IP - 1))
                            out_sb = pf.tile([P, H], F16, tag=f"outsb{k}")
                            for n in range(4):
                                nc.vector.tensor_scalar_mul(
                                    out_sb[:, n * 512:(n + 1) * 512],
                                    psd[n][:], wg_own2[ei][k][:, :1])
                            nc.gpsimd.indirect_dma_start(
                                out=rs2_in[:],
                                out_offset=bass.IndirectOffsetOnAxis(
                                    ap=idx_sb[k][:, :1], axis=0),
                                in_=out_sb[:], in_offset=None,
                                bounds_check=T - 1, oob_is_err=False,
                                compute_op=ALU.add)

        nc.gpsimd.collective_compute(
            "ReduceScatter", ALU.add, ins=[rs2_in.opt()], outs=[rs2_out.opt()],
            replica_groups=[list(range(NCN))])
        with tc.tile_pool(name="pz", bufs=2) as pz:
            fin16 = pz.tile([P, H], F16)
            nc.sync.dma_start(fin16[:], rs2_out[:])
            fin = pz.tile([P, H], F32)
            nc.vector.tensor_copy(fin[:], fin16[:])
            nc.sync.dma_start(out_slice[:], fin[:])


_CACHE = {}


def _build():
    key = "nc"
    if key in _CACHE:
        return _CACHE[key]
    nc = bacc.Bacc("TRN2", target_bir_lowering=False, debug=False,
                   num_devices=NCN)
    with tile.TileContext(nc) as tc:
        _emit(nc, tc)
    nc.compile()
    _CACHE[key] = nc
    return nc


def _host_prep(inputs):
    f16 = np.float16
    pos = np.asarray(inputs["positions"]).astype(np.float64)
    hid = np.asarray(inputs["hidden_states"], np.float32)
    w_in = np.asarray(inputs["w_in_ln"], np.float32)
    w_post = np.asarray(inputs["w_post_ln"], np.float32)
    wq = (np.asarray(inputs["wq"], np.float32) * w_in[:, None]).astype(f16)
    wk = (np.asarray(inputs["wk"], np.float32) * w_in[:, None]).astype(f16)
    wv = (np.asarray(inputs["wv"], np.float32) * w_in[:, None]).astype(f16)
    wo = np.asarray(inputs["wo"], np.float32).astype(f16)
    gate_w = np.asarray(inputs["gate_w"], np.float32) * w_post[None, :]
    gate_b = np.asarray(inputs["gate_bias"], np.float32).reshape(1, E)
    we_g = (np.asarray(inputs["we_gate"], np.float32)
            * w_post[None, :, None]).astype(f16)
    we_u = (np.asarray(inputs["we_up"], np.float32)
            * w_post[None, :, None]).astype(f16)
    we_d = np.asarray(inputs["we_down"], np.float32).astype(f16)
    ws_g = (np.asarray(inputs["ws_gate"], np.float32)
            * w_post[:, None]).astype(f16)
    ws_u = (np.asarray(inputs["ws_up"], np.float32)
            * w_post[:, None]).astype(f16)
    ws_d = np.asarray(inputs["ws_down"], np.float32).astype(f16)

    inv_freq = 1.0 / (THETA ** (np.arange(0, D, 2, dtype=np.float64) / D))
    f = pos[None, :] * inv_freq[:, None]
    cos2, sin2 = np.cos(f), np.sin(f)
    cosT = np.repeat(cos2, 2, axis=0).astype(np.float32)
    sinT = np.empty((D, T), np.float32)
    sinT[0::2] = -sin2
    sinT[1::2] = sin2
    s = 1.0 / np.sqrt(D)
    cosq, sinq = (cosT * s).astype(np.float32), (sinT * s).astype(np.float32)

    import ml_dtypes
    bf = ml_dtypes.bfloat16
    ii = np.arange(P)
    diag_mask = np.where(ii[:, None] >= ii[None, :], 0.0, NEG).astype(bf)

    identr_in = np.eye(P, dtype=np.float32)
    identh_in = np.eye(P, dtype=f16)
    ut_in = np.triu(np.ones((P, P), np.float32))
    slb_in = np.zeros((8, TB * P), np.float32)
    for b in range(TB):
        slb_in[:b, b * P:(b + 1) * P] = 1.0
    perm = np.zeros((P, P), np.float32)
    for i in range(0, P, 2):
        perm[i, i + 1] = 1.0
        perm[i + 1, i] = 1.0
    slot_iota = np.broadcast_to(np.arange(CAP, dtype=np.float32),
                                (P, CAP)).copy()
    tokid2 = np.zeros((P, 2 * TB), f16)
    for b in range(TB):
        tokid2[:, 2 * b] = (b * P + np.arange(P)).astype(f16)
        tokid2[:, 2 * b + 1] = 1.0

    ISC = IS // NCN
    maps = []
    for c in range(NCN):
        g = c // 2
        emask01 = np.zeros((P, 2 * E), np.float32)
        emask01[:, 2 * c] = 1.0          # ei = 0 -> expert 2c
        emask01[:, E + 2 * c + 1] = 1.0  # ei = 1 -> expert 2c+1
        maps.append({
            "hid": hid,
            "hid_slice": np.ascontiguousarray(hid[c * P:(c + 1) * P]),
            "wq_s": np.ascontiguousarray(wq[:, 2 * c * D:(2 * c + 2) * D]),
            "wk_s": np.ascontiguousarray(wk[:, g * D:(g + 1) * D]),
            "wv_s": np.ascontiguousarray(wv[:, g * D:(g + 1) * D]),
            "wo_s": np.ascontiguousarray(wo[2 * c * D:(2 * c + 2) * D, :]),
            "cosq": cosq, "sinq": sinq, "cosk": cosT, "sink": sinT,
            "perm": perm, "diag_mask": diag_mask,
            "identr_in": identr_in, "identh_in": identh_in,
            "ut_in": ut_in, "ut_h": ut_in.astype(f16), "slb_in": slb_in,
            "slot_iota": slot_iota, "tokid2": tokid2,
            "gate_wT": np.ascontiguousarray(gate_w.T),
            "gate_b": np.broadcast_to(gate_b, (P, E)).copy(),
            "emask01": emask01,
            "ws_g": np.ascontiguousarray(ws_g[:, c * ISC:(c + 1) * ISC]),
            "ws_u": np.ascontiguousarray(ws_u[:, c * ISC:(c + 1) * ISC]),
            "ws_d": np.ascontiguousarray(ws_d[c * ISC:(c + 1) * ISC, :]),
            "we_g": np.ascontiguousarray(we_g[2 * c:2 * c + 2]),
            "we_u": np.ascontiguousarray(we_u[2 * c:2 * c + 2]),
            "we_d": np.ascontiguousarray(we_d[2 * c:2 * c + 2]),
        })
    return maps


def kernel(trace=False, **inputs):
    nc = _build()
    maps = _host_prep(inputs)
    res = bass_utils.run_bass_kernel_spmd(
        nc, maps, core_ids=list(range(NCN)), trace=trace)
    out = np.concatenate([res.results[c]["out_slice"] for c in range(NCN)], 0)
    resid = np.concatenate([res.results[c]["res_slice"] for c in range(NCN)], 0)
    kernel.last_results = res
    return out, resid


# revision 33
# speedup vs baseline: 1.6208x; 1.0046x over previous
"""Ernie4 decoder layer (RMSNorm + GQA attention + shared expert + 16-expert
top-2 MoE) on 8 Trainium2 NeuronCores.

v2 — fp16 data path everywhere except the router (which must reproduce the
reference top-2 selection exactly; margins are ~3e-5 so it stays fp32 and is
computed locally per core before the AllGather):
  - Attention: head-parallel (2 q-heads + 1 kv-head per core), fp16 QKV /
    scores / probs / o_proj with causal-block skipping; fp16 ReduceScatter.
  - Router: fp32 logits on each core's own 128 tokens; W+sel AllGathered in a
    tiny fp32 collective that precedes the fp16 x AllGather so the token-list
    build overlaps it.
  - Shared expert: intermediate-sharded (IS/8 per core) fp16, output seeds
    the MoE combine buffer.
  - MoE: expert-parallel (2 experts per core), token lists via
    triangular-matmul prefix ranks, indirect-DMA gather/scatter-add in fp16,
    fp16 ReduceScatter for the combine.
"""
import sys
sys.path.insert(0, "/opt/trn_rl_repo")

import numpy as np

import concourse.bass as bass
import concourse.bacc as bacc
import concourse.tile as tile
import concourse.mybir as mybir
from concourse import bass_utils
from concourse.masks import make_identity
from concourse.tile import add_dep_helper

dt = mybir.dt
F32 = dt.float32
F32R = dt.float32r
F16 = dt.float16
I32 = dt.int32
BF16 = dt.bfloat16
AF = mybir.ActivationFunctionType
ALU = mybir.AluOpType
AX = mybir.AxisListType

T, H, NH, NKV, D = 1024, 2048, 16, 4, 128
E, I, IS = 16, 1024, 2048
EPS = 1e-6
THETA = 10000.0
NCN = 8
P = 128
TB = T // P            # 8 token blocks
HC = H // P            # 16 hidden chunks
IP = I // P            # 8 expert-intermediate chunks
SP = IS // NCN // P    # 2 shared-intermediate chunks per core
CAP = 256              # per-expert token capacity
BIG = 1.0e6            # OOB sentinel
NEG = -1e9


def _emit(nc, tc):
    ex = {}
    for name, shape, d in [
        ("hid", [T, H], F32), ("hid_slice", [P, H], F32),
        ("wq_s", [H, 2 * D], F16), ("wk_s", [H, D], F16), ("wv_s", [H, D], F16),
        ("wo_s", [2 * D, H], F16),
        ("cosq", [D, T], F32), ("sinq", [D, T], F32),
        ("cosk", [D, T], F32), ("sink", [D, T], F32),
        ("perm", [P, P], F32),
        ("diag_mask", [P, P], BF16),
        ("gate_wT", [H, E], F32), ("gate_b", [P, E], F32),
        ("emask01", [P, 2 * E], F32),
        ("ws_g", [H, SP * P], F16), ("ws_u", [H, SP * P], F16),
        ("ws_d", [SP * P, H], F16),
        ("we_g", [2, H, I], F16), ("we_u", [2, H, I], F16),
        ("we_d", [2, I, H], F16),
        ("identr_in", [P, P], F32), ("identh_in", [P, P], F16),
        ("ut_in", [P, P], F32), ("ut_h", [P, P], F16),
        ("slb_in", [8, TB * P], F32),
        ("slot_iota", [P, CAP], F32), ("tokid2", [P, 2 * TB], F16),
    ]:
        ex[name] = nc.dram_tensor(name, shape, d, kind="ExternalInput").ap()
    out_slice = nc.dram_tensor("out_slice", [P, H], F32, kind="ExternalOutput").ap()
    res_slice = nc.dram_tensor("res_slice", [P, H], F32, kind="ExternalOutput").ap()
    dbg_w = nc.dram_tensor("dbg_w", [P, E], F32, kind="ExternalOutput").ap()

    with tc.tile_pool(name="persist", bufs=1) as pp, \
         tc.tile_pool(name="dram", bufs=1, space="DRAM") as dram:
        rs_in = dram.tile([T, H], F16)
        rs_out = dram.tile([P, H], F16)
        agx_in = dram.tile([P, H + 2 * E], F16)
        xw_all = dram.tile([T, H + 2 * E], F16, addr_space="Shared")
        agw_in = dram.tile([P, 2 * E], F16)
        w_all = dram.tile([T, 2 * E], F16, addr_space="Shared")
        warm_in = dram.tile([8, 8], F16)
        warm_out = dram.tile([64, 8], F16, addr_space="Shared")
        rs2_in = dram.tile([T, H], F16)
        rs2_out = dram.tile([P, H], F16)

        ident = pp.tile([P, P], F32)
        make_identity(nc, ident[:])
        identr = pp.tile([P, P], F32R)
        nc.sync.dma_start(identr[:], ex["identr_in"][:].bitcast(F32R))
        identh = pp.tile([P, P], F16)
        nc.sync.dma_start(identh[:], ex["identh_in"][:])
        hid_sl = pp.tile([P, H], F32)
        nc.sync.dma_start(hid_sl[:], ex["hid_slice"][:])
        eps_t = pp.tile([P, 1], F32)
        nc.vector.memset(eps_t[:], EPS)
        emask01 = pp.tile([P, 2 * E], F32)
        nc.sync.dma_start(emask01[:], ex["emask01"][:])
        wz = pp.tile([8, 8], F16)
        nc.vector.memset(wz[:], 0.0)
        nc.sync.dma_start(warm_in[:], wz[:])
        nc.gpsimd.collective_compute(
            "AllGather", ALU.bypass, ins=[warm_in.opt()],
            outs=[warm_out.opt()], replica_groups=[list(range(NCN))])
        # per-expert token lists live in SBUF end-to-end (built by the
        # matmul-based inverse permutation in phase E, consumed in F)
        idx_sb2 = [[pp.tile([P, 1], I32, tag=f"idx{ei}{k}",
                            name=f"idx{ei}{k}") for k in range(2)]
                   for ei in range(2)]
        # shared-expert weights are pure inputs: load them from t=0 so the
        # post-AllGather phase never waits on weight DMAs
        wsg_sb = pp.tile([P, HC * SP * P], F16)
        wsu_sb = pp.tile([P, HC * SP * P], F16)
        for t_, s_ in [(wsg_sb, "ws_g"), (wsu_sb, "ws_u")]:
            nc.sync.dma_start(
                t_[:].rearrange("p (hc m) -> p hc m", hc=HC),
                ex[s_][:].rearrange("(hc p) m -> p hc m", p=P))
        wsd_sb = [pp.tile([P, H], F16, tag=f"wsd{sp}", name=f"wsd{sp}")
                  for sp in range(SP)]
        for sp in range(SP):
            nc.sync.dma_start(wsd_sb[sp][:],
                              ex["ws_d"][sp * P:(sp + 1) * P, :])

        # ======== Phases A-C: attention (fp16) ========
        with tc.tile_pool(name="pab", bufs=1) as pab:
            qT = [pab.tile([P, T], F16, tag=f"qT{j}", name=f"qT{j}")
                  for j in range(2)]
            kT = pab.tile([P, T], F16)
            vT = pab.tile([P, T], F16)
            v_tm = [pab.tile([P, D], F16, tag=f"vtm{b}", name=f"vtm{b}")
                    for b in range(TB)]
            oT = [pab.tile([P, T], F16, tag=f"oT{j}", name=f"oT{j}")
                  for j in range(2)]

            # ---- A: norm + transpose + QKV + rope ----
            with tc.tile_pool(name="pa", bufs=1) as pa, \
                 tc.tile_pool(name="pa2", bufs=3) as pa2:
                cosq = pa.tile([D, T], F32)
                sinq = pa.tile([D, T], F32)
                cosk = pa.tile([D, T], F32)
                sink = pa.tile([D, T], F32)
                for t_, s_ in [(cosq, "cosq"), (sinq, "sinq"),
                               (cosk, "cosk"), (sink, "sink")]:
                    nc.sync.dma_start(t_[:], ex[s_][:])
                permr = pa.tile([P, P], F32R)
                nc.sync.dma_start(permr[:], ex["perm"][:].bitcast(F32R))
                wq_sb = pa.tile([P, HC * 2 * D], F16)
                wk_sb = pa.tile([P, HC * D], F16)
                wv_sb = pa.tile([P, HC * D], F16)
                for t_, s_, m in [(wq_sb, "wq_s", 2 * D), (wk_sb, "wk_s", D),
                                  (wv_sb, "wv_s", D)]:
                    nc.sync.dma_start(
                        t_[:].rearrange("p (hc m) -> p hc m", hc=HC),
                        ex[s_][:].rearrange("(hc p) m -> p hc m", p=P))

                dump = pa.tile([P, H], F32)
                qraw = [pa.tile([P, T], F32R, tag=f"qraw{j}", name=f"qraw{j}")
                        for j in range(2)]
                kraw = pa.tile([P, T], F32R)
                with tc.tile_pool(name="psA1", bufs=2, space="PSUM") as psA1, \
                     tc.tile_pool(name="psA2", bufs=2, space="PSUM") as psA2:
                    for n in range(2):
                        x0T = [pa.tile([P, 512], F16, tag=f"x0T{hc}",
                                       name=f"x0T{hc}_{n}") for hc in range(HC)]
                        for bb in range(TB // 2):
                            b = n * (TB // 2) + bb
                            hidb = pa2.tile([P, H], F32, tag="hidb", bufs=2)
                            nc.sync.dma_start(hidb[:],
                                              ex["hid"][b * P:(b + 1) * P, :])
                            ssum = pa2.tile([P, 1], F32, tag="ssum")
                            nc.scalar.activation(dump[:], hidb[:], AF.Square,
                                                 accum_out=ssum[:, :1])
                            rms = pa2.tile([P, 1], F32, tag="rms")
                            nc.scalar.activation(rms[:], ssum[:],
                                                 AF.Sqrt, bias=eps_t[:, :1],
                                                 scale=1.0 / H)
                            inv = pa2.tile([P, 1], F32, tag="inv")
                            nc.vector.reciprocal(inv[:], rms[:])
                            x0b = pa2.tile([P, H], F16, tag="x0b", bufs=2)
                            nc.vector.tensor_scalar_mul(x0b[:], hidb[:],
                                                        inv[:, :1])
                            for hc in range(HC):
                                tp = psA1.tile([P, P], F16, tag="tpA")
                                nc.tensor.transpose(
                                    tp[:], x0b[:, hc * P:(hc + 1) * P],
                                    identh[:])
                                nc.vector.tensor_copy(
                                    x0T[hc][:, bb * P:(bb + 1) * P], tp[:])

                        def proj(w_sb, m, c0, dst, n=n, x0T=x0T, fp16=False):
                            ps = psA2.tile([P, 512], F32, tag="psQKV",
                                           name="psQKV")
                            for hc in range(HC):
                                nc.tensor.matmul(
                                    ps[:],
                                    w_sb[:, hc * m + c0:hc * m + c0 + P],
                                    x0T[hc][:],
                                    start=(hc == 0), stop=(hc == HC - 1))
                            nc.vector.tensor_copy(
                                dst[:, n * 512:(n + 1) * 512], ps[:])
                        proj(wq_sb, 2 * D, 0, qraw[0])
                        proj(wq_sb, 2 * D, D, qraw[1])
                        proj(wk_sb, D, 0, kraw)
                        proj(wv_sb, D, 0, vT, fp16=True)

                with tc.tile_pool(name="psA3", bufs=2, space="PSUM") as psA3:
                    for src, dst, c_, s_ in [(qraw[0], qT[0], cosq, sinq),
                                             (qraw[1], qT[1], cosq, sinq),
                                             (kraw, kT, cosk, sink)]:
                        for n in range(2):
                            sl = slice(n * 512, (n + 1) * 512)
                            sw = psA3.tile([P, 512], F32, tag="psSW")
                            nc.tensor.matmul(sw[:], permr[:], src[:, sl],
                                             start=True, stop=True)
                            t1 = pa2.tile([P, 512], F32, tag="ropeT1")
                            nc.vector.tensor_mul(t1[:], src[:, sl], c_[:, sl])
                            t2 = pa2.tile([P, 512], F32, tag="ropeT2")
                            nc.vector.tensor_mul(t2[:], sw[:], s_[:, sl])
                            nc.vector.tensor_add(dst[:, sl], t1[:], t2[:])
                    for b in range(TB):
                        tp = psA3.tile([P, P], F16, tag="tpV")
                        nc.tensor.transpose(tp[:], vT[:, b * P:(b + 1) * P],
                                            identh[:])
                        nc.vector.tensor_copy(v_tm[b][:], tp[:])

            # ---- B: attention (causal-block skipped) ----
            with tc.tile_pool(name="pb", bufs=1) as pb, \
                 tc.tile_pool(name="pb2", bufs=3) as pb2:
                dmask = pb.tile([P, P], BF16)
                nc.sync.dma_start(dmask[:], ex["diag_mask"][:])
                wo_sb = [pb.tile([P, H], F16, tag=f"wo{j}", name=f"wo{j}")
                         for j in range(2)]
                nc.sync.dma_start(wo_sb[0][:], ex["wo_s"][0:P, :])
                nc.sync.dma_start(wo_sb[1][:], ex["wo_s"][P:2 * P, :])

                attnT = [pb.tile([P, T], F16, tag=f"attnT{kc}",
                                 name=f"attnT{kc}") for kc in range(TB)]
                for kc in range(1, TB):
                    nc.vector.memset(attnT[kc][:, 0:kc * P], 0.0)
                with tc.tile_pool(name="psB1", bufs=2, space="PSUM") as psB1, \
                     tc.tile_pool(name="psB2", bufs=2, space="PSUM") as psB2, \
                     tc.tile_pool(name="psB3", bufs=2, space="PSUM") as psB3:
                  for h in range(2):
                    for qc in range(TB):
                        cols = (qc + 1) * P
                        prob = pb2.tile([P, T], F32, tag="prob")
                        nsl = (cols + 511) // 512
                        for n in range(nsl):
                            w_ = min(512, cols - n * 512)
                            ps = psB1.tile([P, 512], F32, tag="psSC")
                            nc.tensor.matmul(ps[:, :w_],
                                             qT[h][:, qc * P:(qc + 1) * P],
                                             kT[:, n * 512:n * 512 + w_],
                                             start=True, stop=True)
                            # diagonal block gets the causal mask; the rest
                            # of this slice is fully visible
                            d0 = qc * P - n * 512
                            if 0 <= d0 < w_:
                                if d0 > 0:
                                    nc.vector.tensor_copy(
                                        prob[:, n * 512:n * 512 + d0],
                                        ps[:, :d0])
                                nc.vector.tensor_add(
                                    prob[:, qc * P:qc * P + P],
                                    ps[:, d0:d0 + P], dmask[:])
                            else:
                                nc.vector.tensor_copy(
                                    prob[:, n * 512:n * 512 + w_], ps[:, :w_])
                        mx = pb2.tile([P, 1], F32, tag="mx")
                        nc.vector.reduce_max(mx[:], prob[:, :cols], axis=AX.X)
                        negm = pb2.tile([P, 1], F32, tag="negm")
                        nc.vector.tensor_scalar_mul(negm[:], mx[:], -1.0)
                        ssum = pb2.tile([P, 1], F32, tag="esum")
                        probe_ = pb2.tile([P, T], F32, tag="probe")
                        nc.scalar.activation(probe_[:, :cols], prob[:, :cols],
                                             AF.Exp, bias=negm[:, :1],
                                             accum_out=ssum[:, :1])
                        rec = pb2.tile([P, 1], F32, tag="rec")
                        nc.vector.reciprocal(rec[:], ssum[:])
                        probS = pb2.tile([P, T], F16, tag="probS")
                        nc.vector.tensor_scalar_mul(probS[:, :cols],
                                                    probe_[:, :cols],
                                                    rec[:, :1])
                        for kc in range(qc + 1):
                            tp = psB2.tile([P, P], F16, tag="tpB")
                            nc.tensor.transpose(
                                tp[:], probS[:, kc * P:(kc + 1) * P],
                                identh[:])
                            nc.vector.tensor_copy(
                                attnT[kc][:, qc * P:(qc + 1) * P], tp[:])
                    for n in range(2):
                        sl = slice(n * 512, (n + 1) * 512)
                        kc_hi = 4 * n + 3
                        ps = psB3.tile([P, 512], F32, tag="psAV")
                        for kc in range(kc_hi + 1):
                            nc.tensor.matmul(ps[:], v_tm[kc][:],
                                             attnT[kc][:, sl],
                                             start=(kc == 0),
                                             stop=(kc == kc_hi))
                        nc.vector.tensor_copy(oT[h][:, sl], ps[:])

                # ---- C: o_proj ----
                with tc.tile_pool(name="psC", bufs=8, space="PSUM") as psC:
                    for tb_ in range(TB):
                        pso = [psC.tile([P, 512], F32, tag="psO",
                                        name=f"psO{n}") for n in range(4)]
                        for hp in range(2):
                            for n in range(4):
                                nc.tensor.matmul(
                                    pso[n][:],
                                    oT[hp][:, tb_ * P:(tb_ + 1) * P],
                                    wo_sb[hp][:, n * 512:(n + 1) * 512],
                                    start=(hp == 0), stop=(hp == 1))
                        ob = pb2.tile([P, H], F16, tag="ob", bufs=2)
                        for n in range(4):
                            nc.vector.tensor_copy(
                                ob[:, n * 512:(n + 1) * 512], pso[n][:])
                        nc.sync.dma_start(rs_in[tb_ * P:(tb_ + 1) * P, :],
                                          ob[:])

        nc.gpsimd.collective_compute(
            "ReduceScatter", ALU.add, ins=[rs_in.opt()], outs=[rs_out.opt()],
            replica_groups=[list(range(NCN))])

        # ======== D: residual + norm + local fp32 router + AGs ========
        with tc.tile_pool(name="pd", bufs=1) as pd, \
             tc.tile_pool(name="pd2", bufs=2) as pd2, \
             tc.tile_pool(name="psD", bufs=2, space="PSUM") as psD:
            attn_sl = pd.tile([P, H], F16)
            nc.sync.dma_start(attn_sl[:], rs_out[:])
            res_sb = pd.tile([P, H], F32)
            nc.vector.tensor_add(res_sb[:], hid_sl[:], attn_sl[:])
            nc.sync.dma_start(res_slice[:], res_sb[:])
            dump2 = pd.tile([P, H], F32)
            ssum = pd.tile([P, 1], F32)
            nc.scalar.activation(dump2[:], res_sb[:], AF.Square,
                                 accum_out=ssum[:, :1])
            rms = pd.tile([P, 1], F32)
            nc.scalar.activation(rms[:], ssum[:], AF.Sqrt, bias=eps_t[:, :1],
                                 scale=1.0 / H)
            inv = pd.tile([P, 1], F32)
            nc.vector.reciprocal(inv[:], rms[:])
            x_sl = pd.tile([P, H], F32)
            nc.vector.tensor_scalar_mul(x_sl[:], res_sb[:], inv[:, :1])
            x_sl_h = pd.tile([P, H], F16)
            nc.vector.tensor_copy(x_sl_h[:], x_sl[:])
            nc.sync.dma_start(agx_in[:, 0:H], x_sl_h[:])

            # exact fp32 router on the un-normalized residual: transposes and
            # logit matmuls run in parallel with the rmsnorm stats, and the
            # 1/rms scale folds into the sigmoid's per-token scale operand
            gw_sb = pd.tile([P, HC * E], F32)
            nc.sync.dma_start(
                gw_sb[:].rearrange("p (hc e) -> p hc e", hc=HC),
                ex["gate_wT"][:].rearrange("(hc p) e -> p hc e", p=P))
            gate_b = pd.tile([P, E], F32)
            nc.sync.dma_start(gate_b[:], ex["gate_b"][:])
            resT = pd.tile([P, H], F32)
            for hc in range(HC):
                tp = psD.tile([P, P], F32, tag="tpD")
                nc.tensor.transpose(tp[:], res_sb[:, hc * P:(hc + 1) * P],
                                    ident[:])
                nc.vector.tensor_copy(resT[:, hc * P:(hc + 1) * P], tp[:])
            lg_ps = psD.tile([P, E], F32, tag="lgps", name="lgps")
            for hc in range(HC):
                nc.tensor.matmul(lg_ps[:], resT[:, hc * P:(hc + 1) * P],
                                 gw_sb[:, hc * E:(hc + 1) * E],
                                 start=(hc == 0), stop=(hc == HC - 1))
            sig = pd2.tile([P, E], F32, tag="sig")
            nc.scalar.activation(sig[:], lg_ps[:], AF.Sigmoid,
                                 scale=inv[:, :1])
            sb_ = pd2.tile([P, E], F32, tag="sb_")
            nc.vector.tensor_add(sb_[:], sig[:], gate_b[:])
            mx = pd2.tile([P, 8], F32, tag="mx8")
            nc.vector.max(out=mx[:], in_=sb_[:])
            s1 = pd2.tile([P, E], F32, tag="s1")
            nc.vector.tensor_tensor(out=s1[:], in0=sb_[:],
                                    in1=mx[:, 0:1].to_broadcast([P, E]),
                                    op=ALU.is_equal)
            s2 = pd2.tile([P, E], F32, tag="s2")
            nc.vector.tensor_tensor(out=s2[:], in0=sb_[:],
                                    in1=mx[:, 1:2].to_broadcast([P, E]),
                                    op=ALU.is_equal)
            nc.vector.tensor_add(s1[:], s1[:], s2[:])
            sel_own = pd2.tile([P, E], F32, tag="sel_own")
            nc.vector.tensor_scalar_min(sel_own[:], s1[:], 1.0)
            wa = pd2.tile([P, E], F32, tag="wa")
            nc.vector.tensor_mul(wa[:], sel_own[:], sig[:])
            nrm = pd2.tile([P, 1], F32, tag="nrm")
            nc.vector.reduce_sum(nrm[:], wa[:], axis=AX.X)
            rec = pd2.tile([P, 1], F32, tag="recw")
            nc.vector.reciprocal(rec[:], nrm[:])
            w_tm = pd2.tile([P, E], F32, tag="wtm")
            nc.vector.tensor_scalar_mul(w_tm[:], wa[:], rec[:, :1])
            wsel_h = pd2.tile([P, 2 * E], F16, tag="wselh")
            nc.vector.tensor_copy(wsel_h[:, 0:E], w_tm[:])
            nc.vector.tensor_copy(wsel_h[:, E:2 * E], sel_own[:])
            nc.sync.dma_start(agx_in[:, H:H + 2 * E], wsel_h[:])
            nc.sync.dma_start(agw_in[:], wsel_h[:])
            nc.sync.dma_start(dbg_w[:], w_tm[:])

        cc_w = nc.gpsimd.collective_compute(
            "AllGather", ALU.bypass, ins=[agw_in.opt()], outs=[w_all.opt()],
            replica_groups=[list(range(NCN))])
        cc_x = nc.gpsimd.collective_compute(
            "AllGather", ALU.bypass, ins=[agx_in.opt()], outs=[xw_all.opt()],
            replica_groups=[list(range(NCN))])
        # tiny W/sel AllGather first: the token-list build overlaps the big
        # x AllGather (CC queue executes in trigger order)
        add_dep_helper(cc_x.ins, cc_w.ins, sync=True,
                       reason="AG_W before AG_x")

        # ======== E: token lists from AllGathered router decisions ========
        # Inverse permutation (slot -> token id) built with matmuls instead of
        # 16 serialized indirect scatters: M[token, slot] = (rank == slot),
        # tok_list[slot] = sum_t M[t, slot] * t, with +BIG for empty slots.
        with tc.tile_pool(name="pe", bufs=1) as pe, \
             tc.tile_pool(name="pe2", bufs=3) as pe2, \
             tc.tile_pool(name="psE", bufs=2, space="PSUM") as psE, \
             tc.tile_pool(name="psE2", bufs=1, space="PSUM") as psE2:
            ut = pe.tile([P, P], F16)
            nc.sync.dma_start(ut[:], ex["ut_h"][:])
            slb = pe.tile([8, TB * P], F32R)
            nc.sync.dma_start(slb[:], ex["slb_in"][:].bitcast(F32R))
            s_iota = pe.tile([P, CAP], F32)
            nc.sync.dma_start(s_iota[:], ex["slot_iota"][:])
            tokid2 = pe.tile([P, 2 * TB], F16)
            nc.sync.dma_start(tokid2[:], ex["tokid2"][:])
            totals = pe.tile([8, E], F32R)
            pre_sb = [pe.tile([P, E], F32, tag=f"pre{b}", name=f"pre{b}")
                      for b in range(TB)]
            sel_all = [pe.tile([P, E], F16, tag=f"sela{b}", name=f"sela{b}")
                       for b in range(TB)]
            for b in range(TB):
                nc.sync.dma_start(
                    sel_all[b][:],
                    w_all[b * P:(b + 1) * P, E:2 * E])
                pr_ps = psE.tile([P, E], F32, tag="prps")
                nc.tensor.matmul(pr_ps[:], ut[:], sel_all[b][:],
                                 start=True, stop=True)
                nc.vector.tensor_copy(pre_sb[b][:], pr_ps[:])
                nc.sync.dma_start(totals[b:b + 1, :],
                                  pre_sb[b][127:128, :].bitcast(F32R))
            tl_ps = [[psE2.tile([P, 2], F32, tag=f"tl{ei}{ch}",
                                name=f"tl{ei}{ch}") for ch in range(2)]
                     for ei in range(2)]
            for b in range(TB):
                ofs_ps = psE.tile([P, E], F32, tag="ofsps", name="ofsps")
                nc.tensor.matmul(ofs_ps[:], slb[:, b * P:(b + 1) * P],
                                 totals[:], start=True, stop=True)
                grank = pe2.tile([P, E], F32, tag="grank")
                nc.vector.tensor_add(grank[:], pre_sb[b][:], ofs_ps[:])
                nc.vector.tensor_scalar_add(grank[:], grank[:], -1.0)
                gm = pe2.tile([P, E], F32, tag="gm")
                nc.vector.tensor_scalar(out=gm[:], in0=grank[:],
                                        scalar1=float(CAP - 1), scalar2=BIG,
                                        op0=ALU.is_gt, op1=ALU.mult)
                nc.vector.tensor_add(grank[:], grank[:], gm[:])
                um = pe2.tile([P, E], F32, tag="um")
                nc.vector.tensor_scalar(out=um[:], in0=sel_all[b][:],
                                        scalar1=-BIG, scalar2=BIG,
                                        op0=ALU.mult, op1=ALU.add)
                nc.vector.tensor_add(grank[:], grank[:], um[:])
                for ei in range(2):
                    ge = pe2.tile([P, E], F32, tag="ge")
                    nc.vector.tensor_mul(ge[:], grank[:],
                                         emask01[:, ei * E:(ei + 1) * E])
                    ridx = pe2.tile([P, 1], F32, tag="ridx")
                    nc.vector.reduce_sum(ridx[:], ge[:], axis=AX.X)
                    mb = pe2.tile([P, CAP], F16, tag="mb")
                    nc.vector.tensor_tensor(
                        out=mb[:], in0=s_iota[:],
                        in1=ridx[:, 0:1].to_broadcast([P, CAP]),
                        op=ALU.is_equal)
                    for ch in range(2):
                        nc.tensor.matmul(tl_ps[ei][ch][:],
                                         mb[:, ch * P:(ch + 1) * P],
                                         tokid2[:, 2 * b:2 * b + 2],
                                         start=(b == 0), stop=(b == TB - 1))
            for ei in range(2):
                for ch in range(2):
                    tl = pe2.tile([P, 2], F32, tag="tlsb")
                    nc.vector.tensor_copy(tl[:], tl_ps[ei][ch][:])
                    pad = pe2.tile([P, 1], F32, tag="pad")
                    nc.vector.tensor_scalar(out=pad[:], in0=tl[:, 1:2],
                                            scalar1=-BIG, scalar2=BIG,
                                            op0=ALU.mult, op1=ALU.add)
                    tok_f = pe2.tile([P, 1], F32, tag="tokf")
                    nc.vector.tensor_add(tok_f[:], tl[:, 0:1], pad[:])
                    nc.vector.tensor_copy(idx_sb2[ei][ch][:], tok_f[:])

        # ======== F: xT + shared expert + experts (fp16) ========
        with tc.tile_pool(name="pxt", bufs=1) as pxt, \
             tc.tile_pool(name="pfs", bufs=1) as pfs, \
             tc.tile_pool(name="pfs2", bufs=2) as pfs2:
            xc = [pxt.tile([P, T], F16, tag=f"xc{hc}", name=f"xc{hc}")
                  for hc in range(HC)]
            with tc.tile_pool(name="pxt2", bufs=3) as pxt2, \
                 tc.tile_pool(name="psX", bufs=2, space="PSUM") as psX:
                for b in range(TB):
                    xb = pxt2.tile([P, H], F16, tag="xb", bufs=2)
                    nc.sync.dma_start(xb[:], xw_all[b * P:(b + 1) * P, 0:H])
                    for hc in range(HC):
                        tp = psX.tile([P, P], F16, tag="tpX")
                        nc.tensor.transpose(tp[:], xb[:, hc * P:(hc + 1) * P],
                                            identh[:])
                        nc.vector.tensor_copy(xc[hc][:, b * P:(b + 1) * P],
                                              tp[:])

            # ---- both experts' setup: gathers, gxT, weights,
            # per-token gate weights — overlaps the shared expert below ----
            gxT2 = [pfs.tile([P, HC * 2 * P], F16, tag=f"gxT{ei}",
                             name=f"gxT{ei}") for ei in range(2)]
            wd_res2 = [[pfs.tile([P, H], F16, tag=f"wd{ei}{ip}",
                                 name=f"wd{ei}{ip}") for ip in range(IP)]
                       for ei in range(2)]
            wg_own2 = [[pfs.tile([P, 1], F32, tag=f"wgo{ei}{k}",
                                 name=f"wgo{ei}{k}") for k in range(2)]
                       for ei in range(2)]
            psS_cm = tc.tile_pool(name="psS", bufs=2, space="PSUM")
            psS = psS_cm.__enter__()
            for ei in range(2):
                for k in range(2):
                    # gather full rows: x plus the 32 W/sel columns ride along
                    gx = pfs2.tile([P, H + 2 * E], F16, tag="gx")
                    nc.vector.memset(gx[:], 0.0)
                    nc.gpsimd.indirect_dma_start(
                        out=gx[:], out_offset=None,
                        in_=xw_all[:],
                        in_offset=bass.IndirectOffsetOnAxis(
                            ap=idx_sb2[ei][k][:, :1], axis=0),
                        bounds_check=T - 1, oob_is_err=False)
                    for hc in range(HC):
                        tp = psS.tile([P, P], F16, tag="tpS")
                        nc.tensor.transpose(tp[:], gx[:, hc * P:(hc + 1) * P],
                                            identh[:])
                        nc.vector.tensor_copy(
                            gxT2[ei][:, hc * 2 * P + k * P:
                                  hc * 2 * P + (k + 1) * P], tp[:])
                    we_ = pfs2.tile([P, E], F32, tag="we_")
                    nc.vector.tensor_mul(we_[:], gx[:, H:H + E],
                                         emask01[:, ei * E:(ei + 1) * E])
                    nc.vector.reduce_sum(wg_own2[ei][k][:], we_[:], axis=AX.X)

            # ---- shared expert ----
            with tc.tile_pool(name="pg", bufs=1) as pg, \
                 tc.tile_pool(name="pg2", bufs=3) as pg2:
                g_act = [pg.tile([P, T], F16, tag=f"gact{sp}", name=f"gact{sp}")
                         for sp in range(SP)]
                hs = [pg.tile([P, T], F16, tag=f"hs{sp}", name=f"hs{sp}")
                      for sp in range(SP)]
                with tc.tile_pool(name="psG1", bufs=1, space="PSUM") as psG1:
                    g_ps = [psG1.tile([P, T], F32, tag=f"gps{sp}",
                                      name=f"gps{sp}") for sp in range(SP)]
                    for hc in range(HC):
                        for sp in range(SP):
                            c0 = hc * SP * P + sp * P
                            for n in range(2):
                                sl = slice(n * 512, (n + 1) * 512)
                                nc.tensor.matmul(g_ps[sp][:, sl],
                                                 wsg_sb[:, c0:c0 + P],
                                                 xc[hc][:, sl],
                                                 start=(hc == 0),
                                                 stop=(hc == HC - 1))
                    for sp in range(SP):
                        nc.scalar.activation(g_act[sp][:], g_ps[sp][:],
                                             AF.Silu)
                with tc.tile_pool(name="psG2", bufs=1, space="PSUM") as psG2:
                    u_ps = [psG2.tile([P, T], F32, tag=f"ups{sp}",
                                      name=f"ups{sp}") for sp in range(SP)]
                    for hc in range(HC):
                        for sp in range(SP):
                            c0 = hc * SP * P + sp * P
                            for n in range(2):
                                sl = slice(n * 512, (n + 1) * 512)
                                nc.tensor.matmul(u_ps[sp][:, sl],
                                                 wsu_sb[:, c0:c0 + P],
                                                 xc[hc][:, sl],
                                                 start=(hc == 0),
                                                 stop=(hc == HC - 1))
                    for sp in range(SP):
                        nc.vector.tensor_mul(hs[sp][:], g_act[sp][:],
                                             u_ps[sp][:])
                with tc.tile_pool(name="psG3", bufs=6, space="PSUM") as psG3:
                    for tb_ in range(TB):
                        psd = [psG3.tile([P, 512], F32, tag="psGd",
                                         name=f"psGd{n}") for n in range(4)]
                        for sp in range(SP):
                            for n in range(4):
                                nc.tensor.matmul(
                                    psd[n][:],
                                    hs[sp][:, tb_ * P:(tb_ + 1) * P],
                                    wsd_sb[sp][:, n * 512:(n + 1) * 512],
                                    start=(sp == 0), stop=(sp == SP - 1))
                        sbd = pg2.tile([P, H], F16, tag="sbGd", bufs=2)
                        for n in range(4):
                            nc.vector.tensor_copy(
                                sbd[:, n * 512:(n + 1) * 512], psd[n][:])
                        nc.sync.dma_start(rs2_in[tb_ * P:(tb_ + 1) * P, :],
                                          sbd[:])

            psS_cm.__exit__(None, None, None)

            # expert down-proj weights: emitted late so these 8 MB of DMAs
            # sit behind the x-block/gather traffic in queue priority, but
            # they still have ~100us of slack before first use
            for ei in range(2):
                for ip in range(IP):
                    nc.sync.dma_start(wd_res2[ei][ip][:],
                                      ex["we_d"][ei, ip * P:(ip + 1) * P, :])

            # ---- experts (setup already done above) ----
            for ei in range(2):
                with tc.tile_pool(name=f"pf{ei}", bufs=1) as pf, \
                     tc.tile_pool(name=f"pf2{ei}", bufs=2) as pf2:
                    idx_sb = idx_sb2[ei]
                    gxT = gxT2[ei]
                    wd_res = wd_res2[ei]

                    # merged gate+up pass (8 PSUM banks)
                    g_tm = [pf.tile([P, I], F16, tag=f"gtm{k}", name=f"gtm{k}")
                            for k in range(2)]
                    h_tm = [pf.tile([P, I], F16, tag=f"htm{k}", name=f"htm{k}")
                            for k in range(2)]
                    with tc.tile_pool(name=f"psF2{ei}", bufs=1,
                                      space="PSUM") as psF2:
                        gu_ps = [[psF2.tile([P, 512], F32, tag=f"gups{k}{j}",
                                            name=f"gups{k}{j}")
                                  for j in range(4)] for k in range(2)]
                        for hc in range(HC):
                            wg_c = pf2.tile([P, I], F16, tag="wgF", bufs=3)
                            nc.sync.dma_start(
                                wg_c[:], ex["we_g"][ei, hc * P:(hc + 1) * P, :])
                            wu_c = pf2.tile([P, I], F16, tag="wuF", bufs=3)
                            nc.sync.dma_start(
                                wu_c[:], ex["we_u"][ei, hc * P:(hc + 1) * P, :])
                            for k in range(2):
                                s_ = gxT[:, hc * 2 * P + k * P:
                                         hc * 2 * P + (k + 1) * P]
                                for n in range(2):
                                    nc.tensor.matmul(
                                        gu_ps[k][n][:], s_,
                                        wg_c[:, n * 512:(n + 1) * 512],
                                        start=(hc == 0), stop=(hc == HC - 1))
                                for n in range(2):
                                    nc.tensor.matmul(
                                        gu_ps[k][2 + n][:], s_,
                                        wu_c[:, n * 512:(n + 1) * 512],
                                        start=(hc == 0), stop=(hc == HC - 1))
                        for k in range(2):
                            for n in range(2):
                                sl = slice(n * 512, (n + 1) * 512)
                                nc.scalar.activation(g_tm[k][:, sl],
                                                     gu_ps[k][n][:], AF.Silu)
                                nc.vector.tensor_mul(h_tm[k][:, sl],
                                                     g_tm[k][:, sl],
                                                     gu_ps[k][2 + n][:])
                    h_sb = [pf.tile([P, 2 * P], F16, tag=f"hsb{ip}",
                                    name=f"hsb{ip}") for ip in range(IP)]
                    with tc.tile_pool(name=f"psF4{ei}", bufs=2,
                                      space="PSUM") as psF4:
                        for k in range(2):
                            for ip in range(IP):
                                tp = psF4.tile([P, P], F16, tag="tpF2")
                                nc.tensor.transpose(
                                    tp[:], h_tm[k][:, ip * P:(ip + 1) * P],
                                    identh[:])
                                nc.vector.tensor_copy(
                                    h_sb[ip][:, k * P:(k + 1) * P], tp[:])
                    with tc.tile_pool(name=f"psF5{ei}", bufs=8,
                                      space="PSUM") as psF5:
                        for k in range(2):
                            psd = [psF5.tile([P, 512], F32, tag="psFd",
                                             name=f"psFd{n}")
                                   for n in range(4)]
                            for ip in range(IP):
                                for n in range(4):
                                    nc.tensor.matmul(
                                        psd[n][:],
                                        h_sb[ip][:, k * P:(k + 1) * P],
                                        wd_res[ip][:, n * 512:(n + 1) * 512],
                                        start=(ip == 0), stop=(ip == IP - 1))
                            out_sb = pf.tile([P, H], F16, tag=f"outsb{k}")
                            for n in range(4):
                                nc.vector.tensor_scalar_mul(
                                    out_sb[:, n * 512:(n + 1) * 512],
                                    psd[n][:], wg_own2[ei][k][:, :1])
                            nc.gpsimd.indirect_dma_start(
                                out=rs2_in[:],
                                out_offset=bass.IndirectOffsetOnAxis(
                                    ap=idx_sb[k][:, :1], axis=0),
                                in_=out_sb[:], in_offset=None,
                                bounds_check=T - 1, oob_is_err=False,
                                compute_op=ALU.add)

        nc.gpsimd.collective_compute(
            "ReduceScatter", ALU.add, ins=[rs2_in.opt()], outs=[rs2_out.opt()],
            replica_groups=[list(range(NCN))])
        with tc.tile_pool(name="pz", bufs=2) as pz:
            fin16 = pz.tile([P, H], F16)
            nc.sync.dma_start(fin16[:], rs2_out[:])
            fin = pz.tile([P, H], F32)
            nc.vector.tensor_copy(fin[:], fin16[:])
            nc.sync.dma_start(out_slice[:], fin[:])


_CACHE = {}


def _build():
    key = "nc"
    if key in _CACHE:
        return _CACHE[key]
    nc = bacc.Bacc("TRN2", target_bir_lowering=False, debug=False,
                   num_devices=NCN)
    with tile.TileContext(nc) as tc:
        _emit(nc, tc)
    nc.compile()
    _CACHE[key] = nc
    return nc


def _host_prep(inputs):
    f16 = np.float16
    pos = np.asarray(inputs["positions"]).astype(np.float64)
    hid = np.asarray(inputs["hidden_states"], np.float32)
    w_in = np.asarray(inputs["w_in_ln"], np.float32)
    w_post = np.asarray(inputs["w_post_ln"], np.float32)
    wq = (np.asarray(inputs["wq"], np.float32) * w_in[:, None]).astype(f16)
    wk = (np.asarray(inputs["wk"], np.float32) * w_in[:, None]).astype(f16)
    wv = (np.asarray(inputs["wv"], np.float32) * w_in[:, None]).astype(f16)
    wo = np.asarray(inputs["wo"], np.float32).astype(f16)
    gate_w = np.asarray(inputs["gate_w"], np.float32) * w_post[None, :]
    gate_b = np.asarray(inputs["gate_bias"], np.float32).reshape(1, E)
    we_g = (np.asarray(inputs["we_gate"], np.float32)
            * w_post[None, :, None]).astype(f16)
    we_u = (np.asarray(inputs["we_up"], np.float32)
            * w_post[None, :, None]).astype(f16)
    we_d = np.asarray(inputs["we_down"], np.float32).astype(f16)
    ws_g = (np.asarray(inputs["ws_gate"], np.float32)
            * w_post[:, None]).astype(f16)
    ws_u = (np.asarray(inputs["ws_up"], np.float32)
            * w_post[:, None]).astype(f16)
    ws_d = np.asarray(inputs["ws_down"], np.float32).astype(f16)

    inv_freq = 1.0 / (THETA ** (np.arange(0, D, 2, dtype=np.float64) / D))
    f = pos[None, :] * inv_freq[:, None]
    cos2, sin2 = np.cos(f), np.sin(f)
    cosT = np.repeat(cos2, 2, axis=0).astype(np.float32)
    sinT = np.empty((D, T), np.float32)
    sinT[0::2] = -sin2
    sinT[1::2] = sin2
    s = 1.0 / np.sqrt(D)
    cosq, sinq = (cosT * s).astype(np.float32), (sinT * s).astype(np.float32)

    import ml_dtypes
    bf = ml_dtypes.bfloat16
    ii = np.arange(P)
    diag_mask = np.where(ii[:, None] >= ii[None, :], 0.0, NEG).astype(bf)

    identr_in = np.eye(P, dtype=np.float32)
    identh_in = np.eye(P, dtype=f16)
    ut_in = np.triu(np.ones((P, P), np.float32))
    slb_in = np.zeros((8, TB * P), np.float32)
    for b in range(TB):
        slb_in[:b, b * P:(b + 1) * P] = 1.0
    perm = np.zeros((P, P), np.float32)
    for i in range(0, P, 2):
        perm[i, i + 1] = 1.0
        perm[i + 1, i] = 1.0
    slot_iota = np.broadcast_to(np.arange(CAP, dtype=np.float32),
                                (P, CAP)).copy()
    tokid2 = np.zeros((P, 2 * TB), f16)
    for b in range(TB):
        tokid2[:, 2 * b] = (b * P + np.arange(P)).astype(f16)
        tokid2[:, 2 * b + 1] = 1.0

    ISC = IS // NCN
    maps = []
    for c in range(NCN):
        g = c // 2
        emask01 = np.zeros((P, 2 * E), np.float32)
        emask01[:, 2 * c] = 1.0          # ei = 0 -> expert 2c
        emask01[:, E + 2 * c + 1] = 1.0  # ei = 1 -> expert 2c+1
        maps.append({
            "hid": hid,
            "hid_slice": np.ascontiguousarray(hid[c * P:(c + 1) * P]),
            "wq_s": np.ascontiguousarray(wq[:, 2 * c * D:(2 * c + 2) * D]),
            "wk_s": np.ascontiguousarray(wk[:, g * D:(g + 1) * D]),
            "wv_s": np.ascontiguousarray(wv[:, g * D:(g + 1) * D]),
            "wo_s": np.ascontiguousarray(wo[2 * c * D:(2 * c + 2) * D, :]),
            "cosq": cosq, "sinq": sinq, "cosk": cosT, "sink": sinT,
            "perm": perm, "diag_mask": diag_mask,
            "identr_in": identr_in, "identh_in": identh_in,
            "ut_in": ut_in, "ut_h": ut_in.astype(f16), "slb_in": slb_in,
            "slot_iota": slot_iota, "tokid2": tokid2,
            "gate_wT": np.ascontiguousarray(gate_w.T),
            "gate_b": np.broadcast_to(gate_b, (P, E)).copy(),
            "emask01": emask01,
            "ws_g": np.ascontiguousarray(ws_g[:, c * ISC:(c + 1) * ISC]),
            "ws_u": np.ascontiguousarray(ws_u[:, c * ISC:(c + 1) * ISC]),
            "ws_d": np.ascontiguousarray(ws_d[c * ISC:(c + 1) * ISC, :]),
            "we_g": np.ascontiguousarray(we_g[2 * c:2 * c + 2]),
            "we_u": np.ascontiguousarray(we_u[2 * c:2 * c + 2]),
            "we_d": np.ascontiguousarray(we_d[2 * c:2 * c + 2]),
        })
    return maps


def kernel(trace=False, **inputs):
    nc = _build()
    maps = _host_prep(inputs)
    res = bass_utils.run_bass_kernel_spmd(
        nc, maps, core_ids=list(range(NCN)), trace=trace)
    out = np.concatenate([res.results[c]["out_slice"] for c in range(NCN)], 0)
    resid = np.concatenate([res.results[c]["res_slice"] for c in range(NCN)], 0)
    kernel.last_results = res
    return out, resid


# revision 35
# speedup vs baseline: 1.6609x; 1.0248x over previous
"""Ernie4 decoder layer (RMSNorm + GQA attention + shared expert + 16-expert
top-2 MoE) on 8 Trainium2 NeuronCores.

v2 — fp16 data path everywhere except the router (which must reproduce the
reference top-2 selection exactly; margins are ~3e-5 so it stays fp32 and is
computed locally per core before the AllGather):
  - Attention: head-parallel (2 q-heads + 1 kv-head per core), fp16 QKV /
    scores / probs / o_proj with causal-block skipping; fp16 ReduceScatter.
  - Router: fp32 logits on each core's own 128 tokens; W+sel AllGathered in a
    tiny fp32 collective that precedes the fp16 x AllGather so the token-list
    build overlaps it.
  - Shared expert: intermediate-sharded (IS/8 per core) fp16, output seeds
    the MoE combine buffer.
  - MoE: expert-parallel (2 experts per core), token lists via
    triangular-matmul prefix ranks, indirect-DMA gather/scatter-add in fp16,
    fp16 ReduceScatter for the combine.
"""
import sys
sys.path.insert(0, "/opt/trn_rl_repo")

import numpy as np

import concourse.bass as bass
import concourse.bacc as bacc
import concourse.tile as tile
import concourse.mybir as mybir
from concourse import bass_utils
from concourse.masks import make_identity
from concourse.tile import add_dep_helper

dt = mybir.dt
F32 = dt.float32
F32R = dt.float32r
F16 = dt.float16
I32 = dt.int32
BF16 = dt.bfloat16
AF = mybir.ActivationFunctionType
ALU = mybir.AluOpType
AX = mybir.AxisListType

T, H, NH, NKV, D = 1024, 2048, 16, 4, 128
E, I, IS = 16, 1024, 2048
EPS = 1e-6
THETA = 10000.0
NCN = 8
P = 128
TB = T // P            # 8 token blocks
HC = H // P            # 16 hidden chunks
IP = I // P            # 8 expert-intermediate chunks
SP = IS // NCN // P    # 2 shared-intermediate chunks per core
CAP = 256              # per-expert token capacity
BIG = 1.0e6            # OOB sentinel
NEG = -1e9


def _emit(nc, tc):
    ex = {}
    for name, shape, d in [
        ("hid", [T, H], F32), ("hid_slice", [P, H], F32),
        ("wq_s", [H, 2 * D], F16), ("wk_s", [H, D], F16), ("wv_s", [H, D], F16),
        ("wo_s", [2 * D, H], F16),
        ("cosq", [D, T], F32), ("sinq", [D, T], F32),
        ("cosk", [D, T], F32), ("sink", [D, T], F32),
        ("perm", [P, P], F32),
        ("diag_mask", [P, P], BF16),
        ("gate_wT", [H, E], F32), ("gate_b", [P, E], F32),
        ("emask01", [P, 2 * E], F32),
        ("ws_g", [H, SP * P], F16), ("ws_u", [H, SP * P], F16),
        ("ws_d", [SP * P, H], F16),
        ("we_g", [2, H, I], F16), ("we_u", [2, H, I], F16),
        ("we_d", [2, I, H], F16),
        ("identr_in", [P, P], F32), ("identh_in", [P, P], F16),
        ("ut_in", [P, P], F32), ("ut_h", [P, P], F16),
        ("slb_in", [8, TB * P], F32),
        ("slot_iota", [P, CAP], F32), ("tokid2", [P, 2 * TB], F16),
    ]:
        ex[name] = nc.dram_tensor(name, shape, d, kind="ExternalInput").ap()
    out_slice = nc.dram_tensor("out_slice", [P, H], F32, kind="ExternalOutput").ap()
    res_slice = nc.dram_tensor("res_slice", [P, H], F32, kind="ExternalOutput").ap()
    dbg_w = nc.dram_tensor("dbg_w", [P, E], F32, kind="ExternalOutput").ap()

    with tc.tile_pool(name="persist", bufs=1) as pp, \
         tc.tile_pool(name="dram", bufs=1, space="DRAM") as dram:
        rs_in = dram.tile([T, H], F16)
        rs_out = dram.tile([P, H], F16)
        agx_in = dram.tile([P, H + 2 * E], F16)
        xw_all = dram.tile([T, H + 2 * E], F16, addr_space="Shared")
        agw_in = dram.tile([P, 2 * E], F16)
        w_all = dram.tile([T, 2 * E], F16, addr_space="Shared")
        warm_in = dram.tile([8, 8], F16)
        warm_out = dram.tile([64, 8], F16, addr_space="Shared")
        rs2_in = dram.tile([T, H], F16)
        rs2_out = dram.tile([P, H], F16)

        ident = pp.tile([P, P], F32)
        make_identity(nc, ident[:])
        identr = pp.tile([P, P], F32R)
        nc.sync.dma_start(identr[:], ex["identr_in"][:].bitcast(F32R))
        identh = pp.tile([P, P], F16)
        nc.sync.dma_start(identh[:], ex["identh_in"][:])
        hid_sl = pp.tile([P, H], F32)
        nc.sync.dma_start(hid_sl[:], ex["hid_slice"][:])
        eps_t = pp.tile([P, 1], F32)
        nc.vector.memset(eps_t[:], EPS)
        emask01 = pp.tile([P, 2 * E], F32)
        nc.sync.dma_start(emask01[:], ex["emask01"][:])
        wz = pp.tile([8, 8], F16)
        nc.vector.memset(wz[:], 0.0)
        nc.sync.dma_start(warm_in[:], wz[:])
        nc.gpsimd.collective_compute(
            "AllGather", ALU.bypass, ins=[warm_in.opt()],
            outs=[warm_out.opt()], replica_groups=[list(range(NCN))])
        # per-expert token lists live in SBUF end-to-end (built by the
        # matmul-based inverse permutation in phase E, consumed in F)
        idx_sb2 = [[pp.tile([P, 1], I32, tag=f"idx{ei}{k}",
                            name=f"idx{ei}{k}") for k in range(2)]
                   for ei in range(2)]
        # shared-expert weights are pure inputs: load them from t=0 so the
        # post-AllGather phase never waits on weight DMAs
        wsg_sb = pp.tile([P, HC * SP * P], F16)
        wsu_sb = pp.tile([P, HC * SP * P], F16)
        for t_, s_ in [(wsg_sb, "ws_g"), (wsu_sb, "ws_u")]:
            nc.sync.dma_start(
                t_[:].rearrange("p (hc m) -> p hc m", hc=HC),
                ex[s_][:].rearrange("(hc p) m -> p hc m", p=P))
        wsd_sb = [pp.tile([P, H], F16, tag=f"wsd{sp}", name=f"wsd{sp}")
                  for sp in range(SP)]
        for sp in range(SP):
            nc.sync.dma_start(wsd_sb[sp][:],
                              ex["ws_d"][sp * P:(sp + 1) * P, :])

        # ======== Phases A-C: attention (fp16) ========
        with tc.tile_pool(name="pab", bufs=1) as pab:
            qT = [pab.tile([P, T], F16, tag=f"qT{j}", name=f"qT{j}")
                  for j in range(2)]
            kT = pab.tile([P, T], F16)
            vT = pab.tile([P, T], F16)
            v_tm = [pab.tile([P, D], F16, tag=f"vtm{b}", name=f"vtm{b}")
                    for b in range(TB)]
            oT = [pab.tile([P, T], F16, tag=f"oT{j}", name=f"oT{j}")
                  for j in range(2)]

            # ---- A: norm + transpose + QKV + rope ----
            with tc.tile_pool(name="pa", bufs=1) as pa, \
                 tc.tile_pool(name="pa2", bufs=3) as pa2:
                cosq = pa.tile([D, T], F32)
                sinq = pa.tile([D, T], F32)
                cosk = pa.tile([D, T], F32)
                sink = pa.tile([D, T], F32)
                for t_, s_ in [(cosq, "cosq"), (sinq, "sinq"),
                               (cosk, "cosk"), (sink, "sink")]:
                    nc.sync.dma_start(t_[:], ex[s_][:])
                permr = pa.tile([P, P], F32R)
                nc.sync.dma_start(permr[:], ex["perm"][:].bitcast(F32R))
                wq_sb = pa.tile([P, HC * 2 * D], F16)
                wk_sb = pa.tile([P, HC * D], F16)
                wv_sb = pa.tile([P, HC * D], F16)
                for t_, s_, m in [(wq_sb, "wq_s", 2 * D), (wk_sb, "wk_s", D),
                                  (wv_sb, "wv_s", D)]:
                    nc.sync.dma_start(
                        t_[:].rearrange("p (hc m) -> p hc m", hc=HC),
                        ex[s_][:].rearrange("(hc p) m -> p hc m", p=P))

                dump = pa.tile([P, H], F32)
                qraw = [pa.tile([P, T], F32R, tag=f"qraw{j}", name=f"qraw{j}")
                        for j in range(2)]
                kraw = pa.tile([P, T], F32R)
                with tc.tile_pool(name="psA1", bufs=3, space="PSUM") as psA1, \
                     tc.tile_pool(name="psA2", bufs=3, space="PSUM") as psA2:
                    for n in range(2):
                        x0T = [pa.tile([P, 512], F16, tag=f"x0T{hc}",
                                       name=f"x0T{hc}_{n}") for hc in range(HC)]
                        for bb in range(TB // 2):
                            b = n * (TB // 2) + bb
                            hidb = pa2.tile([P, H], F32, tag="hidb", bufs=2)
                            nc.sync.dma_start(hidb[:],
                                              ex["hid"][b * P:(b + 1) * P, :])
                            ssum = pa2.tile([P, 1], F32, tag="ssum")
                            nc.scalar.activation(dump[:], hidb[:], AF.Square,
                                                 accum_out=ssum[:, :1])
                            rms = pa2.tile([P, 1], F32, tag="rms")
                            nc.scalar.activation(rms[:], ssum[:],
                                                 AF.Sqrt, bias=eps_t[:, :1],
                                                 scale=1.0 / H)
                            inv = pa2.tile([P, 1], F32, tag="inv")
                            nc.vector.reciprocal(inv[:], rms[:])
                            x0b = pa2.tile([P, H], F16, tag="x0b", bufs=2)
                            nc.vector.tensor_scalar_mul(x0b[:], hidb[:],
                                                        inv[:, :1])
                            for hc in range(HC):
                                tp = psA1.tile([P, P], F16, tag="tpA")
                                nc.tensor.transpose(
                                    tp[:], x0b[:, hc * P:(hc + 1) * P],
                                    identh[:])
                                nc.vector.tensor_copy(
                                    x0T[hc][:, bb * P:(bb + 1) * P], tp[:])

                        def proj(w_sb, m, c0, dst, n=n, x0T=x0T, fp16=False):
                            ps = psA2.tile([P, 512], F32, tag="psQKV",
                                           name="psQKV")
                            for hc in range(HC):
                                nc.tensor.matmul(
                                    ps[:],
                                    w_sb[:, hc * m + c0:hc * m + c0 + P],
                                    x0T[hc][:],
                                    start=(hc == 0), stop=(hc == HC - 1))
                            nc.vector.tensor_copy(
                                dst[:, n * 512:(n + 1) * 512], ps[:])
                        proj(wq_sb, 2 * D, 0, qraw[0])
                        proj(wq_sb, 2 * D, D, qraw[1])
                        proj(wk_sb, D, 0, kraw)
                        proj(wv_sb, D, 0, vT, fp16=True)

                with tc.tile_pool(name="psA3", bufs=2, space="PSUM") as psA3:
                    for src, dst, c_, s_ in [(qraw[0], qT[0], cosq, sinq),
                                             (qraw[1], qT[1], cosq, sinq),
                                             (kraw, kT, cosk, sink)]:
                        for n in range(2):
                            sl = slice(n * 512, (n + 1) * 512)
                            sw = psA3.tile([P, 512], F32, tag="psSW")
                            nc.tensor.matmul(sw[:], permr[:], src[:, sl],
                                             start=True, stop=True)
                            t1 = pa2.tile([P, 512], F32, tag="ropeT1")
                            nc.vector.tensor_mul(t1[:], src[:, sl], c_[:, sl])
                            t2 = pa2.tile([P, 512], F32, tag="ropeT2")
                            nc.vector.tensor_mul(t2[:], sw[:], s_[:, sl])
                            nc.vector.tensor_add(dst[:, sl], t1[:], t2[:])
                    for b in range(TB):
                        tp = psA3.tile([P, P], F16, tag="tpV")
                        nc.tensor.transpose(tp[:], vT[:, b * P:(b + 1) * P],
                                            identh[:])
                        nc.vector.tensor_copy(v_tm[b][:], tp[:])

            # ---- B: attention (causal-block skipped) ----
            with tc.tile_pool(name="pb", bufs=1) as pb, \
                 tc.tile_pool(name="pb2", bufs=3) as pb2:
                dmask = pb.tile([P, P], BF16)
                nc.sync.dma_start(dmask[:], ex["diag_mask"][:])
                wo_sb = [pb.tile([P, H], F16, tag=f"wo{j}", name=f"wo{j}")
                         for j in range(2)]
                nc.sync.dma_start(wo_sb[0][:], ex["wo_s"][0:P, :])
                nc.sync.dma_start(wo_sb[1][:], ex["wo_s"][P:2 * P, :])

                attnT = [pb.tile([P, T], F16, tag=f"attnT{kc}",
                                 name=f"attnT{kc}") for kc in range(TB)]
                for kc in range(1, TB):
                    nc.vector.memset(attnT[kc][:, 0:kc * P], 0.0)
                with tc.tile_pool(name="psB1", bufs=3, space="PSUM") as psB1, \
                     tc.tile_pool(name="psB2", bufs=2, space="PSUM") as psB2, \
                     tc.tile_pool(name="psB3", bufs=3, space="PSUM") as psB3:
                  for h in range(2):
                    for qc in range(TB):
                        cols = (qc + 1) * P
                        prob = pb2.tile([P, T], F32, tag="prob")
                        nsl = (cols + 511) // 512
                        for n in range(nsl):
                            w_ = min(512, cols - n * 512)
                            ps = psB1.tile([P, 512], F32, tag="psSC")
                            nc.tensor.matmul(ps[:, :w_],
                                             qT[h][:, qc * P:(qc + 1) * P],
                                             kT[:, n * 512:n * 512 + w_],
                                             start=True, stop=True)
                            # diagonal block gets the causal mask; the rest
                            # of this slice is fully visible
                            d0 = qc * P - n * 512
                            if 0 <= d0 < w_:
                                if d0 > 0:
                                    nc.vector.tensor_copy(
                                        prob[:, n * 512:n * 512 + d0],
                                        ps[:, :d0])
                                nc.vector.tensor_add(
                                    prob[:, qc * P:qc * P + P],
                                    ps[:, d0:d0 + P], dmask[:])
                            else:
                                nc.vector.tensor_copy(
                                    prob[:, n * 512:n * 512 + w_], ps[:, :w_])
                        mx = pb2.tile([P, 1], F32, tag="mx")
                        nc.vector.reduce_max(mx[:], prob[:, :cols], axis=AX.X)
                        negm = pb2.tile([P, 1], F32, tag="negm")
                        nc.vector.tensor_scalar_mul(negm[:], mx[:], -1.0)
                        ssum = pb2.tile([P, 1], F32, tag="esum")
                        probe_ = pb2.tile([P, T], F32, tag="probe")
                        nc.scalar.activation(probe_[:, :cols], prob[:, :cols],
                                             AF.Exp, bias=negm[:, :1],
                                             accum_out=ssum[:, :1])
                        rec = pb2.tile([P, 1], F32, tag="rec")
                        nc.vector.reciprocal(rec[:], ssum[:])
                        probS = pb2.tile([P, T], F16, tag="probS")
                        nc.vector.tensor_scalar_mul(probS[:, :cols],
                                                    probe_[:, :cols],
                                                    rec[:, :1])
                        for kc in range(qc + 1):
                            tp = psB2.tile([P, P], F16, tag="tpB")
                            nc.tensor.transpose(
                                tp[:], probS[:, kc * P:(kc + 1) * P],
                                identh[:])
                            nc.vector.tensor_copy(
                                attnT[kc][:, qc * P:(qc + 1) * P], tp[:])
                    for n in range(2):
                        sl = slice(n * 512, (n + 1) * 512)
                        kc_hi = 4 * n + 3
                        ps = psB3.tile([P, 512], F32, tag="psAV")
                        for kc in range(kc_hi + 1):
                            nc.tensor.matmul(ps[:], v_tm[kc][:],
                                             attnT[kc][:, sl],
                                             start=(kc == 0),
                                             stop=(kc == kc_hi))
                        nc.vector.tensor_copy(oT[h][:, sl], ps[:])

                # ---- C: o_proj ----
                with tc.tile_pool(name="psC", bufs=8, space="PSUM") as psC:
                    for tb_ in range(TB):
                        pso = [psC.tile([P, 512], F32, tag="psO",
                                        name=f"psO{n}") for n in range(4)]
                        for hp in range(2):
                            for n in range(4):
                                nc.tensor.matmul(
                                    pso[n][:],
                                    oT[hp][:, tb_ * P:(tb_ + 1) * P],
                                    wo_sb[hp][:, n * 512:(n + 1) * 512],
                                    start=(hp == 0), stop=(hp == 1))
                        ob = pb2.tile([P, H], F16, tag="ob", bufs=2)
                        for n in range(4):
                            if n % 2 == 0:
                                nc.vector.tensor_copy(
                                    ob[:, n * 512:(n + 1) * 512], pso[n][:])
                            else:
                                nc.scalar.activation(
                                    ob[:, n * 512:(n + 1) * 512], pso[n][:],
                                    AF.Copy)
                        nc.sync.dma_start(rs_in[tb_ * P:(tb_ + 1) * P, :],
                                          ob[:])

        nc.gpsimd.collective_compute(
            "ReduceScatter", ALU.add, ins=[rs_in.opt()], outs=[rs_out.opt()],
            replica_groups=[list(range(NCN))])

        # ======== D: residual + norm + local fp32 router + AGs ========
        with tc.tile_pool(name="pd", bufs=1) as pd, \
             tc.tile_pool(name="pd2", bufs=2) as pd2, \
             tc.tile_pool(name="psD", bufs=2, space="PSUM") as psD:
            attn_sl = pd.tile([P, H], F16)
            nc.sync.dma_start(attn_sl[:], rs_out[:])
            res_sb = pd.tile([P, H], F32)
            dump2 = pd.tile([P, H], F32)
            ssum4 = pd.tile([P, 4], F32)
            for q in range(4):
                sl = slice(q * 512, (q + 1) * 512)
                nc.vector.tensor_add(res_sb[:, sl], hid_sl[:, sl],
                                     attn_sl[:, sl])
                nc.scalar.activation(dump2[:, sl], res_sb[:, sl], AF.Square,
                                     accum_out=ssum4[:, q:q + 1])
            nc.sync.dma_start(res_slice[:], res_sb[:])
            ssum2 = pd.tile([P, 2], F32)
            nc.vector.tensor_add(ssum2[:], ssum4[:, 0:2], ssum4[:, 2:4])
            ssum = pd.tile([P, 1], F32)
            nc.vector.tensor_add(ssum[:], ssum2[:, 0:1], ssum2[:, 1:2])
            rms = pd.tile([P, 1], F32)
            nc.scalar.activation(rms[:], ssum[:], AF.Sqrt, bias=eps_t[:, :1],
                                 scale=1.0 / H)
            inv = pd.tile([P, 1], F32)
            nc.vector.reciprocal(inv[:], rms[:])
            x_sl = pd.tile([P, H], F32)
            nc.vector.tensor_scalar_mul(x_sl[:], res_sb[:], inv[:, :1])
            x_sl_h = pd.tile([P, H], F16)
            nc.vector.tensor_copy(x_sl_h[:], x_sl[:])
            nc.sync.dma_start(agx_in[:, 0:H], x_sl_h[:])

            # exact fp32 router on the un-normalized residual: transposes and
            # logit matmuls run in parallel with the rmsnorm stats, and the
            # 1/rms scale folds into the sigmoid's per-token scale operand
            gw_sb = pd.tile([P, HC * E], F32)
            nc.sync.dma_start(
                gw_sb[:].rearrange("p (hc e) -> p hc e", hc=HC),
                ex["gate_wT"][:].rearrange("(hc p) e -> p hc e", p=P))
            gate_b = pd.tile([P, E], F32)
            nc.sync.dma_start(gate_b[:], ex["gate_b"][:])
            resT = pd.tile([P, H], F32)
            for hc in range(HC):
                tp = psD.tile([P, P], F32, tag="tpD")
                nc.tensor.transpose(tp[:], res_sb[:, hc * P:(hc + 1) * P],
                                    ident[:])
                nc.vector.tensor_copy(resT[:, hc * P:(hc + 1) * P], tp[:])
            lg_ps = psD.tile([P, E], F32, tag="lgps", name="lgps")
            for hc in range(HC):
                nc.tensor.matmul(lg_ps[:], resT[:, hc * P:(hc + 1) * P],
                                 gw_sb[:, hc * E:(hc + 1) * E],
                                 start=(hc == 0), stop=(hc == HC - 1))
            sig = pd2.tile([P, E], F32, tag="sig")
            nc.scalar.activation(sig[:], lg_ps[:], AF.Sigmoid,
                                 scale=inv[:, :1])
            sb_ = pd2.tile([P, E], F32, tag="sb_")
            nc.vector.tensor_add(sb_[:], sig[:], gate_b[:])
            mx = pd2.tile([P, 8], F32, tag="mx8")
            nc.vector.max(out=mx[:], in_=sb_[:])
            s1 = pd2.tile([P, E], F32, tag="s1")
            nc.vector.tensor_tensor(out=s1[:], in0=sb_[:],
                                    in1=mx[:, 0:1].to_broadcast([P, E]),
                                    op=ALU.is_equal)
            s2 = pd2.tile([P, E], F32, tag="s2")
            nc.vector.tensor_tensor(out=s2[:], in0=sb_[:],
                                    in1=mx[:, 1:2].to_broadcast([P, E]),
                                    op=ALU.is_equal)
            nc.vector.tensor_add(s1[:], s1[:], s2[:])
            sel_own = pd2.tile([P, E], F32, tag="sel_own")
            nc.vector.tensor_scalar_min(sel_own[:], s1[:], 1.0)
            wa = pd2.tile([P, E], F32, tag="wa")
            nc.vector.tensor_mul(wa[:], sel_own[:], sig[:])
            nrm = pd2.tile([P, 1], F32, tag="nrm")
            nc.vector.reduce_sum(nrm[:], wa[:], axis=AX.X)
            rec = pd2.tile([P, 1], F32, tag="recw")
            nc.vector.reciprocal(rec[:], nrm[:])
            w_tm = pd2.tile([P, E], F32, tag="wtm")
            nc.vector.tensor_scalar_mul(w_tm[:], wa[:], rec[:, :1])
            wsel_h = pd2.tile([P, 2 * E], F16, tag="wselh")
            nc.vector.tensor_copy(wsel_h[:, 0:E], w_tm[:])
            nc.vector.tensor_copy(wsel_h[:, E:2 * E], sel_own[:])
            nc.sync.dma_start(agx_in[:, H:H + 2 * E], wsel_h[:])
            nc.sync.dma_start(agw_in[:], wsel_h[:])
            nc.sync.dma_start(dbg_w[:], w_tm[:])

        cc_w = nc.gpsimd.collective_compute(
            "AllGather", ALU.bypass, ins=[agw_in.opt()], outs=[w_all.opt()],
            replica_groups=[list(range(NCN))])
        cc_x = nc.gpsimd.collective_compute(
            "AllGather", ALU.bypass, ins=[agx_in.opt()], outs=[xw_all.opt()],
            replica_groups=[list(range(NCN))])
        # tiny W/sel AllGather first: the token-list build overlaps the big
        # x AllGather (CC queue executes in trigger order)
        add_dep_helper(cc_x.ins, cc_w.ins, sync=True,
                       reason="AG_W before AG_x")

        # ======== E: token lists from AllGathered router decisions ========
        # Inverse permutation (slot -> token id) built with matmuls instead of
        # 16 serialized indirect scatters: M[token, slot] = (rank == slot),
        # tok_list[slot] = sum_t M[t, slot] * t, with +BIG for empty slots.
        with tc.tile_pool(name="pe", bufs=1) as pe, \
             tc.tile_pool(name="pe2", bufs=3) as pe2, \
             tc.tile_pool(name="psE", bufs=2, space="PSUM") as psE, \
             tc.tile_pool(name="psE2", bufs=1, space="PSUM") as psE2:
            ut = pe.tile([P, P], F16)
            nc.sync.dma_start(ut[:], ex["ut_h"][:])
            slb = pe.tile([8, TB * P], F32R)
            nc.sync.dma_start(slb[:], ex["slb_in"][:].bitcast(F32R))
            s_iota = pe.tile([P, CAP], F32)
            nc.sync.dma_start(s_iota[:], ex["slot_iota"][:])
            tokid2 = pe.tile([P, 2 * TB], F16)
            nc.sync.dma_start(tokid2[:], ex["tokid2"][:])
            totals = pe.tile([8, E], F32R)
            pre_sb = [pe.tile([P, E], F32, tag=f"pre{b}", name=f"pre{b}")
                      for b in range(TB)]
            sel_all = [pe.tile([P, E], F16, tag=f"sela{b}", name=f"sela{b}")
                       for b in range(TB)]
            for b in range(TB):
                nc.sync.dma_start(
                    sel_all[b][:],
                    w_all[b * P:(b + 1) * P, E:2 * E])
                pr_ps = psE.tile([P, E], F32, tag="prps")
                nc.tensor.matmul(pr_ps[:], ut[:], sel_all[b][:],
                                 start=True, stop=True)
                nc.vector.tensor_copy(pre_sb[b][:], pr_ps[:])
                nc.sync.dma_start(totals[b:b + 1, :],
                                  pre_sb[b][127:128, :].bitcast(F32R))
            tl_ps = [[psE2.tile([P, 2], F32, tag=f"tl{ei}{ch}",
                                name=f"tl{ei}{ch}") for ch in range(2)]
                     for ei in range(2)]
            for b in range(TB):
                ofs_ps = psE.tile([P, E], F32, tag="ofsps", name="ofsps")
                nc.tensor.matmul(ofs_ps[:], slb[:, b * P:(b + 1) * P],
                                 totals[:], start=True, stop=True)
                grank = pe2.tile([P, E], F32, tag="grank")
                nc.vector.tensor_add(grank[:], pre_sb[b][:], ofs_ps[:])
                nc.vector.tensor_scalar_add(grank[:], grank[:], -1.0)
                gm = pe2.tile([P, E], F32, tag="gm")
                nc.vector.tensor_scalar(out=gm[:], in0=grank[:],
                                        scalar1=float(CAP - 1), scalar2=BIG,
                                        op0=ALU.is_gt, op1=ALU.mult)
                nc.vector.tensor_add(grank[:], grank[:], gm[:])
                um = pe2.tile([P, E], F32, tag="um")
                nc.vector.tensor_scalar(out=um[:], in0=sel_all[b][:],
                                        scalar1=-BIG, scalar2=BIG,
                                        op0=ALU.mult, op1=ALU.add)
                nc.vector.tensor_add(grank[:], grank[:], um[:])
                for ei in range(2):
                    ge = pe2.tile([P, E], F32, tag="ge")
                    nc.vector.tensor_mul(ge[:], grank[:],
                                         emask01[:, ei * E:(ei + 1) * E])
                    ridx = pe2.tile([P, 1], F32, tag="ridx")
                    nc.vector.reduce_sum(ridx[:], ge[:], axis=AX.X)
                    mb = pe2.tile([P, CAP], F16, tag="mb")
                    nc.vector.tensor_tensor(
                        out=mb[:], in0=s_iota[:],
                        in1=ridx[:, 0:1].to_broadcast([P, CAP]),
                        op=ALU.is_equal)
                    for ch in range(2):
                        nc.tensor.matmul(tl_ps[ei][ch][:],
                                         mb[:, ch * P:(ch + 1) * P],
                                         tokid2[:, 2 * b:2 * b + 2],
                                         start=(b == 0), stop=(b == TB - 1))
            for ei in range(2):
                for ch in range(2):
                    tl = pe2.tile([P, 2], F32, tag="tlsb")
                    nc.vector.tensor_copy(tl[:], tl_ps[ei][ch][:])
                    pad = pe2.tile([P, 1], F32, tag="pad")
                    nc.vector.tensor_scalar(out=pad[:], in0=tl[:, 1:2],
                                            scalar1=-BIG, scalar2=BIG,
                                            op0=ALU.mult, op1=ALU.add)
                    tok_f = pe2.tile([P, 1], F32, tag="tokf")
                    nc.vector.tensor_add(tok_f[:], tl[:, 0:1], pad[:])
                    nc.vector.tensor_copy(idx_sb2[ei][ch][:], tok_f[:])

        # ======== F: xT + shared expert + experts (fp16) ========
        with tc.tile_pool(name="pxt", bufs=1) as pxt, \
             tc.tile_pool(name="pfs", bufs=1) as pfs, \
             tc.tile_pool(name="pfs2", bufs=2) as pfs2:
            xc = [pxt.tile([P, T], F16, tag=f"xc{hc}", name=f"xc{hc}")
                  for hc in range(HC)]
            with tc.tile_pool(name="pxt2", bufs=3) as pxt2, \
                 tc.tile_pool(name="psX", bufs=2, space="PSUM") as psX:
                for b in range(TB):
                    xb = pxt2.tile([P, H], F16, tag="xb", bufs=2)
                    nc.sync.dma_start(xb[:], xw_all[b * P:(b + 1) * P, 0:H])
                    for hc in range(HC):
                        tp = psX.tile([P, P], F16, tag="tpX")
                        nc.tensor.transpose(tp[:], xb[:, hc * P:(hc + 1) * P],
                                            identh[:])
                        nc.vector.tensor_copy(xc[hc][:, b * P:(b + 1) * P],
                                              tp[:])

            # ---- both experts' setup: gathers, gxT, weights,
            # per-token gate weights — overlaps the shared expert below ----
            gxT2 = [pfs.tile([P, HC * 2 * P], F16, tag=f"gxT{ei}",
                             name=f"gxT{ei}") for ei in range(2)]
            wd_res2 = [[pfs.tile([P, H], F16, tag=f"wd{ei}{ip}",
                                 name=f"wd{ei}{ip}") for ip in range(IP)]
                       for ei in range(2)]
            wg_own2 = [[pfs.tile([P, 1], F32, tag=f"wgo{ei}{k}",
                                 name=f"wgo{ei}{k}") for k in range(2)]
                       for ei in range(2)]
            psS_cm = tc.tile_pool(name="psS", bufs=2, space="PSUM")
            psS = psS_cm.__enter__()
            for ei in range(2):
                for k in range(2):
                    # gather full rows: x plus the 32 W/sel columns ride along
                    gx = pfs2.tile([P, H + 2 * E], F16, tag="gx")
                    nc.vector.memset(gx[:], 0.0)
                    nc.gpsimd.indirect_dma_start(
                        out=gx[:], out_offset=None,
                        in_=xw_all[:],
                        in_offset=bass.IndirectOffsetOnAxis(
                            ap=idx_sb2[ei][k][:, :1], axis=0),
                        bounds_check=T - 1, oob_is_err=False)
                    for hc in range(HC):
                        tp = psS.tile([P, P], F16, tag="tpS")
                        nc.tensor.transpose(tp[:], gx[:, hc * P:(hc + 1) * P],
                                            identh[:])
                        nc.vector.tensor_copy(
                            gxT2[ei][:, hc * 2 * P + k * P:
                                  hc * 2 * P + (k + 1) * P], tp[:])
                    we_ = pfs2.tile([P, E], F32, tag="we_")
                    nc.vector.tensor_mul(we_[:], gx[:, H:H + E],
                                         emask01[:, ei * E:(ei + 1) * E])
                    nc.vector.reduce_sum(wg_own2[ei][k][:], we_[:], axis=AX.X)

            # ---- shared expert ----
            with tc.tile_pool(name="pg", bufs=1) as pg, \
                 tc.tile_pool(name="pg2", bufs=3) as pg2:
                g_act = [pg.tile([P, T], F16, tag=f"gact{sp}", name=f"gact{sp}")
                         for sp in range(SP)]
                hs = [pg.tile([P, T], F16, tag=f"hs{sp}", name=f"hs{sp}")
                      for sp in range(SP)]
                with tc.tile_pool(name="psG1", bufs=1, space="PSUM") as psG1:
                    g_ps = [psG1.tile([P, T], F32, tag=f"gps{sp}",
                                      name=f"gps{sp}") for sp in range(SP)]
                    for hc in range(HC):
                        for sp in range(SP):
                            c0 = hc * SP * P + sp * P
                            for n in range(2):
                                sl = slice(n * 512, (n + 1) * 512)
                                nc.tensor.matmul(g_ps[sp][:, sl],
                                                 wsg_sb[:, c0:c0 + P],
                                                 xc[hc][:, sl],
                                                 start=(hc == 0),
                                                 stop=(hc == HC - 1))
                    for sp in range(SP):
                        nc.scalar.activation(g_act[sp][:], g_ps[sp][:],
                                             AF.Silu)
                with tc.tile_pool(name="psG2", bufs=1, space="PSUM") as psG2:
                    u_ps = [psG2.tile([P, T], F32, tag=f"ups{sp}",
                                      name=f"ups{sp}") for sp in range(SP)]
                    for hc in range(HC):
                        for sp in range(SP):
                            c0 = hc * SP * P + sp * P
                            for n in range(2):
                                sl = slice(n * 512, (n + 1) * 512)
                                nc.tensor.matmul(u_ps[sp][:, sl],
                                                 wsu_sb[:, c0:c0 + P],
                                                 xc[hc][:, sl],
                                                 start=(hc == 0),
                                                 stop=(hc == HC - 1))
                    for sp in range(SP):
                        nc.vector.tensor_mul(hs[sp][:], g_act[sp][:],
                                             u_ps[sp][:])
                with tc.tile_pool(name="psG3", bufs=6, space="PSUM") as psG3:
                    for tb_ in range(TB):
                        psd = [psG3.tile([P, 512], F32, tag="psGd",
                                         name=f"psGd{n}") for n in range(4)]
                        for sp in range(SP):
                            for n in range(4):
                                nc.tensor.matmul(
                                    psd[n][:],
                                    hs[sp][:, tb_ * P:(tb_ + 1) * P],
                                    wsd_sb[sp][:, n * 512:(n + 1) * 512],
                                    start=(sp == 0), stop=(sp == SP - 1))
                        sbd = pg2.tile([P, H], F16, tag="sbGd", bufs=2)
                        for n in range(4):
                            if n % 2 == 0:
                                nc.vector.tensor_copy(
                                    sbd[:, n * 512:(n + 1) * 512], psd[n][:])
                            else:
                                nc.scalar.activation(
                                    sbd[:, n * 512:(n + 1) * 512], psd[n][:],
                                    AF.Copy)
                        nc.sync.dma_start(rs2_in[tb_ * P:(tb_ + 1) * P, :],
                                          sbd[:])

            psS_cm.__exit__(None, None, None)

            # expert down-proj weights: emitted late so these 8 MB of DMAs
            # sit behind the x-block/gather traffic in queue priority, but
            # they still have ~100us of slack before first use
            for ei in range(2):
                for ip in range(IP):
                    nc.sync.dma_start(wd_res2[ei][ip][:],
                                      ex["we_d"][ei, ip * P:(ip + 1) * P, :])

            # ---- experts (setup already done above) ----
            for ei in range(2):
                with tc.tile_pool(name=f"pf{ei}", bufs=1) as pf, \
                     tc.tile_pool(name=f"pf2{ei}", bufs=2) as pf2:
                    idx_sb = idx_sb2[ei]
                    gxT = gxT2[ei]
                    wd_res = wd_res2[ei]

                    # merged gate+up pass (8 PSUM banks)
                    g_tm = [pf.tile([P, I], F16, tag=f"gtm{k}", name=f"gtm{k}")
                            for k in range(2)]
                    h_tm = [pf.tile([P, I], F16, tag=f"htm{k}", name=f"htm{k}")
                            for k in range(2)]
                    with tc.tile_pool(name=f"psF2{ei}", bufs=1,
                                      space="PSUM") as psF2:
                        gu_ps = [[psF2.tile([P, 512], F32, tag=f"gups{k}{j}",
                                            name=f"gups{k}{j}")
                                  for j in range(4)] for k in range(2)]
                        for hc in range(HC):
                            wg_c = pf2.tile([P, I], F16, tag="wgF", bufs=4)
                            nc.sync.dma_start(
                                wg_c[:], ex["we_g"][ei, hc * P:(hc + 1) * P, :])
                            wu_c = pf2.tile([P, I], F16, tag="wuF", bufs=4)
                            nc.sync.dma_start(
                                wu_c[:], ex["we_u"][ei, hc * P:(hc + 1) * P, :])
                            for k in range(2):
                                s_ = gxT[:, hc * 2 * P + k * P:
                                         hc * 2 * P + (k + 1) * P]
                                for n in range(2):
                                    nc.tensor.matmul(
                                        gu_ps[k][n][:], s_,
                                        wg_c[:, n * 512:(n + 1) * 512],
                                        start=(hc == 0), stop=(hc == HC - 1))
                                for n in range(2):
                                    nc.tensor.matmul(
                                        gu_ps[k][2 + n][:], s_,
                                        wu_c[:, n * 512:(n + 1) * 512],
                                        start=(hc == 0), stop=(hc == HC - 1))
                        for k in range(2):
                            for n in range(2):
                                sl = slice(n * 512, (n + 1) * 512)
                                nc.scalar.activation(g_tm[k][:, sl],
                                                     gu_ps[k][n][:], AF.Silu)
                                nc.vector.tensor_mul(h_tm[k][:, sl],
                                                     g_tm[k][:, sl],
                                                     gu_ps[k][2 + n][:])
                    h_sb = [pf.tile([P, 2 * P], F16, tag=f"hsb{ip}",
                                    name=f"hsb{ip}") for ip in range(IP)]
                    with tc.tile_pool(name=f"psF4{ei}", bufs=2,
                                      space="PSUM") as psF4:
                        for k in range(2):
                            for ip in range(IP):
                                tp = psF4.tile([P, P], F16, tag="tpF2")
                                nc.tensor.transpose(
                                    tp[:], h_tm[k][:, ip * P:(ip + 1) * P],
                                    identh[:])
                                nc.vector.tensor_copy(
                                    h_sb[ip][:, k * P:(k + 1) * P], tp[:])
                    with tc.tile_pool(name=f"psF5{ei}", bufs=8,
                                      space="PSUM") as psF5:
                        for k in range(2):
                            psd = [psF5.tile([P, 512], F32, tag="psFd",
                                             name=f"psFd{n}")
                                   for n in range(4)]
                            for ip in range(IP):
                                for n in range(4):
                                    nc.tensor.matmul(
                                        psd[n][:],
                                        h_sb[ip][:, k * P:(k + 1) * P],
                                        wd_res[ip][:, n * 512:(n + 1) * 512],
                                        start=(ip == 0), stop=(ip == IP - 1))
                            out_sb = pf.tile([P, H], F16, tag=f"outsb{k}")
                            for n in range(4):
                                nc.vector.tensor_scalar_mul(
                                    out_sb[:, n * 512:(n + 1) * 512],
                                    psd[n][:], wg_own2[ei][k][:, :1])
                            nc.gpsimd.indirect_dma_start(
                                out=rs2_in[:],
                                out_offset=bass.IndirectOffsetOnAxis(
                                    ap=idx_sb[k][:, :1], axis=0),
                                in_=out_sb[:], in_offset=None,
                                bounds_check=T - 1, oob_is_err=False,
                                compute_op=ALU.add)

        nc.gpsimd.collective_compute(
            "ReduceScatter", ALU.add, ins=[rs2_in.opt()], outs=[rs2_out.opt()],
            replica_groups=[list(range(NCN))])
        with tc.tile_pool(name="pz", bufs=2) as pz:
            fin16 = pz.tile([P, H], F16)
            nc.sync.dma_start(fin16[:], rs2_out[:])
            fin = pz.tile([P, H], F32)
            nc.vector.tensor_copy(fin[:], fin16[:])
            nc.sync.dma_start(out_slice[:], fin[:])


_CACHE = {}


def _build():
    key = "nc"
    if key in _CACHE:
        return _CACHE[key]
    nc = bacc.Bacc("TRN2", target_bir_lowering=False, debug=False,
                   num_devices=NCN)
    with tile.TileContext(nc) as tc:
        _emit(nc, tc)
    nc.compile()
    _CACHE[key] = nc
    return nc


def _host_prep(inputs):
    f16 = np.float16
    pos = np.asarray(inputs["positions"]).astype(np.float64)
    hid = np.asarray(inputs["hidden_states"], np.float32)
    w_in = np.asarray(inputs["w_in_ln"], np.float32)
    w_post = np.asarray(inputs["w_post_ln"], np.float32)
    wq = (np.asarray(inputs["wq"], np.float32) * w_in[:, None]).astype(f16)
    wk = (np.asarray(inputs["wk"], np.float32) * w_in[:, None]).astype(f16)
    wv = (np.asarray(inputs["wv"], np.float32) * w_in[:, None]).astype(f16)
    wo = np.asarray(inputs["wo"], np.float32).astype(f16)
    gate_w = np.asarray(inputs["gate_w"], np.float32) * w_post[None, :]
    gate_b = np.asarray(inputs["gate_bias"], np.float32).reshape(1, E)
    we_g = (np.asarray(inputs["we_gate"], np.float32)
            * w_post[None, :, None]).astype(f16)
    we_u = (np.asarray(inputs["we_up"], np.float32)
            * w_post[None, :, None]).astype(f16)
    we_d = np.asarray(inputs["we_down"], np.float32).astype(f16)
    ws_g = (np.asarray(inputs["ws_gate"], np.float32)
            * w_post[:, None]).astype(f16)
    ws_u = (np.asarray(inputs["ws_up"], np.float32)
            * w_post[:, None]).astype(f16)
    ws_d = np.asarray(inputs["ws_down"], np.float32).astype(f16)

    inv_freq = 1.0 / (THETA ** (np.arange(0, D, 2, dtype=np.float64) / D))
    f = pos[None, :] * inv_freq[:, None]
    cos2, sin2 = np.cos(f), np.sin(f)
    cosT = np.repeat(cos2, 2, axis=0).astype(np.float32)
    sinT = np.empty((D, T), np.float32)
    sinT[0::2] = -sin2
    sinT[1::2] = sin2
    s = 1.0 / np.sqrt(D)
    cosq, sinq = (cosT * s).astype(np.float32), (sinT * s).astype(np.float32)

    import ml_dtypes
    bf = ml_dtypes.bfloat16
    ii = np.arange(P)
    diag_mask = np.where(ii[:, None] >= ii[None, :], 0.0, NEG).astype(bf)

    identr_in = np.eye(P, dtype=np.float32)
    identh_in = np.eye(P, dtype=f16)
    ut_in = np.triu(np.ones((P, P), np.float32))
    slb_in = np.zeros((8, TB * P), np.float32)
    for b in range(TB):
        slb_in[:b, b * P:(b + 1) * P] = 1.0
    perm = np.zeros((P, P), np.float32)
    for i in range(0, P, 2):
        perm[i, i + 1] = 1.0
        perm[i + 1, i] = 1.0
    slot_iota = np.broadcast_to(np.arange(CAP, dtype=np.float32),
                                (P, CAP)).copy()
    tokid2 = np.zeros((P, 2 * TB), f16)
    for b in range(TB):
        tokid2[:, 2 * b] = (b * P + np.arange(P)).astype(f16)
        tokid2[:, 2 * b + 1] = 1.0

    ISC = IS // NCN
    maps = []
    for c in range(NCN):
        g = c // 2
        emask01 = np.zeros((P, 2 * E), np.float32)
        emask01[:, 2 * c] = 1.0          # ei = 0 -> expert 2c
        emask01[:, E + 2 * c + 1] = 1.0  # ei = 1 -> expert 2c+1
        maps.append({
            "hid": hid,
            "hid_slice": np.ascontiguousarray(hid[c * P:(c + 1) * P]),
            "wq_s": np.ascontiguousarray(wq[:, 2 * c * D:(2 * c + 2) * D]),
            "wk_s": np.ascontiguousarray(wk[:, g * D:(g + 1) * D]),
            "wv_s": np.ascontiguousarray(wv[:, g * D:(g + 1) * D]),
            "wo_s": np.ascontiguousarray(wo[2 * c * D:(2 * c + 2) * D, :]),
            "cosq": cosq, "sinq": sinq, "cosk": cosT, "sink": sinT,
            "perm": perm, "diag_mask": diag_mask,
            "identr_in": identr_in, "identh_in": identh_in,
            "ut_in": ut_in, "ut_h": ut_in.astype(f16), "slb_in": slb_in,
            "slot_iota": slot_iota, "tokid2": tokid2,
            "gate_wT": np.ascontiguousarray(gate_w.T),
            "gate_b": np.broadcast_to(gate_b, (P, E)).copy(),
            "emask01": emask01,
            "ws_g": np.ascontiguousarray(ws_g[:, c * ISC:(c + 1) * ISC]),
            "ws_u": np.ascontiguousarray(ws_u[:, c * ISC:(c + 1) * ISC]),
            "ws_d": np.ascontiguousarray(ws_d[c * ISC:(c + 1) * ISC, :]),
            "we_g": np.ascontiguousarray(we_g[2 * c:2 * c + 2]),
            "we_u": np.ascontiguousarray(we_u[2 * c:2 * c + 2]),
            "we_d": np.ascontiguousarray(we_d[2 * c:2 * c + 2]),
        })
    return maps


def kernel(trace=False, **inputs):
    nc = _build()
    maps = _host_prep(inputs)
    res = bass_utils.run_bass_kernel_spmd(
        nc, maps, core_ids=list(range(NCN)), trace=trace)
    out = np.concatenate([res.results[c]["out_slice"] for c in range(NCN)], 0)
    resid = np.concatenate([res.results[c]["res_slice"] for c in range(NCN)], 0)
    kernel.last_results = res
    return out, resid
